# revision 43
# baseline (speedup 1.0000x reference)
"""AtomicDipolesMACE on 8 TRN2 NeuronCores.

Sharding: nodes are split into 8 contiguous ranges of 1250 (padded to 1280 =
10 blocks of 128). Each core owns the edges whose receiver falls in its range,
sorted and grouped by 128-node receiver block, each block segment padded to a
uniform T_BLK tiles of 128 edges. Scatters become per-block one-hot matmuls
(PSUM f32 accumulation, Yv folded into scaled one-hots); the layer-2 sender
gather is a dma_gather from an AllGathered bf16 node-feature table.
"""
import sys
sys.path.insert(0, "/opt/trn_rl_repo")
import numpy as np
import ml_dtypes

from concourse import bass, bacc, tile, mybir
from concourse.bass_utils import run_bass_kernel_spmd

f32 = mybir.dt.float32
bf16 = mybir.dt.bfloat16
i16 = mybir.dt.int16
i32 = mybir.dt.int32
AF = mybir.ActivationFunctionType
OP = mybir.AluOpType
nbf16 = ml_dtypes.bfloat16
USE_SILU = True
PACK_MLP = True
TSUB = 5

N, E, C, NE, G, NB = 10000, 160000, 128, 10, 16, 8
RMAX, AVG, SQ3 = 5.0, 16.0, 3.0 ** 0.5
NCORES = 8
NPC = N // NCORES          # 1250 real nodes per core
NBLK = 10                  # node blocks per core
NLOC = NBLK * 128          # 1280 padded local nodes
C4 = 4 * C                 # 512: table row / message width

WSHAPE = dict(W_embT=[C, NE], W_up1=[C, C], Wr1_1=[NB, 64], Wr1_2=[64, 64],
              Wr1_3=[64, 64], Wr1_o=[64, 2 * C], Wr2_1=[NB, 64],
              Wr2_2=[64, 64], Wr2_3=[64, 64], Wr2_o=[64, 4 * C],
              Wlin1_s=[C, C], Wlin1_v=[C, C], Wlin2_s=[C, C],
              Wlin2_v=[C, C], Lp1_sT=[C, C], Lp1_vT=[C, C], Lp1_v=[C, C],
              Lp2_v=[C, C], Wup2_s=[C, C], Wup2_v=[C, C],
              P1s1=[NE, C], P1ss=[NE, C], P1vv=[NE, C], P1v1=[NE, C],
              P1sv=[NE, C], P2v1=[NE, C], P2sv=[NE, C],
              R1c=[C, 1], RmidT=[16, C], Routc=[16, 1], kpi5=[128, NB])


# ---------------------------------------------------------------- host prep
def prep_shards(inp):
    snd = np.asarray(inp["edge_index"][0], dtype=np.int64)
    rcv = np.asarray(inp["edge_index"][1], dtype=np.int64)
    pos = np.asarray(inp["positions"], dtype=np.float32)
    na = np.asarray(inp["node_attrs"], dtype=np.float32)
    shf = np.asarray(inp["shifts"], dtype=np.float32)
    chg = np.asarray(inp["charges"], dtype=np.float32)
    bat = np.asarray(inp["batch"], dtype=np.int64)

    core = rcv // NPC
    loc = rcv - core * NPC
    blk = loc // 128

    order = np.lexsort((blk, core))
    snd_s, rcv_s, loc_s = snd[order], rcv[order], loc[order]
    core_s, blk_s = core[order], blk[order]
    shf_s = shf[order]

    counts = np.zeros((NCORES, NBLK), dtype=np.int64)
    np.add.at(counts, (core_s, blk_s), 1)
    t_blk = max(1, int(np.max((counts + 127) // 128)))
    eb = t_blk * 128
    epad = NBLK * eb
    tt = NBLK * t_blk

    snd_pad = (snd_s // NPC) * NLOC + (snd_s % NPC)  # padded-table row index

    starts = np.zeros(NCORES * NBLK, dtype=np.int64)
    starts[1:] = np.cumsum(counts.reshape(-1))[:-1]
    starts = starts.reshape(NCORES, NBLK)

    w = weights_prep(inp)
    in_maps = []
    for k in range(NCORES):
        pos_s = np.zeros((epad, 3), np.float32)
        pos_r = np.ones((epad, 3), np.float32)
        shfe = np.zeros((epad, 3), np.float32)
        attrsT = np.zeros((NE, epad), np.float32)
        sndp = np.zeros(epad, np.int64)
        rcvb = -np.ones(epad, np.float32)
        for b in range(NBLK):
            s0, n = starts[k, b], counts[k, b]
            sl = slice(s0, s0 + n)
            d0 = b * eb
            pos_s[d0:d0 + n] = pos[snd_s[sl]]
            pos_r[d0:d0 + n] = pos[rcv_s[sl]]
            shfe[d0:d0 + n] = shf_s[sl]
            attrsT[:, d0:d0 + n] = na[snd_s[sl]].T
            sndp[d0:d0 + n] = snd_pad[sl]
            rcvb[d0:d0 + n] = (loc_s[sl] - b * 128).astype(np.float32)

        def emaj(a):  # [epad, d] -> [128, tt, d]  (edge i -> (i%128, i//128))
            return np.ascontiguousarray(a.reshape(tt, 128, -1).transpose(1, 0, 2))

        sndw = np.zeros((128, epad // 16), np.int16)
        for b in range(NBLK):
            seg = sndp[b * eb:(b + 1) * eb].reshape(eb // 16, 16).T
            sndw[:, b * (eb // 16):(b + 1) * (eb // 16)] = np.tile(seg, (8, 1))

        nl0 = k * NPC
        na_nm = np.zeros((NLOC, NE), np.float32)
        na_nm[:NPC] = na[nl0:nl0 + NPC]
        pos_nm = np.zeros((NLOC, 3), np.float32)
        pos_nm[:NPC] = pos[nl0:nl0 + NPC]
        chg_nm = np.zeros((NLOC, 1), np.float32)
        chg_nm[:NPC, 0] = chg[nl0:nl0 + NPC]
        boh = np.zeros((NLOC, G), np.float32)
        boh[np.arange(NPC), bat[nl0:nl0 + NPC]] = 1.0

        m = dict(
            pos_s=emaj(pos_s), pos_r=emaj(pos_r), shfe=emaj(shfe),
            attrsT=attrsT.astype(nbf16), sndw=sndw,
            rcvb=emaj(rcvb)[:, :, 0].astype(nbf16),
            na_nm=na_nm, naT=np.ascontiguousarray(na_nm.T).astype(nbf16),
            pos_nm=pos_nm, chg_nm=chg_nm, boh=boh,
        )
        m.update(w)
        in_maps.append(m)
    return in_maps, t_blk


def weights_prep(inp):
    g = lambda k: np.ascontiguousarray(np.asarray(inp[k], dtype=np.float32))
    kvec = (np.pi / RMAX) * np.arange(1, NB + 1, dtype=np.float32)
    return dict(
        W_embT=g("W_emb").T.copy(), W_up1=g("W_up1"),
        Wr1_1=g("Wr1_1"), Wr1_2=g("Wr1_2"), Wr1_3=g("Wr1_3"), Wr1_o=g("Wr1_o"),
        Wr2_1=g("Wr2_1"), Wr2_2=g("Wr2_2"), Wr2_3=g("Wr2_3"), Wr2_o=g("Wr2_o"),
        Wlin1_s=g("Wlin1_s"), Wlin1_v=g("Wlin1_v"),
        Wlin2_s=g("Wlin2_s"), Wlin2_v=g("Wlin2_v"),
        Lp1_sT=g("Lp1_s").T.copy(), Lp1_vT=g("Lp1_v").T.copy(),
        Lp1_v=g("Lp1_v"), Lp2_v=g("Lp2_v"),
        Wup2_s=g("Wup2_s"), Wup2_v=g("Wup2_v"),
        P1s1=g("P1_s1"), P1ss=g("P1_ss"), P1vv=g("P1_vv"),
        P1v1=g("P1_v1"), P1sv=g("P1_sv"), P2v1=g("P2_v1"), P2sv=g("P2_sv"),
        Wsk=g("Wsk"),
        R1c=g("R1").reshape(C, 1), RmidT=g("Rmid").T.copy(),
        Routc=g("Rout").reshape(16, 1),
        kpi5=np.tile(kvec, (128, 1)),
    )


# ---------------------------------------------------------------- builder
def build(t_blk, debug=False):
    eb = t_blk * 128
    tt = NBLK * t_blk
    epad = NBLK * eb
    nc = bacc.Bacc(None, target_bir_lowering=False, num_devices=NCORES)

    def din(name, shape, dt=f32):
        return nc.declare_dram_parameter(name, shape, dt, isOutput=False)

    pos_s = din("pos_s", [128, tt, 3]); pos_r = din("pos_r", [128, tt, 3])
    shfe = din("shfe", [128, tt, 3])
    attrsT_in = din("attrsT", [NE, epad], bf16)
    sndw_in = din("sndw", [128, epad // 16], i16)
    rcvb_in = din("rcvb", [128, tt], bf16)
    na_in = din("na_nm", [NLOC, NE]); naT_in = din("naT", [NE, NLOC], bf16)
    posn_in = din("pos_nm", [NLOC, 3]); chg_in = din("chg_nm", [NLOC, 1])
    boh_in = din("boh", [NLOC, G])
    win = {n: din(n, WSHAPE[n]) for n in WSHAPE}
    wsk_in = din("Wsk", [NE, C, C])
    out_dip = nc.declare_dram_parameter("dip", [NLOC, 3], f32, isOutput=True)
    out_gs = nc.declare_dram_parameter("gsum", [16, 3], f32, isOutput=True)
    dbg = {}
    if debug:
        for nm, shp in [("dbg_a1", [128, NBLK, C4]), ("dbg_tab", [128, NBLK, C4]),
                        ("dbg_a2", [128, NBLK, C4]), ("dbg_hvT0", [C, NLOC]),
                        ("dbg_Bs", [C, NLOC]), ("dbg_h2T0", [C, NLOC]),
                        ("dbg_dipf0", [1, NLOC]), ("dbg_gat", [128, t_blk, C4]),
                        ("dbg_m4", [128, t_blk, 4, C]),
                        ("dbg_ft", [NB, epad]), ("dbg_uv", [128, t_blk, 2, C]),
                        ("dbg_S", [128, t_blk, 128]),
                        ("dbg_fts", [128, t_blk, NB]),
                        ("dbg_ln", [128, t_blk]), ("dbg_arg", [128, t_blk, NB]),
                        ("dbg_snb", [128, t_blk, NB]),
                        ("dbg_bes", [128, t_blk, NB]),
                        ("dbg_fc", [128, t_blk])]:
            dbg[nm] = nc.declare_dram_parameter(nm, shp, f32, isOutput=True)

    nchunks = []
    _o = 0
    while _o < NLOC:
        nchunks.append((_o, min(512, NLOC - _o)))
        _o += nchunks[-1][1]

    def echunks(ebs):
        o, out = 0, []
        while o < ebs:
            w_ = min(512, ebs - o)
            out.append((o, w_))
            o += w_
        return out

    with tile.TileContext(nc) as tc:
        wp = tc.alloc_tile_pool(name="wp", bufs=1)
        dramp = tc.alloc_tile_pool(name="dram", bufs=1, space="DRAM")

        # ---- load + prep weights -------------------------------------
        wf = {}
        for n in WSHAPE:
            wf[n] = wp.tile(WSHAPE[n], f32, name="f" + n, tag="f" + n)
            nc.sync.dma_start(wf[n][:], win[n][:])
        wsk = wp.tile([C, NE, C], bf16, name="wsk", tag="wsk")
        nc.gpsimd.dma_start(wsk[:], wsk_in[:].transpose([1, 0, 2]))
        naTs = wp.tile([NE, NLOC], bf16, name="naTs", tag="naTs")
        nc.sync.dma_start(naTs[:], naT_in[:])
        rcvb = wp.tile([128, tt], bf16, name="rcvb", tag="rcvb")
        nc.sync.dma_start(rcvb[:], rcvb_in[:])
        sndi = wp.tile([128, epad // 16], i16, name="sndi", tag="sndi")
        nc.sync.dma_start(sndi[:], sndw_in[:])

        def tobf(name, src_ap, shape, scale=None):
            t = wp.tile(shape, bf16, name=name, tag=name)
            if scale is None:
                nc.vector.tensor_copy(t[:], src_ap)
            else:
                nc.vector.tensor_scalar(t[:], src_ap, float(scale), None,
                                        op0=OP.mult)
            return t

        wb = {}
        for n in ["Wr1_1", "Wr1_2", "Wr1_3", "Wr1_o", "Wr2_1", "Wr2_2",
                  "Wr2_3", "Lp1_v", "Lp2_v", "P1s1", "P1ss", "P1v1", "P1sv",
                  "P2v1", "P2sv", "R1c"]:
            wb[n] = tobf("b" + n, wf[n][:], WSHAPE[n])
        for n in ["Wlin1_s", "Wlin1_v", "Wlin2_s", "Wlin2_v"]:
            wb[n] = tobf("b" + n, wf[n][:], WSHAPE[n], scale=1.0 / AVG)
        wb["P1vv"] = tobf("bP1vv", wf["P1vv"][:], WSHAPE["P1vv"], scale=1.0 / SQ3)
        w2o = wp.tile([128, 4 * C], bf16, name="w2o", tag="w2o")
        for h in (0, 64):
            nc.vector.tensor_copy(w2o[h:h + 64, :], wf["Wr2_o"][:])
            nc.vector.tensor_scalar(w2o[h:h + 64, C:2 * C],
                                    wf["Wr2_o"][:, C:2 * C],
                                    1.0 / SQ3, None, op0=OP.mult)
        w1o = wp.tile([128, 2 * C], bf16, name="w1o", tag="w1o")
        for h in (0, 64):
            nc.vector.tensor_copy(w1o[h:h + 64, :], wf["Wr1_o"][:])
        # hidden-layer MLP weights duplicated into the upper PE quadrant
        wb2 = {}
        for n in ["Wr1_1", "Wr1_2", "Wr1_3", "Wr2_1", "Wr2_2", "Wr2_3"]:
            kk = WSHAPE[n][0]
            t = wp.tile([128, 64], bf16, name="q" + n, tag="q" + n)
            for h in (0, 64):
                nc.vector.tensor_copy(t[h:h + kk, :], wf[n][:])
            wb2[n] = t

        psw = tc.alloc_tile_pool(name="psw", bufs=2, space="PSUM")
        eup = psw.tile([NE, C], f32, tag="pw")
        nc.tensor.matmul(eup[:], wf["W_embT"][:], wf["W_up1"][:],
                         start=True, stop=True)
        embup = tobf("embup", eup[:], [NE, C])
        cs = psw.tile([C, C], f32, tag="pw")
        nc.tensor.matmul(cs[:], wf["Lp1_sT"][:], wf["Wup2_s"][:],
                         start=True, stop=True)
        combS = tobf("combS", cs[:], [C, C])
        cv = psw.tile([C, C], f32, tag="pw")
        nc.tensor.matmul(cv[:], wf["Lp1_vT"][:], wf["Wup2_v"][:],
                         start=True, stop=True)
        combV = tobf("combV", cv[:], [C, C])
        qp = psw.tile([C, 1], f32, tag="pw")
        nc.tensor.matmul(qp[:], wf["RmidT"][:], wf["Routc"][:],
                         start=True, stop=True)
        qcol = wp.tile([C, 1], bf16, name="qcol", tag="qcol")
        nc.vector.tensor_scalar(qcol[:], qp[:], 0.5, None, op0=OP.mult)

        ioi = wp.tile([128, t_blk, 128], i32, name="ioi", tag="ioi")
        nc.gpsimd.iota(ioi[:], pattern=[[0, t_blk], [1, 128]], base=0,
                       channel_multiplier=0)
        iob = wp.tile([128, t_blk, 128], bf16, name="iob", tag="iob")
        nc.vector.tensor_copy(iob[:], ioi[:])
        idi = wp.tile([128, 128], i32, name="idi", tag="idi")
        nc.gpsimd.iota(idi[:], pattern=[[1, 128]], base=0, channel_multiplier=-1)
        idf = wp.tile([128, 128], f32, name="idf", tag="idf")
        nc.vector.tensor_copy(idf[:], idi[:])
        ident = wp.tile([128, 128], bf16, name="ident", tag="ident")
        nc.vector.tensor_scalar(ident[:], idf[:], 0.0, None, op0=OP.is_equal)
        identf = wp.tile([128, 128], f32, name="identf", tag="identf")
        nc.vector.tensor_scalar(identf[:], idf[:], 0.0, None, op0=OP.is_equal)
        psw.release()

        eps12 = wp.tile([128, 1], f32, name="eps12", tag="eps12")
        nc.vector.memset(eps12[:], 1e-12)
        mpi = wp.tile([128, 1], f32, name="mpi", tag="mpi")
        nc.vector.memset(mpi[:], -np.pi)
        yv_all = wp.tile([128, tt, 3], bf16, name="yv_all", tag="yv_all")
        a1sb = wp.tile([128, NBLK, C4], bf16, name="a1sb", tag="a1sb")
        a2sb = wp.tile([128, NBLK, C4], bf16, name="a2sb", tag="a2sb")

        # ---------------- phase 1: edges, layer 1 ---------------------
        featsd = dramp.tile([NB, epad], bf16, tag="featsd")
        p0 = tc.alloc_tile_pool(name="p0", bufs=2)
        psG = tc.alloc_tile_pool(name="psG", bufs=2, space="PSUM")

        def run_mlp(ftile, names, psum_pool, act_pool, tagp, ebs):
            """3-layer silu MLP over ebs edges, partition-packed pairs.
            Returns per-512-chunk (a3_tile, half, chunk_off, width)."""
            offs = echunks(ebs)
            cinfo = []
            step = 2 if PACK_MLP else 1
            for pc in range(0, len(offs), step):
                pair = offs[pc:pc + step]
                cw = max(w_ for _, w_ in pair)
                npart = 64 * len(pair)
                a1 = act_pool.tile([128, cw], bf16, tag=tagp + "a1")
                a2 = act_pool.tile([128, cw], bf16, tag=tagp + "a2")
                a3 = act_pool.tile([128, cw], bf16, tag=tagp + "a3")
                prevs = [None, a1, a2]
                outs = [a1, a2, a3]
                for li in range(3):
                    hp = psum_pool.tile([128, cw], f32, tag=tagp + "h")
                    kk = NB if li == 0 else 64
                    for hi, (o_, w_) in enumerate(pair):
                        h = hi * 64
                        if li == 0:
                            rhs = ftile[h:h + NB, o_: o_ + w_]
                        else:
                            rhs = prevs[li][h:h + 64, :w_]
                        nc.tensor.matmul(hp[h:h + 64, :w_],
                                         wb2[names[li]][h:h + kk, :], rhs,
                                         start=True, stop=True)
                    full = len(pair) == 2 and pair[0][1] == pair[1][1]
                    regions = ([(0, 128, cw)] if full else
                               [(hi * 64, hi * 64 + 64, w_)
                                for hi, (o_, w_) in enumerate(pair)])
                    if USE_SILU:
                        for (h0, h1, ww) in regions:
                            nc.scalar.activation(outs[li][h0:h1, :ww],
                                                 hp[h0:h1, :ww], AF.Silu)
                    else:
                        sg = act_pool.tile([128, cw], bf16, tag=tagp + "sg")
                        for (h0, h1, ww) in regions:
                            nc.scalar.activation(sg[h0:h1, :ww],
                                                 hp[h0:h1, :ww], AF.Sigmoid)
                            nc.vector.tensor_tensor(
                                outs[li][h0:h1, :ww], sg[h0:h1, :ww],
                                hp[h0:h1, :ww], op=OP.mult)
                for hi, (o_, w_) in enumerate(pair):
                    cinfo.append((a3, hi, o_, w_))
            return cinfo

        for b in range(NBLK):
            ts0 = b * t_blk
            featsT = p0.tile([NB, eb], bf16, tag="featsT")
            ps_ = p0.tile([128, t_blk, 3], f32, tag="ps")
            pr_ = p0.tile([128, t_blk, 3], f32, tag="pr")
            sh_ = p0.tile([128, t_blk, 3], f32, tag="sh")
            nc.sync.dma_start(ps_[:], pos_s[:, ts0:ts0 + t_blk, :])
            nc.sync.dma_start(pr_[:], pos_r[:, ts0:ts0 + t_blk, :])
            nc.sync.dma_start(sh_[:], shfe[:, ts0:ts0 + t_blk, :])
            vec = p0.tile([128, t_blk, 3], f32, tag="vec")
            nc.vector.tensor_tensor(vec[:], pr_[:], ps_[:], op=OP.subtract)
            nc.vector.tensor_tensor(vec[:], vec[:], sh_[:], op=OP.add)
            sq = p0.tile([128, t_blk, 3], f32, tag="sq")
            nc.vector.tensor_tensor(sq[:], vec[:], vec[:], op=OP.mult)
            ln2 = p0.tile([128, t_blk], f32, tag="ln2")
            nc.vector.tensor_reduce(ln2[:], sq[:], axis=mybir.AxisListType.X,
                                    op=OP.add)
            ln = p0.tile([128, t_blk], f32, tag="ln")
            nc.scalar.activation(ln[:], ln2[:], AF.Sqrt, bias=eps12[:])
            rl = p0.tile([128, t_blk], f32, tag="rl")
            nc.vector.reciprocal(rl[:], ln[:])
            rl3 = rl[:].unsqueeze(-1).broadcast_to([128, t_blk, 3])
            nc.vector.scalar_tensor_tensor(
                yv_all[:, ts0:ts0 + t_blk, :], vec[:], SQ3, rl3,
                op0=OP.mult, op1=OP.mult)
            kb = wf["kpi5"][:].unsqueeze(1).broadcast_to([128, t_blk, NB])
            lnb = ln[:].unsqueeze(-1).broadcast_to([128, t_blk, NB])
            rlb = rl[:].unsqueeze(-1).broadcast_to([128, t_blk, NB])
            arg = p0.tile([128, t_blk, NB], f32, tag="arg")
            nc.vector.tensor_tensor(arg[:], kb, lnb, op=OP.mult)
            yq = p0.tile([128, t_blk, NB], f32, tag="yq")
            nc.vector.tensor_scalar(yq[:], arg[:], 1.0 / (2 * np.pi), None,
                                    op0=OP.mult)
            yqi = p0.tile([128, t_blk, NB], i32, tag="yqi")
            nc.vector.tensor_copy(yqi[:], yq[:])
            nc.vector.tensor_copy(yq[:], yqi[:])
            # r = arg - 2pi*k is in (-pi, 2pi) whether k was trunc or round;
            # fold the (pi, 2pi) tail back by another 2pi
            nc.vector.scalar_tensor_tensor(arg[:], yq[:], -2.0 * np.pi,
                                           arg[:], op0=OP.mult, op1=OP.add)
            nc.vector.tensor_scalar(yq[:], arg[:], np.pi, None, op0=OP.is_gt)
            nc.vector.scalar_tensor_tensor(arg[:], yq[:], -2.0 * np.pi,
                                           arg[:], op0=OP.mult, op1=OP.add)
            snb = p0.tile([128, t_blk, NB], f32, tag="snb")
            nc.scalar.activation(snb[:], arg[:], AF.Sin)
            bes = p0.tile([128, t_blk, NB], f32, tag="bes")
            nc.vector.scalar_tensor_tensor(bes[:], snb[:], (2.0 / RMAX) ** 0.5,
                                           rlb, op0=OP.mult, op1=OP.mult)
            u = p0.tile([128, t_blk], f32, tag="u")
            nc.vector.tensor_scalar(u[:], ln[:], 1.0 / RMAX, None, op0=OP.mult)
            u2 = p0.tile([128, t_blk], f32, tag="u2")
            nc.vector.tensor_tensor(u2[:], u[:], u[:], op=OP.mult)
            u4 = p0.tile([128, t_blk], f32, tag="u4")
            nc.vector.tensor_tensor(u4[:], u2[:], u2[:], op=OP.mult)
            u5 = p0.tile([128, t_blk], f32, tag="u5")
            nc.vector.tensor_tensor(u5[:], u4[:], u[:], op=OP.mult)
            w_ = p0.tile([128, t_blk], f32, tag="w_")
            nc.vector.tensor_scalar(w_[:], u[:], -15.0, 35.0, op0=OP.mult,
                                    op1=OP.add)
            nc.vector.tensor_tensor(w_[:], w_[:], u[:], op=OP.mult)
            nc.vector.tensor_scalar(w_[:], w_[:], -21.0, None, op0=OP.add)
            nc.vector.tensor_tensor(w_[:], w_[:], u5[:], op=OP.mult)
            nc.vector.tensor_scalar(w_[:], w_[:], 1.0, None, op0=OP.add)
            msk = p0.tile([128, t_blk], f32, tag="msk")
            nc.vector.tensor_scalar(msk[:], u[:], 1.0, None, op0=OP.is_lt)
            fc = p0.tile([128, t_blk], f32, tag="fc")
            nc.vector.tensor_tensor(fc[:], w_[:], msk[:], op=OP.mult)
            fcb = fc[:].unsqueeze(-1).broadcast_to([128, t_blk, NB])
            fts = p0.tile([128, t_blk, NB], bf16, tag="fts")
            nc.vector.tensor_tensor(fts[:], bes[:], fcb, op=OP.mult)
            for t in range(t_blk):
                fp = psG.tile([NB, 128], bf16, tag="fp")
                nc.tensor.transpose(fp[:], fts[:, t, :], ident[:])
                nc.scalar.copy(featsT[:, t * 128:(t + 1) * 128], fp[:])
            nc.sync.dma_start(featsd[:, b * eb:(b + 1) * eb], featsT[:])
            if debug and b == 0:
                nc.gpsimd.dma_start(dbg["dbg_fts"][:], fts[:])
                nc.gpsimd.dma_start(dbg["dbg_ln"][:], ln[:])
                nc.gpsimd.dma_start(dbg["dbg_arg"][:], arg[:])
                nc.gpsimd.dma_start(dbg["dbg_snb"][:], snb[:])
                nc.gpsimd.dma_start(dbg["dbg_bes"][:], bes[:])
                nc.gpsimd.dma_start(dbg["dbg_fc"][:], fc[:])
        psG.release()
        p0.release()

        # ---------------- phase 1b: MLP1 + messages + scatter ---------
        p1 = tc.alloc_tile_pool(name="p1", bufs=2)
        p1c = tc.alloc_tile_pool(name="p1c", bufs=3)
        psA = tc.alloc_tile_pool(name="psA", bufs=1, space="PSUM")
        psM = tc.alloc_tile_pool(name="psM", bufs=2, space="PSUM")
        psL = tc.alloc_tile_pool(name="psL", bufs=2, space="PSUM")
        psE = tc.alloc_tile_pool(name="psE", bufs=2, space="PSUM")

        def load_feats(pool, e0, ebs, tag):
            ft = pool.tile([128, TSUB * 128], bf16, tag=tag)
            nc.sync.dma_start(ft[0:NB, :ebs], featsd[:, e0:e0 + ebs])
            nc.sync.dma_start(ft[64:64 + NB, :ebs], ft[0:NB, :ebs])
            return ft

        subs = [(s0, min(TSUB, t_blk - s0)) for s0 in range(0, t_blk, TSUB)]

        for b in range(NBLK):
            acc1 = p1.tile([128, C4], f32, tag="acc1", bufs=2)
            for si, (s0, sw) in enumerate(subs):
                ebs = sw * 128
                ts0 = b * t_blk + s0
                e0 = b * eb + s0 * 128
                attrs_sb = p1.tile([NE, TSUB * 128], bf16, tag="attrs_sb")
                nc.sync.dma_start(attrs_sb[:NE, :ebs], attrsT_in[:, e0:e0 + ebs])
                ft = load_feats(p1, e0, ebs, "featsT1")
                cinfo = run_mlp(ft, ["Wr1_1", "Wr1_2", "Wr1_3"], psM, p1c,
                                "m1", ebs)
                uv = p1.tile([128, TSUB, 2, C], bf16, tag="uv", bufs=1)
                for t in range(sw):
                    o = t * 128
                    a3, hi, o_, _ = cinfo[o // 512]
                    lo = o - o_
                    w1p = psL.tile([128, 2 * C], f32, tag="w1p")
                    nc.tensor.matmul(w1p[:],
                                     a3[hi * 64:(hi + 1) * 64, lo:lo + 128],
                                     w1o[hi * 64:(hi + 1) * 64, :],
                                     start=True, stop=True)
                    ep = psE.tile([128, C], f32, tag="ep")
                    nc.tensor.matmul(ep[:], attrs_sb[:NE, o:o + 128],
                                     embup[:], start=True, stop=True)
                    heb = p1c.tile([128, C], bf16, tag="heb")
                    nc.scalar.copy(heb[:], ep[:])
                    epb = heb[:].unsqueeze(1).broadcast_to([128, 2, C])
                    w1v = w1p[:].rearrange("p (x c) -> p x c", c=C)
                    nc.vector.tensor_tensor(uv[:, t, :, :], w1v, epb, op=OP.mult)
                rb = rcvb[:, ts0:ts0 + sw].unsqueeze(-1).broadcast_to(
                    [128, sw, 128])
                S = p1.tile([128, TSUB, 128], bf16, tag="S", bufs=1)
                nc.vector.tensor_tensor(S[:, :sw, :], iob[:, :sw, :], rb,
                                        op=OP.is_equal)
                Sy = p1.tile([128, TSUB, 3, 128], bf16, tag="Sy", bufs=1)
                for d in range(3):
                    yb = yv_all[:, ts0:ts0 + sw, d].unsqueeze(-1).broadcast_to(
                        [128, sw, 128])
                    nc.vector.tensor_tensor(Sy[:, :sw, d, :], S[:, :sw, :], yb,
                                            op=OP.mult)
                A1 = psA.tile([128, C4], f32, tag="A1")
                for t in range(sw):
                    nc.tensor.matmul(A1[:, 0:C], S[:, t, :], uv[:, t, 0, :],
                                     start=(t == 0), stop=(t == sw - 1),
                                     skip_group_check=True)
                for d in range(3):
                    for t in range(sw):
                        nc.tensor.matmul(A1[:, C * (1 + d):C * (2 + d)],
                                         Sy[:, t, d, :], uv[:, t, 1, :],
                                         start=(t == 0), stop=(t == sw - 1),
                                         skip_group_check=True)
                if si == 0:
                    nc.scalar.copy(acc1[:], A1[:])
                else:
                    nc.vector.tensor_tensor(acc1[:], A1[:], acc1[:], op=OP.add)
            nc.vector.tensor_copy(a1sb[:, b, :], acc1[:])

        for p in (psE, psL, psM, psA, p1c, p1):
            p.release()
        if debug:
            nc.gpsimd.dma_start(dbg["dbg_a1"][:], a1sb[:])
            nc.gpsimd.dma_start(dbg["dbg_ft"][:], featsd[:])

        # ---------------- phase 2: node layer 1 -----------------------
        n1 = tc.alloc_tile_pool(name="n1", bufs=1)
        n1t = tc.alloc_tile_pool(name="n1t", bufs=2)
        n1m = tc.alloc_tile_pool(name="n1m", bufs=1)
        psN = tc.alloc_tile_pool(name="psN", bufs=2, space="PSUM")

        def transpose_blocks(src, c0, name, pool, pspool):
            dst = pool.tile([C, NLOC], bf16, name=name, tag=name)
            for b in range(NBLK):
                tp = pspool.tile([128, 128], bf16, tag="tp")
                nc.tensor.transpose(tp[:], src[:, b, c0:c0 + C], ident[:])
                nc.scalar.copy(dst[:, b * 128:(b + 1) * 128], tp[:])
            return dst

        def mm_wide(name, lhsT, rhs_tile, pool, pspool, dtype=bf16):
            out = pool.tile([C, NLOC], dtype, name=name, tag=name)
            for o, w_ in nchunks:
                pm = pspool.tile([C, 512], f32, tag="mmw")
                nc.tensor.matmul(pm[:, :w_], lhsT, rhs_tile[:, o:o + w_],
                                 start=True, stop=True)
                nc.scalar.copy(out[:, o:o + w_], pm[:, :w_])
            return out

        AsT = transpose_blocks(a1sb[:], 0, "AsT", n1m, psN)
        AvT = [transpose_blocks(a1sb[:], C * (1 + d), f"AvT{d}", n1m, psN)
               for d in range(3)]
        AsL = mm_wide("AsL", wb["Wlin1_s"][:], AsT[:], n1m, psN)
        AvL = [mm_wide(f"AvL{d}", wb["Wlin1_v"][:], AvT[d][:], n1m, psN)
               for d in range(3)]
        Pw = {n: mm_wide("w" + n, wb[n][:], naTs[:], n1m, psN)
              for n in ["P1s1", "P1ss", "P1vv", "P1v1", "P1sv"]}
        sqs = n1m.tile([C, NLOC], bf16, name="sqs", tag="sqs")
        nc.scalar.square(sqs[:], AsL[:])
        vv = n1m.tile([C, NLOC], f32, name="vv", tag="vv")
        sqv = n1m.tile([C, NLOC], f32, name="sqv", tag="sqv")
        nc.scalar.square(vv[:], AvL[0][:])
        for d in (1, 2):
            nc.scalar.square(sqv[:], AvL[d][:])
            nc.vector.tensor_tensor(vv[:], vv[:], sqv[:], op=OP.add)
        Bs = n1m.tile([C, NLOC], bf16, name="Bs", tag="Bs")
        t0 = n1t.tile([C, NLOC], bf16, tag="t0")
        nc.vector.tensor_tensor(Bs[:], Pw["P1s1"][:], AsL[:], op=OP.mult)
        nc.vector.tensor_tensor(t0[:], Pw["P1ss"][:], sqs[:], op=OP.mult)
        nc.vector.tensor_tensor(Bs[:], Bs[:], t0[:], op=OP.add)
        t1 = n1t.tile([C, NLOC], bf16, tag="t0")
        nc.vector.tensor_tensor(t1[:], Pw["P1vv"][:], vv[:], op=OP.mult)
        nc.vector.tensor_tensor(Bs[:], Bs[:], t1[:], op=OP.add)
        gsk = n1m.tile([C, NLOC], bf16, name="gsk", tag="gsk")
        nc.vector.tensor_tensor(gsk[:], Pw["P1sv"][:], AsL[:], op=OP.mult)
        nc.vector.tensor_tensor(gsk[:], gsk[:], Pw["P1v1"][:], op=OP.add)
        Bv = []
        for d in range(3):
            bvd = n1m.tile([C, NLOC], bf16, name=f"Bv{d}", tag=f"Bv{d}")
            nc.vector.tensor_tensor(bvd[:], gsk[:], AvL[d][:], op=OP.mult)
            Bv.append(bvd)
        hvT = [mm_wide(f"hvT{d}", wb["Lp1_v"][:], Bv[d][:], n1, psN)
               for d in range(3)]
        tabsb = n1m.tile([128, NBLK, C4], bf16, name="tabsb", tag="tabsb")
        for b in range(NBLK):
            pm = psN.tile([128, C], f32, tag="tab")
            nc.tensor.matmul(pm[:], Bs[:, b * 128:(b + 1) * 128], combS[:],
                             start=True, stop=True)
            nc.scalar.copy(tabsb[:, b, 0:C], pm[:])
            for d in range(3):
                pm2 = psN.tile([128, C], f32, tag="tab")
                nc.tensor.matmul(pm2[:], Bv[d][:, b * 128:(b + 1) * 128],
                                 combV[:], start=True, stop=True)
                nc.scalar.copy(tabsb[:, b, C * (1 + d):C * (2 + d)], pm2[:])
        if debug:
            nc.gpsimd.dma_start(dbg["dbg_tab"][:], tabsb[:])
        bounce = dramp.tile([NLOC, C4], bf16, tag="bounce")
        nc.sync.dma_start(bounce[:].rearrange("(b p) c -> p b c", p=128),
                          tabsb[:])
        tabdram = dramp.tile([NCORES * NLOC, C4], bf16, addr_space="Shared",
                             tag="tabdram")
        nc.gpsimd.collective_compute(
            "AllGather", OP.bypass, replica_groups=[list(range(NCORES))],
            ins=[bounce[:]], outs=[tabdram[:]])
        psN.release()
        n1m.release()

        # ---------------- phase 4: edges, layer 2 ---------------------
        p4 = tc.alloc_tile_pool(name="p4", bufs=2)
        p4c = tc.alloc_tile_pool(name="p4c", bufs=3)
        psA2 = tc.alloc_tile_pool(name="psA2", bufs=1, space="PSUM")
        psM2 = tc.alloc_tile_pool(name="psM2", bufs=2, space="PSUM")
        psL2 = tc.alloc_tile_pool(name="psL2", bufs=2, space="PSUM")

        for b in range(NBLK):
            acc2 = p4.tile([128, C4], f32, tag="acc2", bufs=2)
            for si, (s0, sw) in enumerate(subs):
                ebs = sw * 128
                ts0 = b * t_blk + s0
                e0 = b * eb + s0 * 128
                featsT2 = load_feats(p4, e0, ebs, "featsT2")
                gat = p4.tile([128, TSUB, C4], bf16, tag="gat")
                nc.gpsimd.dma_gather(
                    gat[:, :sw, :], tabdram[:],
                    sndi[:, e0 // 16:(e0 + ebs) // 16],
                    num_idxs=ebs, num_idxs_reg=ebs, elem_size=C4)
                gat4 = gat[:].rearrange("p t (x c) -> p t x c", c=C)
                cinfo = run_mlp(featsT2, ["Wr2_1", "Wr2_2", "Wr2_3"], psM2,
                                p4c, "m2", ebs)
                w2sb = p4.tile([128, TSUB, 4 * C], bf16, tag="w2sb", bufs=1)
                for t in range(sw):
                    o = t * 128
                    a3, hi, o_, _ = cinfo[o // 512]
                    lo = o - o_
                    w2p = psL2.tile([128, 4 * C], f32, tag="w2p")
                    nc.tensor.matmul(w2p[:],
                                     a3[hi * 64:(hi + 1) * 64, lo:lo + 128],
                                     w2o[hi * 64:(hi + 1) * 64, :],
                                     start=True, stop=True)
                    nc.scalar.copy(w2sb[:, t, :], w2p[:])
                rb = rcvb[:, ts0:ts0 + sw].unsqueeze(-1).broadcast_to(
                    [128, sw, 128])
                S = p4.tile([128, TSUB, 128], bf16, tag="S4", bufs=1)
                nc.vector.tensor_tensor(S[:, :sw, :], iob[:, :sw, :], rb,
                                        op=OP.is_equal)
                Sy = p4.tile([128, TSUB, 3, 128], bf16, tag="Sy4", bufs=1)
                for d in range(3):
                    yb = yv_all[:, ts0:ts0 + sw, d].unsqueeze(-1).broadcast_to(
                        [128, sw, 128])
                    nc.vector.tensor_tensor(Sy[:, :sw, d, :], S[:, :sw, :], yb,
                                            op=OP.mult)
                m4 = p4.tile([128, TSUB, 4, C], bf16, tag="m4", bufs=1)
                m5 = p4.tile([128, TSUB, 4, C], bf16, tag="m5", bufs=1)
                hsg = gat4[:, :sw, 0, :]
                nc.vector.tensor_tensor(m4[:, :sw, 0, :], w2sb[:, :sw, 0:C],
                                        hsg, op=OP.mult)
                nc.vector.tensor_tensor(m5[:, :sw, 3, :],
                                        w2sb[:, :sw, 2 * C:3 * C], hsg,
                                        op=OP.mult)
                for d in range(3):
                    hvg = gat4[:, :sw, 1 + d, :]
                    nc.vector.tensor_tensor(m4[:, :sw, 1 + d, :],
                                            w2sb[:, :sw, 3 * C:4 * C], hvg,
                                            op=OP.mult)
                    nc.vector.tensor_tensor(m5[:, :sw, d, :],
                                            w2sb[:, :sw, C:2 * C], hvg,
                                            op=OP.mult)
                A2 = psA2.tile([128, 4, C], f32, tag="A2")
                for t in range(sw):
                    nc.tensor.matmul(A2[:], S[:, t, :],
                                     m4[:, t, :, :], start=(t == 0), stop=False,
                                     skip_group_check=True)
                for d in range(3):
                    for t in range(sw):
                        nc.tensor.matmul(A2[:, 1 + d, :], Sy[:, t, d, :],
                                         m5[:, t, 3, :], start=False,
                                         stop=False, skip_group_check=True)
                for d in range(3):
                    for t in range(sw):
                        sp = (t == sw - 1) and (d == 2)
                        nc.tensor.matmul(A2[:, 0, :], Sy[:, t, d, :],
                                         m5[:, t, d, :], start=False, stop=sp,
                                         skip_group_check=True)
                a2f = A2[:].rearrange("p x c -> p (x c)")
                if si == 0:
                    nc.scalar.copy(acc2[:], a2f)
                else:
                    nc.vector.tensor_tensor(acc2[:], a2f, acc2[:], op=OP.add)
            nc.vector.tensor_copy(a2sb[:, b, :], acc2[:])
            if debug and b == 0:
                nc.gpsimd.dma_start(dbg["dbg_gat"][:, 0:min(TSUB, t_blk)],
                                    gat[:, 0:min(TSUB, t_blk)])
                nc.gpsimd.dma_start(dbg["dbg_m4"][:, 0:min(TSUB, t_blk)],
                                    m4[:, 0:min(TSUB, t_blk)])

        for p in (psL2, psM2, psA2, p4c, p4):
            p.release()
        if debug:
            nc.gpsimd.dma_start(dbg["dbg_a2"][:], a2sb[:])

        # ---------------- phase 5: node layer 2 + outputs -------------
        n2 = tc.alloc_tile_pool(name="n2", bufs=1)
        psN2 = tc.alloc_tile_pool(name="psN2", bufs=2, space="PSUM")
        A2sT = transpose_blocks(a2sb[:], 0, "A2sT", n2, psN2)
        A2vT = [transpose_blocks(a2sb[:], C * (1 + d), f"A2vT{d}", n2, psN2)
                for d in range(3)]
        A2sL = mm_wide("A2sL", wb["Wlin2_s"][:], A2sT[:], n2, psN2)
        A2vL = [mm_wide(f"A2vL{d}", wb["Wlin2_v"][:], A2vT[d][:], n2, psN2)
                for d in range(3)]
        P2w = {n: mm_wide("w" + n, wb[n][:], naTs[:], n2, psN2)
               for n in ["P2v1", "P2sv"]}
        g2 = n2.tile([C, NLOC], bf16, name="g2", tag="g2")
        nc.vector.tensor_tensor(g2[:], P2w["P2sv"][:], A2sL[:], op=OP.mult)
        nc.vector.tensor_tensor(g2[:], g2[:], P2w["P2v1"][:], op=OP.add)
        B2v = []
        for d in range(3):
            b2d = n2.tile([C, NLOC], bf16, name=f"B2v{d}", tag=f"B2v{d}")
            nc.vector.tensor_tensor(b2d[:], g2[:], A2vL[d][:], op=OP.mult)
            B2v.append(b2d)
        psN2.release()

        # h2 (node-major) = B2v @ Lp2_v + skip-TP, then back to feat-major
        n2t = tc.alloc_tile_pool(name="n2t", bufs=2)
        psH = tc.alloc_tile_pool(name="psH", bufs=1, space="PSUM")
        psHt = tc.alloc_tile_pool(name="psHt", bufs=2, space="PSUM")
        h2T = [n2.tile([C, NLOC], bf16, name=f"h2T{d}", tag=f"h2T{d}")
               for d in range(3)]
        na_sb = n2.tile([128, NBLK, NE], f32, name="na_sb", tag="na_sb")
        nc.sync.dma_start(na_sb[:], na_in[:].rearrange("(b p) e -> p b e", p=128))
        for b in range(NBLK):
            sc = n2t.tile([128, 3, C], f32, tag="sc")
            hp = psH.tile([128, 3, C], f32, tag="h2p")
            for d in range(3):
                nc.tensor.matmul(hp[:, d, :], B2v[d][:, b * 128:(b + 1) * 128],
                                 wb["Lp2_v"][:], start=True, stop=True)
            nc.scalar.copy(sc[:], hp[:])
            gp = psH.tile([128, 3, C], f32, tag="gp")
            for s in range(NE):
                for d in range(3):
                    nc.tensor.matmul(gp[:, d, :],
                                     hvT[d][:, b * 128:(b + 1) * 128],
                                     wsk[:, s, :], start=True, stop=True)
                nc.vector.scalar_tensor_tensor(
                    sc[:], gp[:], na_sb[:, b, s:s + 1], sc[:],
                    op0=OP.mult, op1=OP.add)
            sc16 = n2t.tile([128, 3, C], bf16, tag="sc16")
            nc.vector.tensor_copy(sc16[:], sc[:])
            for d in range(3):
                tp = psHt.tile([128, 128], bf16, tag="tph")
                nc.tensor.transpose(tp[:], sc16[:, d, :], ident[:])
                nc.scalar.copy(h2T[d][:, b * 128:(b + 1) * 128], tp[:])
        psHt.release()
        psH.release()

        psD = tc.alloc_tile_pool(name="psD", bufs=2, space="PSUM")
        dipf = [n2.tile([1, NLOC], f32, name=f"dipf{d}", tag=f"dipf{d}")
                for d in range(3)]
        for d in range(3):
            for o, w_ in nchunks:
                dp = psD.tile([1, 512], f32, tag="dp")
                nc.tensor.matmul(dp[:, :w_], wb["R1c"][:], hvT[d][:, o:o + w_],
                                 start=True, stop=False)
                nc.tensor.matmul(dp[:, :w_], qcol[:], h2T[d][:, o:o + w_],
                                 start=False, stop=True)
                nc.scalar.copy(dipf[d][0:1, o:o + w_], dp[:, :w_])

        posb = n2.tile([128, NBLK, 3], f32, name="posb", tag="posb")
        nc.sync.dma_start(posb[:], posn_in[:].rearrange("(b p) c -> p b c", p=128))
        chgb = n2.tile([128, NBLK], f32, name="chgb", tag="chgb")
        nc.sync.dma_start(chgb[:], chg_in[:].rearrange("(b p) c -> p (b c)", p=128))
        bohb = n2.tile([128, NBLK, G], f32, name="bohb", tag="bohb")
        nc.sync.dma_start(bohb[:], boh_in[:].rearrange("(b p) g -> p b g", p=128))
        dipo = n2.tile([128, NBLK, 3], f32, name="dipo", tag="dipo")
        gs = psD.tile([G, 3], f32, tag="gs", bufs=1)
        for b in range(NBLK):
            for d in range(3):
                dpp = psD.tile([128, 1], f32, tag="dpp")
                nc.tensor.transpose(dpp[:], dipf[d][0:1, b * 128:(b + 1) * 128],
                                    identf[0:1, 0:1])
                nc.scalar.copy(dipo[:, b, d:d + 1], dpp[:])
            cp = n2t.tile([128, 3], f32, tag="cp")
            nc.vector.tensor_scalar(cp[:], posb[:, b, :], chgb[:, b:b + 1],
                                    None, op0=OP.mult)
            nc.vector.tensor_tensor(cp[:], cp[:], dipo[:, b, :], op=OP.add)
            nc.tensor.matmul(gs[:], bohb[:, b, :], cp[:], start=(b == 0),
                             stop=(b == NBLK - 1))
        nc.sync.dma_start(out_dip[:].rearrange("(b p) c -> p b c", p=128),
                          dipo[:])
        if debug:
            nc.gpsimd.dma_start(dbg["dbg_hvT0"][:], hvT[0][:])
            nc.gpsimd.dma_start(dbg["dbg_Bs"][:], Bs[:])
            nc.gpsimd.dma_start(dbg["dbg_h2T0"][:], h2T[0][:])
            nc.gpsimd.dma_start(dbg["dbg_dipf0"][:], dipf[0][:])
        gso = n2.tile([G, 3], f32, name="gso", tag="gso")
        nc.scalar.copy(gso[:], gs[:])
        nc.sync.dma_start(out_gs[:], gso[:])

        for p in (psD, n2t, n2, n1t, n1):
            p.release()
        dramp.release()
        wp.release()

    nc.compile()
    return nc


_BUILD_CACHE = {}


def kernel(**inputs):
    in_maps, t_blk = prep_shards(inputs)
    nc = _BUILD_CACHE.get(t_blk)
    if nc is None:
        nc = build(t_blk)
        _BUILD_CACHE[t_blk] = nc
    res = run_bass_kernel_spmd(nc, in_maps, core_ids=list(range(NCORES)))
    dip = np.zeros((N, 3), np.float32)
    tot = np.zeros((G, 3), np.float32)
    for k in range(NCORES):
        dip[k * NPC:(k + 1) * NPC] = res.results[k]["dip"][:NPC]
        tot += res.results[k]["gsum"]
    return tot, dip


# revision 54
# speedup vs baseline: 1.1693x; 1.1693x over previous
"""AtomicDipolesMACE on 8 TRN2 NeuronCores.

Sharding: nodes are split into 8 contiguous ranges of 1250 (padded to 1280 =
10 blocks of 128). Each core owns the edges whose receiver falls in its range,
sorted and grouped by 128-node receiver block, each block segment padded to a
uniform T_BLK tiles of 128 edges. Scatters become per-block one-hot matmuls
(PSUM f32 accumulation, Yv folded into scaled one-hots); the layer-2 sender
gather is a dma_gather from an AllGathered bf16 node-feature table.
"""
import sys
sys.path.insert(0, "/opt/trn_rl_repo")
import numpy as np
import ml_dtypes

from concourse import bass, bacc, tile, mybir
from concourse.bass_utils import run_bass_kernel_spmd

f32 = mybir.dt.float32
bf16 = mybir.dt.bfloat16
i16 = mybir.dt.int16
i32 = mybir.dt.int32
AF = mybir.ActivationFunctionType
OP = mybir.AluOpType
nbf16 = ml_dtypes.bfloat16
USE_SILU = True
PACK_MLP = True
TSUB = 6

N, E, C, NE, G, NB = 10000, 160000, 128, 10, 16, 8
RMAX, AVG, SQ3 = 5.0, 16.0, 3.0 ** 0.5
NCORES = 8
NPC = N // NCORES          # 1250 real nodes per core
NBLK = 10                  # node blocks per core
NLOC = NBLK * 128          # 1280 padded local nodes
C4 = 4 * C                 # 512: table row / message width

WSHAPE = dict(W_embT=[C, NE], W_up1=[C, C], Wr1_1=[NB, 64], Wr1_2=[64, 64],
              Wr1_3=[64, 64], Wr1_o=[64, 2 * C], Wr2_1=[NB, 64],
              Wr2_2=[64, 64], Wr2_3=[64, 64], Wr2_o=[64, 4 * C],
              Wlin1_s=[C, C], Wlin1_v=[C, C], Wlin2_s=[C, C],
              Wlin2_v=[C, C], Lp1_sT=[C, C], Lp1_vT=[C, C], Lp1_v=[C, C],
              Lp2_v=[C, C], Wup2_s=[C, C], Wup2_v=[C, C],
              P1s1=[NE, C], P1ss=[NE, C], P1vv=[NE, C], P1v1=[NE, C],
              P1sv=[NE, C], P2v1=[NE, C], P2sv=[NE, C],
              R1c=[C, 1], RmidT=[16, C], Routc=[16, 1], kpi5=[128, NB])


# ---------------------------------------------------------------- host prep
def prep_shards(inp):
    snd = np.asarray(inp["edge_index"][0], dtype=np.int64)
    rcv = np.asarray(inp["edge_index"][1], dtype=np.int64)
    pos = np.asarray(inp["positions"], dtype=np.float32)
    na = np.asarray(inp["node_attrs"], dtype=np.float32)
    shf = np.asarray(inp["shifts"], dtype=np.float32)
    chg = np.asarray(inp["charges"], dtype=np.float32)
    bat = np.asarray(inp["batch"], dtype=np.int64)

    core = rcv // NPC
    loc = rcv - core * NPC
    blk = loc // 128

    order = np.lexsort((blk, core))
    snd_s, rcv_s, loc_s = snd[order], rcv[order], loc[order]
    core_s, blk_s = core[order], blk[order]
    shf_s = shf[order]

    counts = np.zeros((NCORES, NBLK), dtype=np.int64)
    np.add.at(counts, (core_s, blk_s), 1)
    t_blk = max(1, int(np.max((counts + 127) // 128)))
    eb = t_blk * 128
    epad = NBLK * eb
    tt = NBLK * t_blk

    snd_pad = (snd_s // NPC) * NLOC + (snd_s % NPC)  # padded-table row index

    starts = np.zeros(NCORES * NBLK, dtype=np.int64)
    starts[1:] = np.cumsum(counts.reshape(-1))[:-1]
    starts = starts.reshape(NCORES, NBLK)

    w = weights_prep(inp)
    in_maps = []
    for k in range(NCORES):
        pos_s = np.zeros((epad, 3), np.float32)
        pos_r = np.ones((epad, 3), np.float32)
        shfe = np.zeros((epad, 3), np.float32)
        attrsT = np.zeros((NE, epad), np.float32)
        sndp = np.zeros(epad, np.int64)
        rcvb = -np.ones(epad, np.float32)
        for b in range(NBLK):
            s0, n = starts[k, b], counts[k, b]
            sl = slice(s0, s0 + n)
            d0 = b * eb
            pos_s[d0:d0 + n] = pos[snd_s[sl]]
            pos_r[d0:d0 + n] = pos[rcv_s[sl]]
            shfe[d0:d0 + n] = shf_s[sl]
            attrsT[:, d0:d0 + n] = na[snd_s[sl]].T
            sndp[d0:d0 + n] = snd_pad[sl]
            rcvb[d0:d0 + n] = (loc_s[sl] - b * 128).astype(np.float32)

        def emaj(a):  # [epad, d] -> [128, tt, d]  (edge i -> (i%128, i//128))
            return np.ascontiguousarray(a.reshape(tt, 128, -1).transpose(1, 0, 2))

        sndw = np.zeros((128, epad // 16), np.int16)
        for b in range(NBLK):
            seg = sndp[b * eb:(b + 1) * eb].reshape(eb // 16, 16).T
            sndw[:, b * (eb // 16):(b + 1) * (eb // 16)] = np.tile(seg, (8, 1))

        nl0 = k * NPC
        na_nm = np.zeros((NLOC, NE), np.float32)
        na_nm[:NPC] = na[nl0:nl0 + NPC]
        pos_nm = np.zeros((NLOC, 3), np.float32)
        pos_nm[:NPC] = pos[nl0:nl0 + NPC]
        chg_nm = np.zeros((NLOC, 1), np.float32)
        chg_nm[:NPC, 0] = chg[nl0:nl0 + NPC]
        boh = np.zeros((NLOC, G), np.float32)
        boh[np.arange(NPC), bat[nl0:nl0 + NPC]] = 1.0

        m = dict(
            pos_s=emaj(pos_s), pos_r=emaj(pos_r), shfe=emaj(shfe),
            attrsT=attrsT.astype(nbf16), sndw=sndw,
            rcvb=emaj(rcvb)[:, :, 0].astype(nbf16),
            na_nm=na_nm, naT=np.ascontiguousarray(na_nm.T).astype(nbf16),
            pos_nm=pos_nm, chg_nm=chg_nm, boh=boh,
        )
        m.update(w)
        in_maps.append(m)
    return in_maps, t_blk


def weights_prep(inp):
    g = lambda k: np.ascontiguousarray(np.asarray(inp[k], dtype=np.float32))
    kvec = (np.pi / RMAX) * np.arange(1, NB + 1, dtype=np.float32)
    return dict(
        W_embT=g("W_emb").T.copy(), W_up1=g("W_up1"),
        Wr1_1=g("Wr1_1"), Wr1_2=g("Wr1_2"), Wr1_3=g("Wr1_3"), Wr1_o=g("Wr1_o"),
        Wr2_1=g("Wr2_1"), Wr2_2=g("Wr2_2"), Wr2_3=g("Wr2_3"), Wr2_o=g("Wr2_o"),
        Wlin1_s=g("Wlin1_s"), Wlin1_v=g("Wlin1_v"),
        Wlin2_s=g("Wlin2_s"), Wlin2_v=g("Wlin2_v"),
        Lp1_sT=g("Lp1_s").T.copy(), Lp1_vT=g("Lp1_v").T.copy(),
        Lp1_v=g("Lp1_v"), Lp2_v=g("Lp2_v"),
        Wup2_s=g("Wup2_s"), Wup2_v=g("Wup2_v"),
        P1s1=g("P1_s1"), P1ss=g("P1_ss"), P1vv=g("P1_vv"),
        P1v1=g("P1_v1"), P1sv=g("P1_sv"), P2v1=g("P2_v1"), P2sv=g("P2_sv"),
        Wsk=g("Wsk"),
        R1c=g("R1").reshape(C, 1), RmidT=g("Rmid").T.copy(),
        Routc=g("Rout").reshape(16, 1),
        kpi5=np.tile(kvec, (128, 1)),
    )


# ---------------------------------------------------------------- builder
def build(t_blk, debug=False):
    eb = t_blk * 128
    tt = NBLK * t_blk
    epad = NBLK * eb
    nc = bacc.Bacc(None, target_bir_lowering=False, num_devices=NCORES)

    def din(name, shape, dt=f32):
        return nc.declare_dram_parameter(name, shape, dt, isOutput=False)

    pos_s = din("pos_s", [128, tt, 3]); pos_r = din("pos_r", [128, tt, 3])
    shfe = din("shfe", [128, tt, 3])
    attrsT_in = din("attrsT", [NE, epad], bf16)
    sndw_in = din("sndw", [128, epad // 16], i16)
    rcvb_in = din("rcvb", [128, tt], bf16)
    na_in = din("na_nm", [NLOC, NE]); naT_in = din("naT", [NE, NLOC], bf16)
    posn_in = din("pos_nm", [NLOC, 3]); chg_in = din("chg_nm", [NLOC, 1])
    boh_in = din("boh", [NLOC, G])
    win = {n: din(n, WSHAPE[n]) for n in WSHAPE}
    wsk_in = din("Wsk", [NE, C, C])
    out_dip = nc.declare_dram_parameter("dip", [NLOC, 3], f32, isOutput=True)
    out_gs = nc.declare_dram_parameter("gsum", [16, 3], f32, isOutput=True)
    dbg = {}
    if debug:
        for nm, shp in [("dbg_a1", [128, NBLK, C4]), ("dbg_tab", [128, NBLK, C4]),
                        ("dbg_a2", [128, NBLK, C4]), ("dbg_hvT0", [C, NLOC]),
                        ("dbg_Bs", [C, NLOC]), ("dbg_h2T0", [C, NLOC]),
                        ("dbg_dipf0", [1, NLOC]), ("dbg_gat", [128, t_blk, C4]),
                        ("dbg_m4", [128, t_blk, 4, C]),
                        ("dbg_ft", [NB, epad]), ("dbg_uv", [128, t_blk, 2, C]),
                        ("dbg_S", [128, t_blk, 128]),
                        ("dbg_fts", [128, t_blk, NB]),
                        ("dbg_ln", [128, t_blk]), ("dbg_arg", [128, t_blk, NB]),
                        ("dbg_snb", [128, t_blk, NB]),
                        ("dbg_bes", [128, t_blk, NB]),
                        ("dbg_fc", [128, t_blk])]:
            dbg[nm] = nc.declare_dram_parameter(nm, shp, f32, isOutput=True)

    nchunks = []
    _o = 0
    while _o < NLOC:
        nchunks.append((_o, min(512, NLOC - _o)))
        _o += nchunks[-1][1]

    def echunks(ebs):
        half = ebs // 2
        if ebs <= 1024 and half % 128 == 0 and half > 0:
            return [(0, half), (half, ebs - half)]
        o, out = 0, []
        while o < ebs:
            w_ = min(512, ebs - o)
            out.append((o, w_))
            o += w_
        return out

    with tile.TileContext(nc) as tc:
        wp = tc.alloc_tile_pool(name="wp", bufs=1)
        dramp = tc.alloc_tile_pool(name="dram", bufs=1, space="DRAM")

        # ---- load + prep weights -------------------------------------
        wf = {}
        for n in WSHAPE:
            wf[n] = wp.tile(WSHAPE[n], f32, name="f" + n, tag="f" + n)
            nc.sync.dma_start(wf[n][:], win[n][:])
        wsk = wp.tile([C, NE, C], bf16, name="wsk", tag="wsk")
        nc.gpsimd.dma_start(wsk[:], wsk_in[:].transpose([1, 0, 2]))
        naTs = wp.tile([NE, NLOC], bf16, name="naTs", tag="naTs")
        nc.sync.dma_start(naTs[:], naT_in[:])
        rcvb = wp.tile([128, tt], bf16, name="rcvb", tag="rcvb")
        nc.sync.dma_start(rcvb[:], rcvb_in[:])
        sndi = wp.tile([128, epad // 16], i16, name="sndi", tag="sndi")
        nc.sync.dma_start(sndi[:], sndw_in[:])

        def tobf(name, src_ap, shape, scale=None):
            t = wp.tile(shape, bf16, name=name, tag=name)
            if scale is None:
                nc.vector.tensor_copy(t[:], src_ap)
            else:
                nc.vector.tensor_scalar(t[:], src_ap, float(scale), None,
                                        op0=OP.mult)
            return t

        wb = {}
        for n in ["Wr1_1", "Wr1_2", "Wr1_3", "Wr1_o", "Wr2_1", "Wr2_2",
                  "Wr2_3", "Lp1_v", "Lp2_v", "P1s1", "P1ss", "P1v1", "P1sv",
                  "P2v1", "P2sv", "R1c"]:
            wb[n] = tobf("b" + n, wf[n][:], WSHAPE[n])
        for n in ["Wlin1_s", "Wlin1_v", "Wlin2_s", "Wlin2_v"]:
            wb[n] = tobf("b" + n, wf[n][:], WSHAPE[n], scale=1.0 / AVG)
        wb["P1vv"] = tobf("bP1vv", wf["P1vv"][:], WSHAPE["P1vv"], scale=1.0 / SQ3)
        w2o = wp.tile([128, 4 * C], bf16, name="w2o", tag="w2o")
        for h in (0, 64):
            nc.vector.tensor_copy(w2o[h:h + 64, :], wf["Wr2_o"][:])
            nc.vector.tensor_scalar(w2o[h:h + 64, C:2 * C],
                                    wf["Wr2_o"][:, C:2 * C],
                                    1.0 / SQ3, None, op0=OP.mult)
        w1o = wp.tile([128, 2 * C], bf16, name="w1o", tag="w1o")
        for h in (0, 64):
            nc.vector.tensor_copy(w1o[h:h + 64, :], wf["Wr1_o"][:])
        # hidden-layer MLP weights duplicated into the upper PE quadrant
        wb2 = {}
        for n in ["Wr1_1", "Wr1_2", "Wr1_3", "Wr2_1", "Wr2_2", "Wr2_3"]:
            kk = WSHAPE[n][0]
            t = wp.tile([128, 64], bf16, name="q" + n, tag="q" + n)
            for h in (0, 64):
                nc.vector.tensor_copy(t[h:h + kk, :], wf[n][:])
            wb2[n] = t

        psw = tc.alloc_tile_pool(name="psw", bufs=2, space="PSUM")
        eup = psw.tile([NE, C], f32, tag="pw")
        nc.tensor.matmul(eup[:], wf["W_embT"][:], wf["W_up1"][:],
                         start=True, stop=True)
        embup = tobf("embup", eup[:], [NE, C])
        cs = psw.tile([C, C], f32, tag="pw")
        nc.tensor.matmul(cs[:], wf["Lp1_sT"][:], wf["Wup2_s"][:],
                         start=True, stop=True)
        combS = tobf("combS", cs[:], [C, C])
        cv = psw.tile([C, C], f32, tag="pw")
        nc.tensor.matmul(cv[:], wf["Lp1_vT"][:], wf["Wup2_v"][:],
                         start=True, stop=True)
        combV = tobf("combV", cv[:], [C, C])
        qp = psw.tile([C, 1], f32, tag="pw")
        nc.tensor.matmul(qp[:], wf["RmidT"][:], wf["Routc"][:],
                         start=True, stop=True)
        qcol = wp.tile([C, 1], bf16, name="qcol", tag="qcol")
        nc.vector.tensor_scalar(qcol[:], qp[:], 0.5, None, op0=OP.mult)

        ioi = wp.tile([128, t_blk, 128], i32, name="ioi", tag="ioi")
        nc.gpsimd.iota(ioi[:], pattern=[[0, t_blk], [1, 128]], base=0,
                       channel_multiplier=0)
        iob = wp.tile([128, t_blk, 128], bf16, name="iob", tag="iob")
        nc.vector.tensor_copy(iob[:], ioi[:])
        idi = wp.tile([128, 128], i32, name="idi", tag="idi")
        nc.gpsimd.iota(idi[:], pattern=[[1, 128]], base=0, channel_multiplier=-1)
        idf = wp.tile([128, 128], f32, name="idf", tag="idf")
        nc.vector.tensor_copy(idf[:], idi[:])
        ident = wp.tile([128, 128], bf16, name="ident", tag="ident")
        nc.vector.tensor_scalar(ident[:], idf[:], 0.0, None, op0=OP.is_equal)
        identf = wp.tile([128, 128], f32, name="identf", tag="identf")
        nc.vector.tensor_scalar(identf[:], idf[:], 0.0, None, op0=OP.is_equal)
        psw.release()

        eps12 = wp.tile([128, 1], f32, name="eps12", tag="eps12")
        nc.vector.memset(eps12[:], 1e-12)
        mpi = wp.tile([128, 1], f32, name="mpi", tag="mpi")
        nc.vector.memset(mpi[:], -np.pi)
        yv_all = wp.tile([128, tt, 3], bf16, name="yv_all", tag="yv_all")
        a1sb = wp.tile([128, NBLK, C4], bf16, name="a1sb", tag="a1sb")
        a2sb = wp.tile([128, NBLK, C4], bf16, name="a2sb", tag="a2sb")

        # ---------------- phase 1: edges, layer 1 ---------------------
        featsd = dramp.tile([NB, epad], bf16, tag="featsd")
        sdram = dramp.tile([128, tt, 128], bf16, tag="sdram")
        sydram = dramp.tile([128, tt, 3, 128], bf16, tag="sydram")
        p1 = tc.alloc_tile_pool(name="p1", bufs=2)
        p1c = tc.alloc_tile_pool(name="p1c", bufs=3)
        psA = tc.alloc_tile_pool(name="psA", bufs=2, space="PSUM")
        psM = tc.alloc_tile_pool(name="psM", bufs=2, space="PSUM")
        psL = tc.alloc_tile_pool(name="psL", bufs=2, space="PSUM")
        psE = tc.alloc_tile_pool(name="psE", bufs=2, space="PSUM")

        def run_mlp(ftile, names, psum_pool, act_pool, tagp, ebs):
            """3-layer silu MLP over ebs edges, partition-packed pairs.
            Returns per-512-chunk (a3_tile, half, chunk_off, width)."""
            offs = echunks(ebs)
            cinfo = []
            step = 2 if PACK_MLP else 1
            for pc in range(0, len(offs), step):
                pair = offs[pc:pc + step]
                cw = max(w_ for _, w_ in pair)
                npart = 64 * len(pair)
                a1 = act_pool.tile([128, cw], bf16, tag=tagp + "a1")
                a2 = act_pool.tile([128, cw], bf16, tag=tagp + "a2")
                a3 = act_pool.tile([128, cw], bf16, tag=tagp + "a3")
                prevs = [None, a1, a2]
                outs = [a1, a2, a3]
                for li in range(3):
                    hp = psum_pool.tile([128, cw], f32, tag=tagp + "h")
                    kk = NB if li == 0 else 64
                    for hi, (o_, w_) in enumerate(pair):
                        h = hi * 64
                        if li == 0:
                            rhs = ftile[h:h + NB, o_: o_ + w_]
                        else:
                            rhs = prevs[li][h:h + 64, :w_]
                        nc.tensor.matmul(hp[h:h + 64, :w_],
                                         wb2[names[li]][h:h + kk, :], rhs,
                                         start=True, stop=True)
                    full = len(pair) == 2 and pair[0][1] == pair[1][1]
                    regions = ([(0, 128, cw)] if full else
                               [(hi * 64, hi * 64 + 64, w_)
                                for hi, (o_, w_) in enumerate(pair)])
                    if USE_SILU:
                        for (h0, h1, ww) in regions:
                            nc.scalar.activation(outs[li][h0:h1, :ww],
                                                 hp[h0:h1, :ww], AF.Silu)
                    else:
                        sg = act_pool.tile([128, cw], bf16, tag=tagp + "sg")
                        for (h0, h1, ww) in regions:
                            nc.scalar.activation(sg[h0:h1, :ww],
                                                 hp[h0:h1, :ww], AF.Sigmoid)
                            nc.vector.tensor_tensor(
                                outs[li][h0:h1, :ww], sg[h0:h1, :ww],
                                hp[h0:h1, :ww], op=OP.mult)
                for hi, (o_, w_) in enumerate(pair):
                    cinfo.append((a3, hi, o_, w_))
            return cinfo

        for b in range(NBLK):
            ts0 = b * t_blk
            featsT = p1.tile([NB, eb], bf16, tag="featsT")
            ps_ = p1.tile([128, t_blk, 3], f32, tag="ps")
            pr_ = p1.tile([128, t_blk, 3], f32, tag="pr")
            sh_ = p1.tile([128, t_blk, 3], f32, tag="sh")
            nc.sync.dma_start(ps_[:], pos_s[:, ts0:ts0 + t_blk, :])
            nc.sync.dma_start(pr_[:], pos_r[:, ts0:ts0 + t_blk, :])
            nc.sync.dma_start(sh_[:], shfe[:, ts0:ts0 + t_blk, :])
            vec = p1.tile([128, t_blk, 3], f32, tag="vec")
            nc.vector.tensor_tensor(vec[:], pr_[:], ps_[:], op=OP.subtract)
            nc.vector.tensor_tensor(vec[:], vec[:], sh_[:], op=OP.add)
            sq = p1.tile([128, t_blk, 3], f32, tag="sq")
            nc.vector.tensor_tensor(sq[:], vec[:], vec[:], op=OP.mult)
            ln2 = p1.tile([128, t_blk], f32, tag="ln2")
            nc.vector.tensor_reduce(ln2[:], sq[:], axis=mybir.AxisListType.X,
                                    op=OP.add)
            ln = p1.tile([128, t_blk], f32, tag="ln")
            nc.scalar.activation(ln[:], ln2[:], AF.Sqrt, bias=eps12[:])
            rl = p1.tile([128, t_blk], f32, tag="rl")
            nc.vector.reciprocal(rl[:], ln[:])
            rl3 = rl[:].unsqueeze(-1).broadcast_to([128, t_blk, 3])
            nc.vector.scalar_tensor_tensor(
                yv_all[:, ts0:ts0 + t_blk, :], vec[:], SQ3, rl3,
                op0=OP.mult, op1=OP.mult)
            kb = wf["kpi5"][:].unsqueeze(1).broadcast_to([128, t_blk, NB])
            lnb = ln[:].unsqueeze(-1).broadcast_to([128, t_blk, NB])
            rlb = rl[:].unsqueeze(-1).broadcast_to([128, t_blk, NB])
            arg = p1.tile([128, t_blk, NB], f32, tag="arg")
            nc.vector.tensor_tensor(arg[:], kb, lnb, op=OP.mult)
            yq = p1.tile([128, t_blk, NB], f32, tag="yq")
            nc.vector.tensor_scalar(yq[:], arg[:], 1.0 / (2 * np.pi), None,
                                    op0=OP.mult)
            yqi = p1.tile([128, t_blk, NB], i32, tag="yqi")
            nc.vector.tensor_copy(yqi[:], yq[:])
            nc.vector.tensor_copy(yq[:], yqi[:])
            # r = arg - 2pi*k is in (-pi, 2pi) whether k was trunc or round;
            # fold the (pi, 2pi) tail back by another 2pi
            nc.vector.scalar_tensor_tensor(arg[:], yq[:], -2.0 * np.pi,
                                           arg[:], op0=OP.mult, op1=OP.add)
            nc.vector.tensor_scalar(yq[:], arg[:], np.pi, None, op0=OP.is_gt)
            nc.vector.scalar_tensor_tensor(arg[:], yq[:], -2.0 * np.pi,
                                           arg[:], op0=OP.mult, op1=OP.add)
            snb = p1.tile([128, t_blk, NB], f32, tag="snb")
            nc.scalar.activation(snb[:], arg[:], AF.Sin)
            bes = p1.tile([128, t_blk, NB], f32, tag="bes")
            nc.vector.scalar_tensor_tensor(bes[:], snb[:], (2.0 / RMAX) ** 0.5,
                                           rlb, op0=OP.mult, op1=OP.mult)
            u = p1.tile([128, t_blk], f32, tag="u")
            nc.vector.tensor_scalar(u[:], ln[:], 1.0 / RMAX, None, op0=OP.mult)
            u2 = p1.tile([128, t_blk], f32, tag="u2")
            nc.vector.tensor_tensor(u2[:], u[:], u[:], op=OP.mult)
            u4 = p1.tile([128, t_blk], f32, tag="u4")
            nc.vector.tensor_tensor(u4[:], u2[:], u2[:], op=OP.mult)
            u5 = p1.tile([128, t_blk], f32, tag="u5")
            nc.vector.tensor_tensor(u5[:], u4[:], u[:], op=OP.mult)
            w_ = p1.tile([128, t_blk], f32, tag="w_")
            nc.vector.tensor_scalar(w_[:], u[:], -15.0, 35.0, op0=OP.mult,
                                    op1=OP.add)
            nc.vector.tensor_tensor(w_[:], w_[:], u[:], op=OP.mult)
            nc.vector.tensor_scalar(w_[:], w_[:], -21.0, None, op0=OP.add)
            nc.vector.tensor_tensor(w_[:], w_[:], u5[:], op=OP.mult)
            nc.vector.tensor_scalar(w_[:], w_[:], 1.0, None, op0=OP.add)
            msk = p1.tile([128, t_blk], f32, tag="msk")
            nc.vector.tensor_scalar(msk[:], u[:], 1.0, None, op0=OP.is_lt)
            fc = p1.tile([128, t_blk], f32, tag="fc")
            nc.vector.tensor_tensor(fc[:], w_[:], msk[:], op=OP.mult)
            fcb = fc[:].unsqueeze(-1).broadcast_to([128, t_blk, NB])
            fts = p1.tile([128, t_blk, NB], bf16, tag="fts")
            nc.vector.tensor_tensor(fts[:], bes[:], fcb, op=OP.mult)
            for t in range(t_blk):
                fp = psL.tile([NB, 128], bf16, tag="w1p")
                nc.tensor.transpose(fp[:], fts[:, t, :], ident[:])
                nc.scalar.copy(featsT[:, t * 128:(t + 1) * 128], fp[:])
            nc.sync.dma_start(featsd[:, b * eb:(b + 1) * eb], featsT[:])
            if debug and b == 0:
                nc.gpsimd.dma_start(dbg["dbg_fts"][:], fts[:])
                nc.gpsimd.dma_start(dbg["dbg_ln"][:], ln[:])
                nc.gpsimd.dma_start(dbg["dbg_arg"][:], arg[:])
                nc.gpsimd.dma_start(dbg["dbg_snb"][:], snb[:])
                nc.gpsimd.dma_start(dbg["dbg_bes"][:], bes[:])
                nc.gpsimd.dma_start(dbg["dbg_fc"][:], fc[:])
        # ---------------- phase 1b: MLP1 + messages + scatter ---------

        def load_feats(pool, e0, ebs, tag):
            ft = pool.tile([128, TSUB * 128], bf16, tag=tag)
            nc.sync.dma_start(ft[0:NB, :ebs], featsd[:, e0:e0 + ebs])
            nc.sync.dma_start(ft[64:64 + NB, :ebs], ft[0:NB, :ebs])
            return ft

        def find_chunk(cinfo, o):
            for a3, hi, o_, w_ in cinfo:
                if o_ <= o < o_ + w_:
                    return a3, hi, o_, w_
            raise AssertionError(o)

        subs = [(s0, min(TSUB, t_blk - s0)) for s0 in range(0, t_blk, TSUB)]

        for b in range(NBLK):
            acc1 = p1.tile([128, C4], f32, tag="acc1", bufs=2)
            for si, (s0, sw) in enumerate(subs):
                ebs = sw * 128
                ts0 = b * t_blk + s0
                e0 = b * eb + s0 * 128
                attrs_sb = p1.tile([NE, TSUB * 128], bf16, tag="attrs_sb")
                nc.sync.dma_start(attrs_sb[:NE, :ebs], attrsT_in[:, e0:e0 + ebs])
                ft = load_feats(p1, e0, ebs, "featsT1")
                cinfo = run_mlp(ft, ["Wr1_1", "Wr1_2", "Wr1_3"], psM, p1c,
                                "m1", ebs)
                uv = p1.tile([128, TSUB, 2, C], bf16, tag="uv")
                for t in range(sw):
                    o = t * 128
                    a3, hi, o_, _ = find_chunk(cinfo, o)
                    lo = o - o_
                    w1p = psL.tile([128, 2 * C], f32, tag="w1p")
                    nc.tensor.matmul(w1p[:],
                                     a3[hi * 64:(hi + 1) * 64, lo:lo + 128],
                                     w1o[hi * 64:(hi + 1) * 64, :],
                                     start=True, stop=True)
                    ep = psE.tile([128, C], f32, tag="ep")
                    nc.tensor.matmul(ep[:], attrs_sb[:NE, o:o + 128],
                                     embup[:], start=True, stop=True)
                    heb = p1c.tile([128, C], bf16, tag="heb")
                    nc.scalar.copy(heb[:], ep[:])
                    epb = heb[:].unsqueeze(1).broadcast_to([128, 2, C])
                    w1v = w1p[:].rearrange("p (x c) -> p x c", c=C)
                    nc.vector.tensor_tensor(uv[:, t, :, :], w1v, epb, op=OP.mult)
                rb = rcvb[:, ts0:ts0 + sw].unsqueeze(-1).broadcast_to(
                    [128, sw, 128])
                S = p1.tile([128, TSUB, 128], bf16, tag="S")
                nc.vector.tensor_tensor(S[:, :sw, :], iob[:, :sw, :], rb,
                                        op=OP.is_equal)
                Sy = p1.tile([128, TSUB, 3, 128], bf16, tag="Sy")
                for d in range(3):
                    yb = yv_all[:, ts0:ts0 + sw, d].unsqueeze(-1).broadcast_to(
                        [128, sw, 128])
                    nc.vector.tensor_tensor(Sy[:, :sw, d, :], S[:, :sw, :], yb,
                                            op=OP.mult)
                A1 = psA.tile([128, C4], f32, tag="A1")
                for t in range(sw):
                    nc.tensor.matmul(A1[:, 0:C], S[:, t, :], uv[:, t, 0, :],
                                     start=(t == 0), stop=(t == sw - 1),
                                     skip_group_check=True)
                for d in range(3):
                    for t in range(sw):
                        nc.tensor.matmul(A1[:, C * (1 + d):C * (2 + d)],
                                         Sy[:, t, d, :], uv[:, t, 1, :],
                                         start=(t == 0), stop=(t == sw - 1),
                                         skip_group_check=True)
                nc.sync.dma_start(sdram[:, ts0:ts0 + sw, :], S[:, :sw, :])
                nc.sync.dma_start(sydram[:, ts0:ts0 + sw, :, :],
                                  Sy[:, :sw, :, :])
                if si == 0:
                    nc.scalar.copy(acc1[:], A1[:])
                else:
                    nc.vector.tensor_tensor(acc1[:], A1[:], acc1[:], op=OP.add)
            nc.vector.tensor_copy(a1sb[:, b, :], acc1[:])

        for p in (psE, psL, psM, psA, p1c, p1):
            p.release()
        if debug:
            nc.gpsimd.dma_start(dbg["dbg_a1"][:], a1sb[:])
            nc.gpsimd.dma_start(dbg["dbg_ft"][:], featsd[:])

        # ---------------- phase 2: node layer 1 -----------------------
        n1 = tc.alloc_tile_pool(name="n1", bufs=1)
        n1t = tc.alloc_tile_pool(name="n1t", bufs=2)
        n1m = tc.alloc_tile_pool(name="n1m", bufs=1)
        psN = tc.alloc_tile_pool(name="psN", bufs=2, space="PSUM")

        def transpose_blocks(src, c0, name, pool, pspool):
            dst = pool.tile([C, NLOC], bf16, name=name, tag=name)
            for b in range(NBLK):
                tp = pspool.tile([128, 128], bf16, tag="tp")
                nc.tensor.transpose(tp[:], src[:, b, c0:c0 + C], ident[:])
                nc.scalar.copy(dst[:, b * 128:(b + 1) * 128], tp[:])
            return dst

        def mm_wide(name, lhsT, rhs_tile, pool, pspool, dtype=bf16):
            out = pool.tile([C, NLOC], dtype, name=name, tag=name)
            for o, w_ in nchunks:
                pm = pspool.tile([C, 512], f32, tag="mmw")
                nc.tensor.matmul(pm[:, :w_], lhsT, rhs_tile[:, o:o + w_],
                                 start=True, stop=True)
                nc.scalar.copy(out[:, o:o + w_], pm[:, :w_])
            return out

        AsT = transpose_blocks(a1sb[:], 0, "AsT", n1m, psN)
        AvT = [transpose_blocks(a1sb[:], C * (1 + d), f"AvT{d}", n1m, psN)
               for d in range(3)]
        AsL = mm_wide("AsL", wb["Wlin1_s"][:], AsT[:], n1m, psN)
        AvL = [mm_wide(f"AvL{d}", wb["Wlin1_v"][:], AvT[d][:], n1m, psN)
               for d in range(3)]
        Pw = {n: mm_wide("w" + n, wb[n][:], naTs[:], n1m, psN)
              for n in ["P1s1", "P1ss", "P1vv", "P1v1", "P1sv"]}
        sqs = n1m.tile([C, NLOC], bf16, name="sqs", tag="sqs")
        nc.scalar.square(sqs[:], AsL[:])
        vv = n1m.tile([C, NLOC], f32, name="vv", tag="vv")
        sqv = n1m.tile([C, NLOC], f32, name="sqv", tag="sqv")
        nc.scalar.square(vv[:], AvL[0][:])
        for d in (1, 2):
            nc.scalar.square(sqv[:], AvL[d][:])
            nc.vector.tensor_tensor(vv[:], vv[:], sqv[:], op=OP.add)
        Bs = n1m.tile([C, NLOC], bf16, name="Bs", tag="Bs")
        t0 = n1t.tile([C, NLOC], bf16, tag="t0")
        nc.vector.tensor_tensor(Bs[:], Pw["P1s1"][:], AsL[:], op=OP.mult)
        nc.vector.tensor_tensor(t0[:], Pw["P1ss"][:], sqs[:], op=OP.mult)
        nc.vector.tensor_tensor(Bs[:], Bs[:], t0[:], op=OP.add)
        t1 = n1t.tile([C, NLOC], bf16, tag="t0")
        nc.vector.tensor_tensor(t1[:], Pw["P1vv"][:], vv[:], op=OP.mult)
        nc.vector.tensor_tensor(Bs[:], Bs[:], t1[:], op=OP.add)
        gsk = n1m.tile([C, NLOC], bf16, name="gsk", tag="gsk")
        nc.vector.tensor_tensor(gsk[:], Pw["P1sv"][:], AsL[:], op=OP.mult)
        nc.vector.tensor_tensor(gsk[:], gsk[:], Pw["P1v1"][:], op=OP.add)
        Bv = []
        for d in range(3):
            bvd = n1m.tile([C, NLOC], bf16, name=f"Bv{d}", tag=f"Bv{d}")
            nc.vector.tensor_tensor(bvd[:], gsk[:], AvL[d][:], op=OP.mult)
            Bv.append(bvd)
        hvT = [mm_wide(f"hvT{d}", wb["Lp1_v"][:], Bv[d][:], n1, psN)
               for d in range(3)]
        tabsb = n1m.tile([128, NBLK, C4], bf16, name="tabsb", tag="tabsb")
        for b in range(NBLK):
            pm = psN.tile([128, C], f32, tag="tab")
            nc.tensor.matmul(pm[:], Bs[:, b * 128:(b + 1) * 128], combS[:],
                             start=True, stop=True)
            nc.scalar.copy(tabsb[:, b, 0:C], pm[:])
            for d in range(3):
                pm2 = psN.tile([128, C], f32, tag="tab")
                nc.tensor.matmul(pm2[:], Bv[d][:, b * 128:(b + 1) * 128],
                                 combV[:], start=True, stop=True)
                nc.scalar.copy(tabsb[:, b, C * (1 + d):C * (2 + d)], pm2[:])
        if debug:
            nc.gpsimd.dma_start(dbg["dbg_tab"][:], tabsb[:])
        bounce = dramp.tile([NLOC, C4], bf16, tag="bounce")
        nc.sync.dma_start(bounce[:].rearrange("(b p) c -> p b c", p=128),
                          tabsb[:])
        tabdram = dramp.tile([NCORES * NLOC, C4], bf16, addr_space="Shared",
                             tag="tabdram")
        nc.gpsimd.collective_compute(
            "AllGather", OP.bypass, replica_groups=[list(range(NCORES))],
            ins=[bounce[:]], outs=[tabdram[:]])
        psN.release()
        n1m.release()

        # ---- skip-TP (depends only on hvT): overlaps with phase 4 ----
        sc_all = n1.tile([128, NBLK, 3, C], bf16, name="sc_all", tag="sc_all")
        na_sb = n1.tile([128, NBLK, NE], f32, name="na_sb", tag="na_sb")
        nc.sync.dma_start(na_sb[:], na_in[:].rearrange("(b p) e -> p b e", p=128))
        n5a = tc.alloc_tile_pool(name="n5a", bufs=2)
        ps5a = tc.alloc_tile_pool(name="ps5a", bufs=1, space="PSUM")
        for b in range(NBLK):
            sc = n5a.tile([128, 3, C], f32, tag="sc5")
            gp = ps5a.tile([128, 3, C], f32, tag="gp5")
            for s in range(NE):
                for d in range(3):
                    nc.tensor.matmul(gp[:, d, :],
                                     hvT[d][:, b * 128:(b + 1) * 128],
                                     wsk[:, s, :], start=True, stop=True)
                if s == 0:
                    nc.vector.tensor_scalar(sc[:], gp[:], na_sb[:, b, 0:1],
                                            None, op0=OP.mult)
                else:
                    nc.vector.scalar_tensor_tensor(
                        sc[:], gp[:], na_sb[:, b, s:s + 1], sc[:],
                        op0=OP.mult, op1=OP.add)
            nc.vector.tensor_copy(sc_all[:, b, :, :], sc[:])

        # ---------------- phase 4: edges, layer 2 ---------------------
        p4 = tc.alloc_tile_pool(name="p4", bufs=2)
        p4c = tc.alloc_tile_pool(name="p4c", bufs=4)
        psA2 = tc.alloc_tile_pool(name="psA2", bufs=2, space="PSUM")
        psM2 = tc.alloc_tile_pool(name="psM2", bufs=2, space="PSUM")
        psL2 = tc.alloc_tile_pool(name="psL2", bufs=3, space="PSUM")

        for b in range(NBLK):
            acc2 = p4.tile([128, C4], f32, tag="acc2", bufs=2)
            for si, (s0, sw) in enumerate(subs):
                ebs = sw * 128
                ts0 = b * t_blk + s0
                e0 = b * eb + s0 * 128
                featsT2 = load_feats(p4, e0, ebs, "featsT2")
                gat = p4.tile([128, TSUB, C4], bf16, tag="gat")
                nc.gpsimd.dma_gather(
                    gat[:, :sw, :], tabdram[:],
                    sndi[:, e0 // 16:(e0 + ebs) // 16],
                    num_idxs=ebs, num_idxs_reg=ebs, elem_size=C4)
                gat4 = gat[:].rearrange("p t (x c) -> p t x c", c=C)
                cinfo = run_mlp(featsT2, ["Wr2_1", "Wr2_2", "Wr2_3"], psM2,
                                p4c, "m2", ebs)
                w2sb = p4.tile([128, TSUB, 4 * C], bf16, tag="w2sb", bufs=4)
                for t in range(sw):
                    o = t * 128
                    a3, hi, o_, _ = find_chunk(cinfo, o)
                    lo = o - o_
                    w2p = psL2.tile([128, 4 * C], f32, tag="w2p")
                    nc.tensor.matmul(w2p[:],
                                     a3[hi * 64:(hi + 1) * 64, lo:lo + 128],
                                     w2o[hi * 64:(hi + 1) * 64, :],
                                     start=True, stop=True)
                    nc.scalar.copy(w2sb[:, t, :], w2p[:])
                S = p4.tile([128, TSUB, 128], bf16, tag="S4", bufs=3)
                nc.sync.dma_start(S[:, :sw, :], sdram[:, ts0:ts0 + sw, :])
                Sy = p4.tile([128, TSUB, 3, 128], bf16, tag="Sy4", bufs=3)
                nc.sync.dma_start(Sy[:, :sw, :, :],
                                  sydram[:, ts0:ts0 + sw, :, :])
                m4 = p4.tile([128, TSUB, 4, C], bf16, tag="m4")
                m5 = p4.tile([128, TSUB, 4, C], bf16, tag="m5")
                hsg = gat4[:, :sw, 0, :]
                nc.vector.tensor_tensor(m4[:, :sw, 0, :], w2sb[:, :sw, 0:C],
                                        hsg, op=OP.mult)
                nc.vector.tensor_tensor(m5[:, :sw, 3, :],
                                        w2sb[:, :sw, 2 * C:3 * C], hsg,
                                        op=OP.mult)
                for d in range(3):
                    hvg = gat4[:, :sw, 1 + d, :]
                    nc.vector.tensor_tensor(m4[:, :sw, 1 + d, :],
                                            w2sb[:, :sw, 3 * C:4 * C], hvg,
                                            op=OP.mult)
                    nc.vector.tensor_tensor(m5[:, :sw, d, :],
                                            w2sb[:, :sw, C:2 * C], hvg,
                                            op=OP.mult)
                A2 = psA2.tile([128, 4, C], f32, tag="A2")
                for t in range(sw):
                    nc.tensor.matmul(A2[:], S[:, t, :],
                                     m4[:, t, :, :], start=(t == 0), stop=False,
                                     skip_group_check=True)
                for d in range(3):
                    for t in range(sw):
                        nc.tensor.matmul(A2[:, 1 + d, :], Sy[:, t, d, :],
                                         m5[:, t, 3, :], start=False,
                                         stop=False, skip_group_check=True)
                for d in range(3):
                    for t in range(sw):
                        sp = (t == sw - 1) and (d == 2)
                        nc.tensor.matmul(A2[:, 0, :], Sy[:, t, d, :],
                                         m5[:, t, d, :], start=False, stop=sp,
                                         skip_group_check=True)
                a2f = A2[:].rearrange("p x c -> p (x c)")
                if si == 0:
                    nc.scalar.copy(acc2[:], a2f)
                else:
                    nc.vector.tensor_tensor(acc2[:], a2f, acc2[:], op=OP.add)
            nc.vector.tensor_copy(a2sb[:, b, :], acc2[:])
            if debug and b == 0:
                nc.gpsimd.dma_start(dbg["dbg_gat"][:, 0:min(TSUB, t_blk)],
                                    gat[:, 0:min(TSUB, t_blk)])
                nc.gpsimd.dma_start(dbg["dbg_m4"][:, 0:min(TSUB, t_blk)],
                                    m4[:, 0:min(TSUB, t_blk)])

        for p in (psL2, psM2, psA2, p4c, p4, ps5a, n5a):
            p.release()
        if debug:
            nc.gpsimd.dma_start(dbg["dbg_a2"][:], a2sb[:])

        # ---------------- phase 5: node layer 2 + outputs -------------
        n2 = tc.alloc_tile_pool(name="n2", bufs=1)
        psN2 = tc.alloc_tile_pool(name="psN2", bufs=2, space="PSUM")
        A2sT = transpose_blocks(a2sb[:], 0, "A2sT", n2, psN2)
        A2vT = [transpose_blocks(a2sb[:], C * (1 + d), f"A2vT{d}", n2, psN2)
                for d in range(3)]
        A2sL = mm_wide("A2sL", wb["Wlin2_s"][:], A2sT[:], n2, psN2)
        A2vL = [mm_wide(f"A2vL{d}", wb["Wlin2_v"][:], A2vT[d][:], n2, psN2)
                for d in range(3)]
        P2w = {n: mm_wide("w" + n, wb[n][:], naTs[:], n2, psN2)
               for n in ["P2v1", "P2sv"]}
        g2 = n2.tile([C, NLOC], bf16, name="g2", tag="g2")
        nc.vector.tensor_tensor(g2[:], P2w["P2sv"][:], A2sL[:], op=OP.mult)
        nc.vector.tensor_tensor(g2[:], g2[:], P2w["P2v1"][:], op=OP.add)
        B2v = []
        for d in range(3):
            b2d = n2.tile([C, NLOC], bf16, name=f"B2v{d}", tag=f"B2v{d}")
            nc.vector.tensor_tensor(b2d[:], g2[:], A2vL[d][:], op=OP.mult)
            B2v.append(b2d)
        psN2.release()

        # h2 (node-major) = B2v @ Lp2_v + skip-TP, then back to feat-major
        n2t = tc.alloc_tile_pool(name="n2t", bufs=2)
        psH = tc.alloc_tile_pool(name="psH", bufs=1, space="PSUM")
        psHt = tc.alloc_tile_pool(name="psHt", bufs=2, space="PSUM")
        h2T = [n2.tile([C, NLOC], bf16, name=f"h2T{d}", tag=f"h2T{d}")
               for d in range(3)]
        for b in range(NBLK):
            hp = psH.tile([128, 3, C], f32, tag="h2p")
            for d in range(3):
                nc.tensor.matmul(hp[:, d, :], B2v[d][:, b * 128:(b + 1) * 128],
                                 wb["Lp2_v"][:], start=True, stop=True)
            sc16 = n2t.tile([128, 3, C], bf16, tag="sc16")
            nc.vector.tensor_tensor(sc16[:], hp[:], sc_all[:, b, :, :],
                                    op=OP.add)
            for d in range(3):
                tp = psHt.tile([128, 128], bf16, tag="tph")
                nc.tensor.transpose(tp[:], sc16[:, d, :], ident[:])
                nc.scalar.copy(h2T[d][:, b * 128:(b + 1) * 128], tp[:])
        psHt.release()
        psH.release()

        psD = tc.alloc_tile_pool(name="psD", bufs=2, space="PSUM")
        dipf = [n2.tile([1, NLOC], f32, name=f"dipf{d}", tag=f"dipf{d}")
                for d in range(3)]
        for d in range(3):
            for o, w_ in nchunks:
                dp = psD.tile([1, 512], f32, tag="dp")
                nc.tensor.matmul(dp[:, :w_], wb["R1c"][:], hvT[d][:, o:o + w_],
                                 start=True, stop=False)
                nc.tensor.matmul(dp[:, :w_], qcol[:], h2T[d][:, o:o + w_],
                                 start=False, stop=True)
                nc.scalar.copy(dipf[d][0:1, o:o + w_], dp[:, :w_])

        posb = n2.tile([128, NBLK, 3], f32, name="posb", tag="posb")
        nc.sync.dma_start(posb[:], posn_in[:].rearrange("(b p) c -> p b c", p=128))
        chgb = n2.tile([128, NBLK], f32, name="chgb", tag="chgb")
        nc.sync.dma_start(chgb[:], chg_in[:].rearrange("(b p) c -> p (b c)", p=128))
        bohb = n2.tile([128, NBLK, G], f32, name="bohb", tag="bohb")
        nc.sync.dma_start(bohb[:], boh_in[:].rearrange("(b p) g -> p b g", p=128))
        dipo = n2.tile([128, NBLK, 3], f32, name="dipo", tag="dipo")
        gs = psD.tile([G, 3], f32, tag="gs", bufs=1)
        for b in range(NBLK):
            for d in range(3):
                dpp = psD.tile([128, 1], f32, tag="dpp")
                nc.tensor.transpose(dpp[:], dipf[d][0:1, b * 128:(b + 1) * 128],
                                    identf[0:1, 0:1])
                nc.scalar.copy(dipo[:, b, d:d + 1], dpp[:])
            cp = n2t.tile([128, 3], f32, tag="cp")
            nc.vector.tensor_scalar(cp[:], posb[:, b, :], chgb[:, b:b + 1],
                                    None, op0=OP.mult)
            nc.vector.tensor_tensor(cp[:], cp[:], dipo[:, b, :], op=OP.add)
            nc.tensor.matmul(gs[:], bohb[:, b, :], cp[:], start=(b == 0),
                             stop=(b == NBLK - 1))
        nc.sync.dma_start(out_dip[:].rearrange("(b p) c -> p b c", p=128),
                          dipo[:])
        if debug:
            nc.gpsimd.dma_start(dbg["dbg_hvT0"][:], hvT[0][:])
            nc.gpsimd.dma_start(dbg["dbg_Bs"][:], Bs[:])
            nc.gpsimd.dma_start(dbg["dbg_h2T0"][:], h2T[0][:])
            nc.gpsimd.dma_start(dbg["dbg_dipf0"][:], dipf[0][:])
        gso = n2.tile([G, 3], f32, name="gso", tag="gso")
        nc.scalar.copy(gso[:], gs[:])
        nc.sync.dma_start(out_gs[:], gso[:])

        for p in (psD, n2t, n2, n1t, n1):
            p.release()
        dramp.release()
        wp.release()

    nc.compile()
    return nc


_BUILD_CACHE = {}


def kernel(**inputs):
    in_maps, t_blk = prep_shards(inputs)
    nc = _BUILD_CACHE.get(t_blk)
    if nc is None:
        nc = build(t_blk)
        _BUILD_CACHE[t_blk] = nc
    res = run_bass_kernel_spmd(nc, in_maps, core_ids=list(range(NCORES)))
    dip = np.zeros((N, 3), np.float32)
    tot = np.zeros((G, 3), np.float32)
    for k in range(NCORES):
        dip[k * NPC:(k + 1) * NPC] = res.results[k]["dip"][:NPC]
        tot += res.results[k]["gsum"]
    return tot, dip


# revision 55
# speedup vs baseline: 1.1746x; 1.0045x over previous
"""AtomicDipolesMACE on 8 TRN2 NeuronCores.

Sharding: nodes are split into 8 contiguous ranges of 1250 (padded to 1280 =
10 blocks of 128). Each core owns the edges whose receiver falls in its range,
sorted and grouped by 128-node receiver block, each block segment padded to a
uniform T_BLK tiles of 128 edges. Scatters become per-block one-hot matmuls
(PSUM f32 accumulation, Yv folded into scaled one-hots); the layer-2 sender
gather is a dma_gather from an AllGathered bf16 node-feature table.
"""
import sys
sys.path.insert(0, "/opt/trn_rl_repo")
import numpy as np
import ml_dtypes

from concourse import bass, bacc, tile, mybir
from concourse.bass_utils import run_bass_kernel_spmd

f32 = mybir.dt.float32
bf16 = mybir.dt.bfloat16
i16 = mybir.dt.int16
i32 = mybir.dt.int32
AF = mybir.ActivationFunctionType
OP = mybir.AluOpType
nbf16 = ml_dtypes.bfloat16
USE_SILU = True
PACK_MLP = True
TSUB = 6

N, E, C, NE, G, NB = 10000, 160000, 128, 10, 16, 8
RMAX, AVG, SQ3 = 5.0, 16.0, 3.0 ** 0.5
NCORES = 8
NPC = N // NCORES          # 1250 real nodes per core
NBLK = 10                  # node blocks per core
NLOC = NBLK * 128          # 1280 padded local nodes
C4 = 4 * C                 # 512: table row / message width

WSHAPE = dict(W_embT=[C, NE], W_up1=[C, C], Wr1_1=[NB, 64], Wr1_2=[64, 64],
              Wr1_3=[64, 64], Wr1_o=[64, 2 * C], Wr2_1=[NB, 64],
              Wr2_2=[64, 64], Wr2_3=[64, 64], Wr2_o=[64, 4 * C],
              Wlin1_s=[C, C], Wlin1_v=[C, C], Wlin2_s=[C, C],
              Wlin2_v=[C, C], Lp1_sT=[C, C], Lp1_vT=[C, C], Lp1_v=[C, C],
              Lp2_v=[C, C], Wup2_s=[C, C], Wup2_v=[C, C],
              P1s1=[NE, C], P1ss=[NE, C], P1vv=[NE, C], P1v1=[NE, C],
              P1sv=[NE, C], P2v1=[NE, C], P2sv=[NE, C],
              R1c=[C, 1], RmidT=[16, C], Routc=[16, 1], kpi5=[128, NB])


# ---------------------------------------------------------------- host prep
def prep_shards(inp):
    snd = np.asarray(inp["edge_index"][0], dtype=np.int64)
    rcv = np.asarray(inp["edge_index"][1], dtype=np.int64)
    pos = np.asarray(inp["positions"], dtype=np.float32)
    na = np.asarray(inp["node_attrs"], dtype=np.float32)
    shf = np.asarray(inp["shifts"], dtype=np.float32)
    chg = np.asarray(inp["charges"], dtype=np.float32)
    bat = np.asarray(inp["batch"], dtype=np.int64)

    core = rcv // NPC
    loc = rcv - core * NPC
    blk = loc // 128

    order = np.lexsort((blk, core))
    snd_s, rcv_s, loc_s = snd[order], rcv[order], loc[order]
    core_s, blk_s = core[order], blk[order]
    shf_s = shf[order]

    counts = np.zeros((NCORES, NBLK), dtype=np.int64)
    np.add.at(counts, (core_s, blk_s), 1)
    t_blk = max(1, int(np.max((counts + 127) // 128)))
    eb = t_blk * 128
    epad = NBLK * eb
    tt = NBLK * t_blk

    snd_pad = (snd_s // NPC) * NLOC + (snd_s % NPC)  # padded-table row index

    starts = np.zeros(NCORES * NBLK, dtype=np.int64)
    starts[1:] = np.cumsum(counts.reshape(-1))[:-1]
    starts = starts.reshape(NCORES, NBLK)

    w = weights_prep(inp)
    in_maps = []
    for k in range(NCORES):
        pos_s = np.zeros((epad, 3), np.float32)
        pos_r = np.ones((epad, 3), np.float32)
        shfe = np.zeros((epad, 3), np.float32)
        attrsT = np.zeros((NE, epad), np.float32)
        sndp = np.zeros(epad, np.int64)
        rcvb = -np.ones(epad, np.float32)
        for b in range(NBLK):
            s0, n = starts[k, b], counts[k, b]
            sl = slice(s0, s0 + n)
            d0 = b * eb
            pos_s[d0:d0 + n] = pos[snd_s[sl]]
            pos_r[d0:d0 + n] = pos[rcv_s[sl]]
            shfe[d0:d0 + n] = shf_s[sl]
            attrsT[:, d0:d0 + n] = na[snd_s[sl]].T
            sndp[d0:d0 + n] = snd_pad[sl]
            rcvb[d0:d0 + n] = (loc_s[sl] - b * 128).astype(np.float32)

        def emaj(a):  # [epad, d] -> [128, tt, d]  (edge i -> (i%128, i//128))
            return np.ascontiguousarray(a.reshape(tt, 128, -1).transpose(1, 0, 2))

        sndw = np.zeros((128, epad // 16), np.int16)
        for b in range(NBLK):
            seg = sndp[b * eb:(b + 1) * eb].reshape(eb // 16, 16).T
            sndw[:, b * (eb // 16):(b + 1) * (eb // 16)] = np.tile(seg, (8, 1))

        nl0 = k * NPC
        na_nm = np.zeros((NLOC, NE), np.float32)
        na_nm[:NPC] = na[nl0:nl0 + NPC]
        pos_nm = np.zeros((NLOC, 3), np.float32)
        pos_nm[:NPC] = pos[nl0:nl0 + NPC]
        chg_nm = np.zeros((NLOC, 1), np.float32)
        chg_nm[:NPC, 0] = chg[nl0:nl0 + NPC]
        boh = np.zeros((NLOC, G), np.float32)
        boh[np.arange(NPC), bat[nl0:nl0 + NPC]] = 1.0

        m = dict(
            pos_s=emaj(pos_s), pos_r=emaj(pos_r), shfe=emaj(shfe),
            attrsT=attrsT.astype(nbf16), sndw=sndw,
            rcvb=emaj(rcvb)[:, :, 0].astype(nbf16),
            na_nm=na_nm, naT=np.ascontiguousarray(na_nm.T).astype(nbf16),
            pos_nm=pos_nm, chg_nm=chg_nm, boh=boh,
        )
        m.update(w)
        in_maps.append(m)
    return in_maps, t_blk


def weights_prep(inp):
    g = lambda k: np.ascontiguousarray(np.asarray(inp[k], dtype=np.float32))
    kvec = (np.pi / RMAX) * np.arange(1, NB + 1, dtype=np.float32)
    return dict(
        W_embT=g("W_emb").T.copy(), W_up1=g("W_up1"),
        Wr1_1=g("Wr1_1"), Wr1_2=g("Wr1_2"), Wr1_3=g("Wr1_3"), Wr1_o=g("Wr1_o"),
        Wr2_1=g("Wr2_1"), Wr2_2=g("Wr2_2"), Wr2_3=g("Wr2_3"), Wr2_o=g("Wr2_o"),
        Wlin1_s=g("Wlin1_s"), Wlin1_v=g("Wlin1_v"),
        Wlin2_s=g("Wlin2_s"), Wlin2_v=g("Wlin2_v"),
        Lp1_sT=g("Lp1_s").T.copy(), Lp1_vT=g("Lp1_v").T.copy(),
        Lp1_v=g("Lp1_v"), Lp2_v=g("Lp2_v"),
        Wup2_s=g("Wup2_s"), Wup2_v=g("Wup2_v"),
        P1s1=g("P1_s1"), P1ss=g("P1_ss"), P1vv=g("P1_vv"),
        P1v1=g("P1_v1"), P1sv=g("P1_sv"), P2v1=g("P2_v1"), P2sv=g("P2_sv"),
        Wsk=g("Wsk"),
        R1c=g("R1").reshape(C, 1), RmidT=g("Rmid").T.copy(),
        Routc=g("Rout").reshape(16, 1),
        kpi5=np.tile(kvec, (128, 1)),
    )


# ---------------------------------------------------------------- builder
def build(t_blk, debug=False):
    eb = t_blk * 128
    tt = NBLK * t_blk
    epad = NBLK * eb
    nc = bacc.Bacc(None, target_bir_lowering=False, num_devices=NCORES)

    def din(name, shape, dt=f32):
        return nc.declare_dram_parameter(name, shape, dt, isOutput=False)

    pos_s = din("pos_s", [128, tt, 3]); pos_r = din("pos_r", [128, tt, 3])
    shfe = din("shfe", [128, tt, 3])
    attrsT_in = din("attrsT", [NE, epad], bf16)
    sndw_in = din("sndw", [128, epad // 16], i16)
    rcvb_in = din("rcvb", [128, tt], bf16)
    na_in = din("na_nm", [NLOC, NE]); naT_in = din("naT", [NE, NLOC], bf16)
    posn_in = din("pos_nm", [NLOC, 3]); chg_in = din("chg_nm", [NLOC, 1])
    boh_in = din("boh", [NLOC, G])
    win = {n: din(n, WSHAPE[n]) for n in WSHAPE}
    wsk_in = din("Wsk", [NE, C, C])
    out_dip = nc.declare_dram_parameter("dip", [NLOC, 3], f32, isOutput=True)
    out_gs = nc.declare_dram_parameter("gsum", [16, 3], f32, isOutput=True)
    dbg = {}
    if debug:
        for nm, shp in [("dbg_a1", [128, NBLK, C4]), ("dbg_tab", [128, NBLK, C4]),
                        ("dbg_a2", [128, NBLK, C4]), ("dbg_hvT0", [C, NLOC]),
                        ("dbg_Bs", [C, NLOC]), ("dbg_h2T0", [C, NLOC]),
                        ("dbg_dipf0", [1, NLOC]), ("dbg_gat", [128, t_blk, C4]),
                        ("dbg_m4", [128, t_blk, 4, C]),
                        ("dbg_ft", [NB, epad]), ("dbg_uv", [128, t_blk, 2, C]),
                        ("dbg_S", [128, t_blk, 128]),
                        ("dbg_fts", [128, t_blk, NB]),
                        ("dbg_ln", [128, t_blk]), ("dbg_arg", [128, t_blk, NB]),
                        ("dbg_snb", [128, t_blk, NB]),
                        ("dbg_bes", [128, t_blk, NB]),
                        ("dbg_fc", [128, t_blk])]:
            dbg[nm] = nc.declare_dram_parameter(nm, shp, f32, isOutput=True)

    nchunks = []
    _o = 0
    while _o < NLOC:
        nchunks.append((_o, min(512, NLOC - _o)))
        _o += nchunks[-1][1]

    def echunks(ebs):
        half = ebs // 2
        if ebs <= 1024 and half % 128 == 0 and half > 0:
            return [(0, half), (half, ebs - half)]
        o, out = 0, []
        while o < ebs:
            w_ = min(512, ebs - o)
            out.append((o, w_))
            o += w_
        return out

    with tile.TileContext(nc) as tc:
        wp = tc.alloc_tile_pool(name="wp", bufs=1)
        dramp = tc.alloc_tile_pool(name="dram", bufs=1, space="DRAM")

        # ---- load + prep weights -------------------------------------
        wf = {}
        for n in WSHAPE:
            wf[n] = wp.tile(WSHAPE[n], f32, name="f" + n, tag="f" + n)
            nc.sync.dma_start(wf[n][:], win[n][:])
        wsk = wp.tile([C, NE, C], bf16, name="wsk", tag="wsk")
        nc.gpsimd.dma_start(wsk[:], wsk_in[:].transpose([1, 0, 2]))
        naTs = wp.tile([NE, NLOC], bf16, name="naTs", tag="naTs")
        nc.sync.dma_start(naTs[:], naT_in[:])
        rcvb = wp.tile([128, tt], bf16, name="rcvb", tag="rcvb")
        nc.sync.dma_start(rcvb[:], rcvb_in[:])
        sndi = wp.tile([128, epad // 16], i16, name="sndi", tag="sndi")
        nc.sync.dma_start(sndi[:], sndw_in[:])

        def tobf(name, src_ap, shape, scale=None):
            t = wp.tile(shape, bf16, name=name, tag=name)
            if scale is None:
                nc.vector.tensor_copy(t[:], src_ap)
            else:
                nc.vector.tensor_scalar(t[:], src_ap, float(scale), None,
                                        op0=OP.mult)
            return t

        wb = {}
        for n in ["Wr1_1", "Wr1_2", "Wr1_3", "Wr1_o", "Wr2_1", "Wr2_2",
                  "Wr2_3", "Lp1_v", "Lp2_v", "P1s1", "P1ss", "P1v1", "P1sv",
                  "P2v1", "P2sv", "R1c"]:
            wb[n] = tobf("b" + n, wf[n][:], WSHAPE[n])
        for n in ["Wlin1_s", "Wlin1_v", "Wlin2_s", "Wlin2_v"]:
            wb[n] = tobf("b" + n, wf[n][:], WSHAPE[n], scale=1.0 / AVG)
        wb["P1vv"] = tobf("bP1vv", wf["P1vv"][:], WSHAPE["P1vv"], scale=1.0 / SQ3)
        w2o = wp.tile([128, 4 * C], bf16, name="w2o", tag="w2o")
        for h in (0, 64):
            nc.vector.tensor_copy(w2o[h:h + 64, :], wf["Wr2_o"][:])
            nc.vector.tensor_scalar(w2o[h:h + 64, C:2 * C],
                                    wf["Wr2_o"][:, C:2 * C],
                                    1.0 / SQ3, None, op0=OP.mult)
        w1o = wp.tile([128, 2 * C], bf16, name="w1o", tag="w1o")
        for h in (0, 64):
            nc.vector.tensor_copy(w1o[h:h + 64, :], wf["Wr1_o"][:])
        # hidden-layer MLP weights duplicated into the upper PE quadrant
        wb2 = {}
        for n in ["Wr1_1", "Wr1_2", "Wr1_3", "Wr2_1", "Wr2_2", "Wr2_3"]:
            kk = WSHAPE[n][0]
            t = wp.tile([128, 64], bf16, name="q" + n, tag="q" + n)
            for h in (0, 64):
                nc.vector.tensor_copy(t[h:h + kk, :], wf[n][:])
            wb2[n] = t

        psw = tc.alloc_tile_pool(name="psw", bufs=2, space="PSUM")
        eup = psw.tile([NE, C], f32, tag="pw")
        nc.tensor.matmul(eup[:], wf["W_embT"][:], wf["W_up1"][:],
                         start=True, stop=True)
        embup = tobf("embup", eup[:], [NE, C])
        cs = psw.tile([C, C], f32, tag="pw")
        nc.tensor.matmul(cs[:], wf["Lp1_sT"][:], wf["Wup2_s"][:],
                         start=True, stop=True)
        combS = tobf("combS", cs[:], [C, C])
        cv = psw.tile([C, C], f32, tag="pw")
        nc.tensor.matmul(cv[:], wf["Lp1_vT"][:], wf["Wup2_v"][:],
                         start=True, stop=True)
        combV = tobf("combV", cv[:], [C, C])
        qp = psw.tile([C, 1], f32, tag="pw")
        nc.tensor.matmul(qp[:], wf["RmidT"][:], wf["Routc"][:],
                         start=True, stop=True)
        qcol = wp.tile([C, 1], bf16, name="qcol", tag="qcol")
        nc.vector.tensor_scalar(qcol[:], qp[:], 0.5, None, op0=OP.mult)

        ioi = wp.tile([128, t_blk, 128], i32, name="ioi", tag="ioi")
        nc.gpsimd.iota(ioi[:], pattern=[[0, t_blk], [1, 128]], base=0,
                       channel_multiplier=0)
        iob = wp.tile([128, t_blk, 128], bf16, name="iob", tag="iob")
        nc.vector.tensor_copy(iob[:], ioi[:])
        idi = wp.tile([128, 128], i32, name="idi", tag="idi")
        nc.gpsimd.iota(idi[:], pattern=[[1, 128]], base=0, channel_multiplier=-1)
        idf = wp.tile([128, 128], f32, name="idf", tag="idf")
        nc.vector.tensor_copy(idf[:], idi[:])
        ident = wp.tile([128, 128], bf16, name="ident", tag="ident")
        nc.vector.tensor_scalar(ident[:], idf[:], 0.0, None, op0=OP.is_equal)
        identf = wp.tile([128, 128], f32, name="identf", tag="identf")
        nc.vector.tensor_scalar(identf[:], idf[:], 0.0, None, op0=OP.is_equal)
        psw.release()

        eps12 = wp.tile([128, 1], f32, name="eps12", tag="eps12")
        nc.vector.memset(eps12[:], 1e-12)
        mpi = wp.tile([128, 1], f32, name="mpi", tag="mpi")
        nc.vector.memset(mpi[:], -np.pi)
        yv_all = wp.tile([128, tt, 3], bf16, name="yv_all", tag="yv_all")
        a1sb = wp.tile([128, NBLK, C4], bf16, name="a1sb", tag="a1sb")
        a2sb = wp.tile([128, NBLK, C4], bf16, name="a2sb", tag="a2sb")

        # ---------------- phase 1: edges, layer 1 ---------------------
        featsd = dramp.tile([NB, epad], bf16, tag="featsd")
        sdram = dramp.tile([128, tt, 128], bf16, tag="sdram")
        sydram = dramp.tile([128, tt, 3, 128], bf16, tag="sydram")
        p1 = tc.alloc_tile_pool(name="p1", bufs=2)
        p1c = tc.alloc_tile_pool(name="p1c", bufs=3)
        psA = tc.alloc_tile_pool(name="psA", bufs=2, space="PSUM")
        psM = tc.alloc_tile_pool(name="psM", bufs=2, space="PSUM")
        psL = tc.alloc_tile_pool(name="psL", bufs=2, space="PSUM")
        psE = tc.alloc_tile_pool(name="psE", bufs=2, space="PSUM")

        def run_mlp(ftile, names, psum_pool, act_pool, tagp, ebs):
            """3-layer silu MLP over ebs edges, partition-packed pairs.
            Returns per-512-chunk (a3_tile, half, chunk_off, width)."""
            offs = echunks(ebs)
            cinfo = []
            step = 2 if PACK_MLP else 1
            for pc in range(0, len(offs), step):
                pair = offs[pc:pc + step]
                cw = max(w_ for _, w_ in pair)
                npart = 64 * len(pair)
                a1 = act_pool.tile([128, cw], bf16, tag=tagp + "a1")
                a2 = act_pool.tile([128, cw], bf16, tag=tagp + "a2")
                a3 = act_pool.tile([128, cw], bf16, tag=tagp + "a3")
                prevs = [None, a1, a2]
                outs = [a1, a2, a3]
                for li in range(3):
                    hp = psum_pool.tile([128, cw], f32, tag=tagp + "h")
                    kk = NB if li == 0 else 64
                    for hi, (o_, w_) in enumerate(pair):
                        h = hi * 64
                        if li == 0:
                            rhs = ftile[h:h + NB, o_: o_ + w_]
                        else:
                            rhs = prevs[li][h:h + 64, :w_]
                        nc.tensor.matmul(hp[h:h + 64, :w_],
                                         wb2[names[li]][h:h + kk, :], rhs,
                                         start=True, stop=True)
                    full = len(pair) == 2 and pair[0][1] == pair[1][1]
                    regions = ([(0, 128, cw)] if full else
                               [(hi * 64, hi * 64 + 64, w_)
                                for hi, (o_, w_) in enumerate(pair)])
                    if USE_SILU:
                        for (h0, h1, ww) in regions:
                            nc.scalar.activation(outs[li][h0:h1, :ww],
                                                 hp[h0:h1, :ww], AF.Silu)
                    else:
                        sg = act_pool.tile([128, cw], bf16, tag=tagp + "sg")
                        for (h0, h1, ww) in regions:
                            nc.scalar.activation(sg[h0:h1, :ww],
                                                 hp[h0:h1, :ww], AF.Sigmoid)
                            nc.vector.tensor_tensor(
                                outs[li][h0:h1, :ww], sg[h0:h1, :ww],
                                hp[h0:h1, :ww], op=OP.mult)
                for hi, (o_, w_) in enumerate(pair):
                    cinfo.append((a3, hi, o_, w_))
            return cinfo

        for b in range(NBLK):
            ts0 = b * t_blk
            featsT = p1.tile([NB, eb], bf16, tag="featsT")
            ps_ = p1.tile([128, t_blk, 3], f32, tag="ps")
            pr_ = p1.tile([128, t_blk, 3], f32, tag="pr")
            sh_ = p1.tile([128, t_blk, 3], f32, tag="sh")
            nc.sync.dma_start(ps_[:], pos_s[:, ts0:ts0 + t_blk, :])
            nc.sync.dma_start(pr_[:], pos_r[:, ts0:ts0 + t_blk, :])
            nc.sync.dma_start(sh_[:], shfe[:, ts0:ts0 + t_blk, :])
            vec = p1.tile([128, t_blk, 3], f32, tag="vec")
            nc.vector.tensor_tensor(vec[:], pr_[:], ps_[:], op=OP.subtract)
            nc.vector.tensor_tensor(vec[:], vec[:], sh_[:], op=OP.add)
            sq = p1.tile([128, t_blk, 3], f32, tag="sq")
            nc.vector.tensor_tensor(sq[:], vec[:], vec[:], op=OP.mult)
            ln2 = p1.tile([128, t_blk], f32, tag="ln2")
            nc.vector.tensor_reduce(ln2[:], sq[:], axis=mybir.AxisListType.X,
                                    op=OP.add)
            ln = p1.tile([128, t_blk], f32, tag="ln")
            nc.scalar.activation(ln[:], ln2[:], AF.Sqrt, bias=eps12[:])
            rl = p1.tile([128, t_blk], f32, tag="rl")
            nc.vector.reciprocal(rl[:], ln[:])
            rl3 = rl[:].unsqueeze(-1).broadcast_to([128, t_blk, 3])
            nc.vector.scalar_tensor_tensor(
                yv_all[:, ts0:ts0 + t_blk, :], vec[:], SQ3, rl3,
                op0=OP.mult, op1=OP.mult)
            kb = wf["kpi5"][:].unsqueeze(1).broadcast_to([128, t_blk, NB])
            lnb = ln[:].unsqueeze(-1).broadcast_to([128, t_blk, NB])
            rlb = rl[:].unsqueeze(-1).broadcast_to([128, t_blk, NB])
            arg = p1.tile([128, t_blk, NB], f32, tag="arg")
            nc.vector.tensor_tensor(arg[:], kb, lnb, op=OP.mult)
            yq = p1.tile([128, t_blk, NB], f32, tag="yq")
            nc.vector.tensor_scalar(yq[:], arg[:], 1.0 / (2 * np.pi), None,
                                    op0=OP.mult)
            yqi = p1.tile([128, t_blk, NB], i32, tag="yqi")
            nc.vector.tensor_copy(yqi[:], yq[:])
            nc.vector.tensor_copy(yq[:], yqi[:])
            # r = arg - 2pi*k is in (-pi, 2pi) whether k was trunc or round;
            # fold the (pi, 2pi) tail back by another 2pi
            nc.vector.scalar_tensor_tensor(arg[:], yq[:], -2.0 * np.pi,
                                           arg[:], op0=OP.mult, op1=OP.add)
            nc.vector.tensor_scalar(yq[:], arg[:], np.pi, None, op0=OP.is_gt)
            nc.vector.scalar_tensor_tensor(arg[:], yq[:], -2.0 * np.pi,
                                           arg[:], op0=OP.mult, op1=OP.add)
            snb = p1.tile([128, t_blk, NB], f32, tag="snb")
            nc.scalar.activation(snb[:], arg[:], AF.Sin)
            bes = p1.tile([128, t_blk, NB], f32, tag="bes")
            nc.vector.scalar_tensor_tensor(bes[:], snb[:], (2.0 / RMAX) ** 0.5,
                                           rlb, op0=OP.mult, op1=OP.mult)
            u = p1.tile([128, t_blk], f32, tag="u")
            nc.vector.tensor_scalar(u[:], ln[:], 1.0 / RMAX, None, op0=OP.mult)
            u2 = p1.tile([128, t_blk], f32, tag="u2")
            nc.vector.tensor_tensor(u2[:], u[:], u[:], op=OP.mult)
            u4 = p1.tile([128, t_blk], f32, tag="u4")
            nc.vector.tensor_tensor(u4[:], u2[:], u2[:], op=OP.mult)
            u5 = p1.tile([128, t_blk], f32, tag="u5")
            nc.vector.tensor_tensor(u5[:], u4[:], u[:], op=OP.mult)
            w_ = p1.tile([128, t_blk], f32, tag="w_")
            nc.vector.tensor_scalar(w_[:], u[:], -15.0, 35.0, op0=OP.mult,
                                    op1=OP.add)
            nc.vector.tensor_tensor(w_[:], w_[:], u[:], op=OP.mult)
            nc.vector.tensor_scalar(w_[:], w_[:], -21.0, None, op0=OP.add)
            nc.vector.tensor_tensor(w_[:], w_[:], u5[:], op=OP.mult)
            nc.vector.tensor_scalar(w_[:], w_[:], 1.0, None, op0=OP.add)
            msk = p1.tile([128, t_blk], f32, tag="msk")
            nc.vector.tensor_scalar(msk[:], u[:], 1.0, None, op0=OP.is_lt)
            fc = p1.tile([128, t_blk], f32, tag="fc")
            nc.vector.tensor_tensor(fc[:], w_[:], msk[:], op=OP.mult)
            fcb = fc[:].unsqueeze(-1).broadcast_to([128, t_blk, NB])
            fts = p1.tile([128, t_blk, NB], bf16, tag="fts")
            nc.vector.tensor_tensor(fts[:], bes[:], fcb, op=OP.mult)
            for t in range(t_blk):
                fp = psL.tile([NB, 128], bf16, tag="w1p")
                nc.tensor.transpose(fp[:], fts[:, t, :], ident[:])
                nc.scalar.copy(featsT[:, t * 128:(t + 1) * 128], fp[:])
            nc.sync.dma_start(featsd[:, b * eb:(b + 1) * eb], featsT[:])
            if debug and b == 0:
                nc.gpsimd.dma_start(dbg["dbg_fts"][:], fts[:])
                nc.gpsimd.dma_start(dbg["dbg_ln"][:], ln[:])
                nc.gpsimd.dma_start(dbg["dbg_arg"][:], arg[:])
                nc.gpsimd.dma_start(dbg["dbg_snb"][:], snb[:])
                nc.gpsimd.dma_start(dbg["dbg_bes"][:], bes[:])
                nc.gpsimd.dma_start(dbg["dbg_fc"][:], fc[:])
        # ---------------- phase 1b: MLP1 + messages + scatter ---------

        def load_feats(pool, e0, ebs, tag):
            ft = pool.tile([128, TSUB * 128], bf16, tag=tag)
            nc.sync.dma_start(ft[0:NB, :ebs], featsd[:, e0:e0 + ebs])
            nc.sync.dma_start(ft[64:64 + NB, :ebs], ft[0:NB, :ebs])
            return ft

        def find_chunk(cinfo, o):
            for a3, hi, o_, w_ in cinfo:
                if o_ <= o < o_ + w_:
                    return a3, hi, o_, w_
            raise AssertionError(o)

        subs = [(s0, min(TSUB, t_blk - s0)) for s0 in range(0, t_blk, TSUB)]

        for b in range(NBLK):
            acc1 = p1.tile([128, C4], f32, tag="acc1", bufs=2)
            for si, (s0, sw) in enumerate(subs):
                ebs = sw * 128
                ts0 = b * t_blk + s0
                e0 = b * eb + s0 * 128
                attrs_sb = p1.tile([NE, TSUB * 128], bf16, tag="attrs_sb")
                nc.sync.dma_start(attrs_sb[:NE, :ebs], attrsT_in[:, e0:e0 + ebs])
                ft = load_feats(p1, e0, ebs, "featsT1")
                cinfo = run_mlp(ft, ["Wr1_1", "Wr1_2", "Wr1_3"], psM, p1c,
                                "m1", ebs)
                uv = p1.tile([128, TSUB, 2, C], bf16, tag="uv")
                for t in range(sw):
                    o = t * 128
                    a3, hi, o_, _ = find_chunk(cinfo, o)
                    lo = o - o_
                    w1p = psL.tile([128, 2 * C], f32, tag="w1p")
                    nc.tensor.matmul(w1p[:],
                                     a3[hi * 64:(hi + 1) * 64, lo:lo + 128],
                                     w1o[hi * 64:(hi + 1) * 64, :],
                                     start=True, stop=True)
                    ep = psE.tile([128, C], f32, tag="ep")
                    nc.tensor.matmul(ep[:], attrs_sb[:NE, o:o + 128],
                                     embup[:], start=True, stop=True)
                    heb = p1c.tile([128, C], bf16, tag="heb")
                    nc.scalar.copy(heb[:], ep[:])
                    epb = heb[:].unsqueeze(1).broadcast_to([128, 2, C])
                    w1v = w1p[:].rearrange("p (x c) -> p x c", c=C)
                    nc.vector.tensor_tensor(uv[:, t, :, :], w1v, epb, op=OP.mult)
                rb = rcvb[:, ts0:ts0 + sw].unsqueeze(-1).broadcast_to(
                    [128, sw, 128])
                S = p1.tile([128, TSUB, 128], bf16, tag="S")
                nc.vector.tensor_tensor(S[:, :sw, :], iob[:, :sw, :], rb,
                                        op=OP.is_equal)
                Sy = p1.tile([128, TSUB, 3, 128], bf16, tag="Sy")
                for d in range(3):
                    yb = yv_all[:, ts0:ts0 + sw, d].unsqueeze(-1).broadcast_to(
                        [128, sw, 128])
                    nc.vector.tensor_tensor(Sy[:, :sw, d, :], S[:, :sw, :], yb,
                                            op=OP.mult)
                A1 = psA.tile([128, C4], f32, tag="A1")
                for t in range(sw):
                    nc.tensor.matmul(A1[:, 0:C], S[:, t, :], uv[:, t, 0, :],
                                     start=(t == 0), stop=(t == sw - 1),
                                     skip_group_check=True)
                for d in range(3):
                    for t in range(sw):
                        nc.tensor.matmul(A1[:, C * (1 + d):C * (2 + d)],
                                         Sy[:, t, d, :], uv[:, t, 1, :],
                                         start=(t == 0), stop=(t == sw - 1),
                                         skip_group_check=True)
                nc.sync.dma_start(sdram[:, ts0:ts0 + sw, :], S[:, :sw, :])
                nc.sync.dma_start(sydram[:, ts0:ts0 + sw, :, :],
                                  Sy[:, :sw, :, :])
                if si == 0:
                    nc.scalar.copy(acc1[:], A1[:])
                else:
                    nc.vector.tensor_tensor(acc1[:], A1[:], acc1[:], op=OP.add)
            nc.vector.tensor_copy(a1sb[:, b, :], acc1[:])

        for p in (psE, psL, psM, psA, p1c, p1):
            p.release()
        if debug:
            nc.gpsimd.dma_start(dbg["dbg_a1"][:], a1sb[:])
            nc.gpsimd.dma_start(dbg["dbg_ft"][:], featsd[:])

        # ---------------- phase 2: node layer 1 -----------------------
        n1 = tc.alloc_tile_pool(name="n1", bufs=1)
        n1t = tc.alloc_tile_pool(name="n1t", bufs=2)
        n1m = tc.alloc_tile_pool(name="n1m", bufs=1)
        psN = tc.alloc_tile_pool(name="psN", bufs=2, space="PSUM")

        def transpose_blocks(src, c0, name, pool, pspool):
            dst = pool.tile([C, NLOC], bf16, name=name, tag=name)
            for b in range(NBLK):
                tp = pspool.tile([128, 128], bf16, tag="tp")
                nc.tensor.transpose(tp[:], src[:, b, c0:c0 + C], ident[:])
                nc.scalar.copy(dst[:, b * 128:(b + 1) * 128], tp[:])
            return dst

        def mm_wide(name, lhsT, rhs_tile, pool, pspool, dtype=bf16):
            out = pool.tile([C, NLOC], dtype, name=name, tag=name)
            for o, w_ in nchunks:
                pm = pspool.tile([C, 512], f32, tag="mmw")
                nc.tensor.matmul(pm[:, :w_], lhsT, rhs_tile[:, o:o + w_],
                                 start=True, stop=True)
                nc.scalar.copy(out[:, o:o + w_], pm[:, :w_])
            return out

        AsT = transpose_blocks(a1sb[:], 0, "AsT", n1m, psN)
        AvT = [transpose_blocks(a1sb[:], C * (1 + d), f"AvT{d}", n1m, psN)
               for d in range(3)]
        AsL = mm_wide("AsL", wb["Wlin1_s"][:], AsT[:], n1m, psN)
        AvL = [mm_wide(f"AvL{d}", wb["Wlin1_v"][:], AvT[d][:], n1m, psN)
               for d in range(3)]
        Pw = {n: mm_wide("w" + n, wb[n][:], naTs[:], n1m, psN)
              for n in ["P1s1", "P1ss", "P1vv", "P1v1", "P1sv"]}
        sqs = n1m.tile([C, NLOC], bf16, name="sqs", tag="sqs")
        nc.scalar.square(sqs[:], AsL[:])
        vv = n1m.tile([C, NLOC], f32, name="vv", tag="vv")
        sqv = n1m.tile([C, NLOC], f32, name="sqv", tag="sqv")
        nc.scalar.square(vv[:], AvL[0][:])
        for d in (1, 2):
            nc.scalar.square(sqv[:], AvL[d][:])
            nc.vector.tensor_tensor(vv[:], vv[:], sqv[:], op=OP.add)
        Bs = n1m.tile([C, NLOC], bf16, name="Bs", tag="Bs")
        t0 = n1t.tile([C, NLOC], bf16, tag="t0")
        nc.vector.tensor_tensor(Bs[:], Pw["P1s1"][:], AsL[:], op=OP.mult)
        nc.vector.tensor_tensor(t0[:], Pw["P1ss"][:], sqs[:], op=OP.mult)
        nc.vector.tensor_tensor(Bs[:], Bs[:], t0[:], op=OP.add)
        t1 = n1t.tile([C, NLOC], bf16, tag="t0")
        nc.vector.tensor_tensor(t1[:], Pw["P1vv"][:], vv[:], op=OP.mult)
        nc.vector.tensor_tensor(Bs[:], Bs[:], t1[:], op=OP.add)
        gsk = n1m.tile([C, NLOC], bf16, name="gsk", tag="gsk")
        nc.vector.tensor_tensor(gsk[:], Pw["P1sv"][:], AsL[:], op=OP.mult)
        nc.vector.tensor_tensor(gsk[:], gsk[:], Pw["P1v1"][:], op=OP.add)
        Bv = []
        for d in range(3):
            bvd = n1m.tile([C, NLOC], bf16, name=f"Bv{d}", tag=f"Bv{d}")
            nc.vector.tensor_tensor(bvd[:], gsk[:], AvL[d][:], op=OP.mult)
            Bv.append(bvd)
        hvT = [mm_wide(f"hvT{d}", wb["Lp1_v"][:], Bv[d][:], n1, psN)
               for d in range(3)]
        tabsb = n1m.tile([128, NBLK, C4], bf16, name="tabsb", tag="tabsb")
        for b in range(NBLK):
            pm = psN.tile([128, C], f32, tag="tab")
            nc.tensor.matmul(pm[:], Bs[:, b * 128:(b + 1) * 128], combS[:],
                             start=True, stop=True)
            nc.scalar.copy(tabsb[:, b, 0:C], pm[:])
            for d in range(3):
                pm2 = psN.tile([128, C], f32, tag="tab")
                nc.tensor.matmul(pm2[:], Bv[d][:, b * 128:(b + 1) * 128],
                                 combV[:], start=True, stop=True)
                nc.scalar.copy(tabsb[:, b, C * (1 + d):C * (2 + d)], pm2[:])
        if debug:
            nc.gpsimd.dma_start(dbg["dbg_tab"][:], tabsb[:])
        bounce = dramp.tile([NLOC, C4], bf16, tag="bounce")
        nc.sync.dma_start(bounce[:].rearrange("(b p) c -> p b c", p=128),
                          tabsb[:])
        tabdram = dramp.tile([NCORES * NLOC, C4], bf16, addr_space="Shared",
                             tag="tabdram")
        nc.gpsimd.collective_compute(
            "AllGather", OP.bypass, replica_groups=[list(range(NCORES))],
            ins=[bounce[:]], outs=[tabdram[:]])
        psN.release()
        n1m.release()

        # ---- skip-TP (depends only on hvT): overlaps with phase 4 ----
        sc_all = n1.tile([128, NBLK, 3, C], bf16, name="sc_all", tag="sc_all")
        na_sb = n1.tile([128, NBLK, NE], f32, name="na_sb", tag="na_sb")
        nc.sync.dma_start(na_sb[:], na_in[:].rearrange("(b p) e -> p b e", p=128))
        n5a = tc.alloc_tile_pool(name="n5a", bufs=2)
        ps5a = tc.alloc_tile_pool(name="ps5a", bufs=1, space="PSUM")
        for b in range(NBLK):
            sc = n5a.tile([128, 3, C], f32, tag="sc5")
            gp = ps5a.tile([128, 3, C], f32, tag="gp5")
            for s in range(NE):
                for d in range(3):
                    nc.tensor.matmul(gp[:, d, :],
                                     hvT[d][:, b * 128:(b + 1) * 128],
                                     wsk[:, s, :], start=True, stop=True)
                if s == 0:
                    nc.vector.tensor_scalar(sc[:], gp[:], na_sb[:, b, 0:1],
                                            None, op0=OP.mult)
                else:
                    nc.vector.scalar_tensor_tensor(
                        sc[:], gp[:], na_sb[:, b, s:s + 1], sc[:],
                        op0=OP.mult, op1=OP.add)
            nc.vector.tensor_copy(sc_all[:, b, :, :], sc[:])

        # ---------------- phase 4: edges, layer 2 ---------------------
        p4 = tc.alloc_tile_pool(name="p4", bufs=2)
        p4c = tc.alloc_tile_pool(name="p4c", bufs=4)
        psA2 = tc.alloc_tile_pool(name="psA2", bufs=2, space="PSUM")
        psM2 = tc.alloc_tile_pool(name="psM2", bufs=2, space="PSUM")
        psL2 = tc.alloc_tile_pool(name="psL2", bufs=3, space="PSUM")

        w2dram = dramp.tile([128, tt, 4 * C], bf16, tag="w2dram")
        # 4a: radial MLP2 for all subs (no dependency on the AllGather)
        for b in range(NBLK):
            for si, (s0, sw) in enumerate(subs):
                ebs = sw * 128
                ts0 = b * t_blk + s0
                e0 = b * eb + s0 * 128
                featsT2 = load_feats(p4, e0, ebs, "featsT2")
                cinfo = run_mlp(featsT2, ["Wr2_1", "Wr2_2", "Wr2_3"], psM2,
                                p4c, "m2", ebs)
                w2sb = p4.tile([128, TSUB, 4 * C], bf16, tag="w2sb", bufs=2)
                for t in range(sw):
                    o = t * 128
                    a3, hi, o_, _ = find_chunk(cinfo, o)
                    lo = o - o_
                    w2p = psL2.tile([128, 4 * C], f32, tag="w2p")
                    nc.tensor.matmul(w2p[:],
                                     a3[hi * 64:(hi + 1) * 64, lo:lo + 128],
                                     w2o[hi * 64:(hi + 1) * 64, :],
                                     start=True, stop=True)
                    nc.scalar.copy(w2sb[:, t, :], w2p[:])
                nc.sync.dma_start(w2dram[:, ts0:ts0 + sw, :], w2sb[:, :sw, :])

        # 4b: gather + messages + scatter
        for b in range(NBLK):
            acc2 = p4.tile([128, C4], f32, tag="acc2", bufs=2)
            for si, (s0, sw) in enumerate(subs):
                ebs = sw * 128
                ts0 = b * t_blk + s0
                e0 = b * eb + s0 * 128
                gat = p4.tile([128, TSUB, C4], bf16, tag="gat")
                nc.gpsimd.dma_gather(
                    gat[:, :sw, :], tabdram[:],
                    sndi[:, e0 // 16:(e0 + ebs) // 16],
                    num_idxs=ebs, num_idxs_reg=ebs, elem_size=C4)
                gat4 = gat[:].rearrange("p t (x c) -> p t x c", c=C)
                w2sb = p4.tile([128, TSUB, 4 * C], bf16, tag="w2l", bufs=3)
                nc.sync.dma_start(w2sb[:, :sw, :], w2dram[:, ts0:ts0 + sw, :])
                S = p4.tile([128, TSUB, 128], bf16, tag="S4", bufs=3)
                nc.sync.dma_start(S[:, :sw, :], sdram[:, ts0:ts0 + sw, :])
                Sy = p4.tile([128, TSUB, 3, 128], bf16, tag="Sy4", bufs=3)
                nc.sync.dma_start(Sy[:, :sw, :, :],
                                  sydram[:, ts0:ts0 + sw, :, :])
                m4 = p4.tile([128, TSUB, 4, C], bf16, tag="m4")
                m5 = p4.tile([128, TSUB, 4, C], bf16, tag="m5")
                hsg = gat4[:, :sw, 0, :]
                nc.vector.tensor_tensor(m4[:, :sw, 0, :], w2sb[:, :sw, 0:C],
                                        hsg, op=OP.mult)
                nc.vector.tensor_tensor(m5[:, :sw, 3, :],
                                        w2sb[:, :sw, 2 * C:3 * C], hsg,
                                        op=OP.mult)
                for d in range(3):
                    hvg = gat4[:, :sw, 1 + d, :]
                    nc.vector.tensor_tensor(m4[:, :sw, 1 + d, :],
                                            w2sb[:, :sw, 3 * C:4 * C], hvg,
                                            op=OP.mult)
                    nc.vector.tensor_tensor(m5[:, :sw, d, :],
                                            w2sb[:, :sw, C:2 * C], hvg,
                                            op=OP.mult)
                A2 = psA2.tile([128, 4, C], f32, tag="A2")
                for t in range(sw):
                    nc.tensor.matmul(A2[:], S[:, t, :],
                                     m4[:, t, :, :], start=(t == 0), stop=False,
                                     skip_group_check=True)
                for d in range(3):
                    for t in range(sw):
                        nc.tensor.matmul(A2[:, 1 + d, :], Sy[:, t, d, :],
                                         m5[:, t, 3, :], start=False,
                                         stop=False, skip_group_check=True)
                for d in range(3):
                    for t in range(sw):
                        sp = (t == sw - 1) and (d == 2)
                        nc.tensor.matmul(A2[:, 0, :], Sy[:, t, d, :],
                                         m5[:, t, d, :], start=False, stop=sp,
                                         skip_group_check=True)
                a2f = A2[:].rearrange("p x c -> p (x c)")
                if si == 0:
                    nc.scalar.copy(acc2[:], a2f)
                else:
                    nc.vector.tensor_tensor(acc2[:], a2f, acc2[:], op=OP.add)
            nc.vector.tensor_copy(a2sb[:, b, :], acc2[:])
            if debug and b == 0:
                nc.gpsimd.dma_start(dbg["dbg_gat"][:, 0:min(TSUB, t_blk)],
                                    gat[:, 0:min(TSUB, t_blk)])
                nc.gpsimd.dma_start(dbg["dbg_m4"][:, 0:min(TSUB, t_blk)],
                                    m4[:, 0:min(TSUB, t_blk)])

        for p in (psL2, psM2, psA2, p4c, p4, ps5a, n5a):
            p.release()
        if debug:
            nc.gpsimd.dma_start(dbg["dbg_a2"][:], a2sb[:])

        # ---------------- phase 5: node layer 2 + outputs -------------
        n2 = tc.alloc_tile_pool(name="n2", bufs=1)
        psN2 = tc.alloc_tile_pool(name="psN2", bufs=2, space="PSUM")
        A2sT = transpose_blocks(a2sb[:], 0, "A2sT", n2, psN2)
        A2vT = [transpose_blocks(a2sb[:], C * (1 + d), f"A2vT{d}", n2, psN2)
                for d in range(3)]
        A2sL = mm_wide("A2sL", wb["Wlin2_s"][:], A2sT[:], n2, psN2)
        A2vL = [mm_wide(f"A2vL{d}", wb["Wlin2_v"][:], A2vT[d][:], n2, psN2)
                for d in range(3)]
        P2w = {n: mm_wide("w" + n, wb[n][:], naTs[:], n2, psN2)
               for n in ["P2v1", "P2sv"]}
        g2 = n2.tile([C, NLOC], bf16, name="g2", tag="g2")
        nc.vector.tensor_tensor(g2[:], P2w["P2sv"][:], A2sL[:], op=OP.mult)
        nc.vector.tensor_tensor(g2[:], g2[:], P2w["P2v1"][:], op=OP.add)
        B2v = []
        for d in range(3):
            b2d = n2.tile([C, NLOC], bf16, name=f"B2v{d}", tag=f"B2v{d}")
            nc.vector.tensor_tensor(b2d[:], g2[:], A2vL[d][:], op=OP.mult)
            B2v.append(b2d)
        psN2.release()

        # h2 (node-major) = B2v @ Lp2_v + skip-TP, then back to feat-major
        n2t = tc.alloc_tile_pool(name="n2t", bufs=2)
        psH = tc.alloc_tile_pool(name="psH", bufs=1, space="PSUM")
        psHt = tc.alloc_tile_pool(name="psHt", bufs=2, space="PSUM")
        h2T = [n2.tile([C, NLOC], bf16, name=f"h2T{d}", tag=f"h2T{d}")
               for d in range(3)]
        for b in range(NBLK):
            hp = psH.tile([128, 3, C], f32, tag="h2p")
            for d in range(3):
                nc.tensor.matmul(hp[:, d, :], B2v[d][:, b * 128:(b + 1) * 128],
                                 wb["Lp2_v"][:], start=True, stop=True)
            sc16 = n2t.tile([128, 3, C], bf16, tag="sc16")
            nc.vector.tensor_tensor(sc16[:], hp[:], sc_all[:, b, :, :],
                                    op=OP.add)
            for d in range(3):
                tp = psHt.tile([128, 128], bf16, tag="tph")
                nc.tensor.transpose(tp[:], sc16[:, d, :], ident[:])
                nc.scalar.copy(h2T[d][:, b * 128:(b + 1) * 128], tp[:])
        psHt.release()
        psH.release()

        psD = tc.alloc_tile_pool(name="psD", bufs=2, space="PSUM")
        dipf = [n2.tile([1, NLOC], f32, name=f"dipf{d}", tag=f"dipf{d}")
                for d in range(3)]
        for d in range(3):
            for o, w_ in nchunks:
                dp = psD.tile([1, 512], f32, tag="dp")
                nc.tensor.matmul(dp[:, :w_], wb["R1c"][:], hvT[d][:, o:o + w_],
                                 start=True, stop=False)
                nc.tensor.matmul(dp[:, :w_], qcol[:], h2T[d][:, o:o + w_],
                                 start=False, stop=True)
                nc.scalar.copy(dipf[d][0:1, o:o + w_], dp[:, :w_])

        posb = n2.tile([128, NBLK, 3], f32, name="posb", tag="posb")
        nc.sync.dma_start(posb[:], posn_in[:].rearrange("(b p) c -> p b c", p=128))
        chgb = n2.tile([128, NBLK], f32, name="chgb", tag="chgb")
        nc.sync.dma_start(chgb[:], chg_in[:].rearrange("(b p) c -> p (b c)", p=128))
        bohb = n2.tile([128, NBLK, G], f32, name="bohb", tag="bohb")
        nc.sync.dma_start(bohb[:], boh_in[:].rearrange("(b p) g -> p b g", p=128))
        dipo = n2.tile([128, NBLK, 3], f32, name="dipo", tag="dipo")
        gs = psD.tile([G, 3], f32, tag="gs", bufs=1)
        for b in range(NBLK):
            for d in range(3):
                dpp = psD.tile([128, 1], f32, tag="dpp")
                nc.tensor.transpose(dpp[:], dipf[d][0:1, b * 128:(b + 1) * 128],
                                    identf[0:1, 0:1])
                nc.scalar.copy(dipo[:, b, d:d + 1], dpp[:])
            cp = n2t.tile([128, 3], f32, tag="cp")
            nc.vector.tensor_scalar(cp[:], posb[:, b, :], chgb[:, b:b + 1],
                                    None, op0=OP.mult)
            nc.vector.tensor_tensor(cp[:], cp[:], dipo[:, b, :], op=OP.add)
            nc.tensor.matmul(gs[:], bohb[:, b, :], cp[:], start=(b == 0),
                             stop=(b == NBLK - 1))
        nc.sync.dma_start(out_dip[:].rearrange("(b p) c -> p b c", p=128),
                          dipo[:])
        if debug:
            nc.gpsimd.dma_start(dbg["dbg_hvT0"][:], hvT[0][:])
            nc.gpsimd.dma_start(dbg["dbg_Bs"][:], Bs[:])
            nc.gpsimd.dma_start(dbg["dbg_h2T0"][:], h2T[0][:])
            nc.gpsimd.dma_start(dbg["dbg_dipf0"][:], dipf[0][:])
        gso = n2.tile([G, 3], f32, name="gso", tag="gso")
        nc.scalar.copy(gso[:], gs[:])
        nc.sync.dma_start(out_gs[:], gso[:])

        for p in (psD, n2t, n2, n1t, n1):
            p.release()
        dramp.release()
        wp.release()

    nc.compile()
    return nc


_BUILD_CACHE = {}


def kernel(**inputs):
    in_maps, t_blk = prep_shards(inputs)
    nc = _BUILD_CACHE.get(t_blk)
    if nc is None:
        nc = build(t_blk)
        _BUILD_CACHE[t_blk] = nc
    res = run_bass_kernel_spmd(nc, in_maps, core_ids=list(range(NCORES)))
    dip = np.zeros((N, 3), np.float32)
    tot = np.zeros((G, 3), np.float32)
    for k in range(NCORES):
        dip[k * NPC:(k + 1) * NPC] = res.results[k]["dip"][:NPC]
        tot += res.results[k]["gsum"]
    return tot, dip


# revision 58
# speedup vs baseline: 1.2177x; 1.0368x over previous
"""AtomicDipolesMACE on 8 TRN2 NeuronCores.

Sharding: nodes are split into 8 contiguous ranges of 1250 (padded to 1280 =
10 blocks of 128). Each core owns the edges whose receiver falls in its range,
sorted and grouped by 128-node receiver block, each block segment padded to a
uniform T_BLK tiles of 128 edges. Scatters become per-block one-hot matmuls
(PSUM f32 accumulation, Yv folded into scaled one-hots); the layer-2 sender
gather is a dma_gather from an AllGathered bf16 node-feature table.
"""
import sys
sys.path.insert(0, "/opt/trn_rl_repo")
import numpy as np
import ml_dtypes

from concourse import bass, bacc, tile, mybir
from concourse.bass_utils import run_bass_kernel_spmd

f32 = mybir.dt.float32
bf16 = mybir.dt.bfloat16
i16 = mybir.dt.int16
i32 = mybir.dt.int32
AF = mybir.ActivationFunctionType
OP = mybir.AluOpType
nbf16 = ml_dtypes.bfloat16
USE_SILU = True
PACK_MLP = True
TSUB = 6

N, E, C, NE, G, NB = 10000, 160000, 128, 10, 16, 8
RMAX, AVG, SQ3 = 5.0, 16.0, 3.0 ** 0.5
NCORES = 8
NPC = N // NCORES          # 1250 real nodes per core
NBLK = 10                  # node blocks per core
NLOC = NBLK * 128          # 1280 padded local nodes
C4 = 4 * C                 # 512: table row / message width

WSHAPE = dict(W_embT=[C, NE], W_up1=[C, C], Wr1_1=[NB, 64], Wr1_2=[64, 64],
              Wr1_3=[64, 64], Wr1_o=[64, 2 * C], Wr2_1=[NB, 64],
              Wr2_2=[64, 64], Wr2_3=[64, 64], Wr2_o=[64, 4 * C],
              Wlin1_s=[C, C], Wlin1_v=[C, C], Wlin2_s=[C, C],
              Wlin2_v=[C, C], Lp1_sT=[C, C], Lp1_vT=[C, C], Lp1_v=[C, C],
              Lp2_v=[C, C], Wup2_s=[C, C], Wup2_v=[C, C],
              P1s1=[NE, C], P1ss=[NE, C], P1vv=[NE, C], P1v1=[NE, C],
              P1sv=[NE, C], P2v1=[NE, C], P2sv=[NE, C],
              R1c=[C, 1], RmidT=[16, C], Routc=[16, 1], kpi5=[128, NB])


# ---------------------------------------------------------------- host prep
def prep_shards(inp):
    snd = np.asarray(inp["edge_index"][0], dtype=np.int64)
    rcv = np.asarray(inp["edge_index"][1], dtype=np.int64)
    pos = np.asarray(inp["positions"], dtype=np.float32)
    na = np.asarray(inp["node_attrs"], dtype=np.float32)
    shf = np.asarray(inp["shifts"], dtype=np.float32)
    chg = np.asarray(inp["charges"], dtype=np.float32)
    bat = np.asarray(inp["batch"], dtype=np.int64)

    core = rcv // NPC
    loc = rcv - core * NPC
    blk = loc // 128

    order = np.lexsort((blk, core))
    snd_s, rcv_s, loc_s = snd[order], rcv[order], loc[order]
    core_s, blk_s = core[order], blk[order]
    shf_s = shf[order]

    counts = np.zeros((NCORES, NBLK), dtype=np.int64)
    np.add.at(counts, (core_s, blk_s), 1)
    t_blk = max(1, int(np.max((counts + 127) // 128)))
    eb = t_blk * 128
    epad = NBLK * eb
    tt = NBLK * t_blk

    snd_pad = (snd_s // NPC) * NLOC + (snd_s % NPC)  # padded-table row index

    starts = np.zeros(NCORES * NBLK, dtype=np.int64)
    starts[1:] = np.cumsum(counts.reshape(-1))[:-1]
    starts = starts.reshape(NCORES, NBLK)

    w = weights_prep(inp)
    in_maps = []
    for k in range(NCORES):
        pos_s = np.zeros((epad, 3), np.float32)
        pos_r = np.ones((epad, 3), np.float32)
        shfe = np.zeros((epad, 3), np.float32)
        attrsT = np.zeros((NE, epad), np.float32)
        sndp = np.zeros(epad, np.int64)
        rcvb = -np.ones(epad, np.float32)
        for b in range(NBLK):
            s0, n = starts[k, b], counts[k, b]
            sl = slice(s0, s0 + n)
            d0 = b * eb
            pos_s[d0:d0 + n] = pos[snd_s[sl]]
            pos_r[d0:d0 + n] = pos[rcv_s[sl]]
            shfe[d0:d0 + n] = shf_s[sl]
            attrsT[:, d0:d0 + n] = na[snd_s[sl]].T
            sndp[d0:d0 + n] = snd_pad[sl]
            rcvb[d0:d0 + n] = (loc_s[sl] - b * 128).astype(np.float32)

        def emaj(a):  # [epad, d] -> [128, tt, d]  (edge i -> (i%128, i//128))
            return np.ascontiguousarray(a.reshape(tt, 128, -1).transpose(1, 0, 2))

        sndw = np.zeros((128, epad // 16), np.int16)
        for b in range(NBLK):
            seg = sndp[b * eb:(b + 1) * eb].reshape(eb // 16, 16).T
            sndw[:, b * (eb // 16):(b + 1) * (eb // 16)] = np.tile(seg, (8, 1))

        nl0 = k * NPC
        na_nm = np.zeros((NLOC, NE), np.float32)
        na_nm[:NPC] = na[nl0:nl0 + NPC]
        pos_nm = np.zeros((NLOC, 3), np.float32)
        pos_nm[:NPC] = pos[nl0:nl0 + NPC]
        chg_nm = np.zeros((NLOC, 1), np.float32)
        chg_nm[:NPC, 0] = chg[nl0:nl0 + NPC]
        boh = np.zeros((NLOC, G), np.float32)
        boh[np.arange(NPC), bat[nl0:nl0 + NPC]] = 1.0

        m = dict(
            pos_s=emaj(pos_s), pos_r=emaj(pos_r), shfe=emaj(shfe),
            attrsT=attrsT.astype(nbf16), sndw=sndw,
            rcvb=emaj(rcvb)[:, :, 0].astype(nbf16),
            na_nm=na_nm, naT=np.ascontiguousarray(na_nm.T).astype(nbf16),
            pos_nm=pos_nm, chg_nm=chg_nm, boh=boh,
        )
        m.update(w)
        in_maps.append(m)
    return in_maps, t_blk


def weights_prep(inp):
    g = lambda k: np.ascontiguousarray(np.asarray(inp[k], dtype=np.float32))
    kvec = (np.pi / RMAX) * np.arange(1, NB + 1, dtype=np.float32)
    return dict(
        W_embT=g("W_emb").T.copy(), W_up1=g("W_up1"),
        Wr1_1=g("Wr1_1"), Wr1_2=g("Wr1_2"), Wr1_3=g("Wr1_3"), Wr1_o=g("Wr1_o"),
        Wr2_1=g("Wr2_1"), Wr2_2=g("Wr2_2"), Wr2_3=g("Wr2_3"), Wr2_o=g("Wr2_o"),
        Wlin1_s=g("Wlin1_s"), Wlin1_v=g("Wlin1_v"),
        Wlin2_s=g("Wlin2_s"), Wlin2_v=g("Wlin2_v"),
        Lp1_sT=g("Lp1_s").T.copy(), Lp1_vT=g("Lp1_v").T.copy(),
        Lp1_v=g("Lp1_v"), Lp2_v=g("Lp2_v"),
        Wup2_s=g("Wup2_s"), Wup2_v=g("Wup2_v"),
        P1s1=g("P1_s1"), P1ss=g("P1_ss"), P1vv=g("P1_vv"),
        P1v1=g("P1_v1"), P1sv=g("P1_sv"), P2v1=g("P2_v1"), P2sv=g("P2_sv"),
        Wsk=g("Wsk"),
        R1c=g("R1").reshape(C, 1), RmidT=g("Rmid").T.copy(),
        Routc=g("Rout").reshape(16, 1),
        kpi5=np.tile(kvec, (128, 1)),
    )


# ---------------------------------------------------------------- builder
def build(t_blk, debug=False):
    eb = t_blk * 128
    tt = NBLK * t_blk
    epad = NBLK * eb
    nc = bacc.Bacc(None, target_bir_lowering=False, num_devices=NCORES)

    def din(name, shape, dt=f32):
        return nc.declare_dram_parameter(name, shape, dt, isOutput=False)

    pos_s = din("pos_s", [128, tt, 3]); pos_r = din("pos_r", [128, tt, 3])
    shfe = din("shfe", [128, tt, 3])
    attrsT_in = din("attrsT", [NE, epad], bf16)
    sndw_in = din("sndw", [128, epad // 16], i16)
    rcvb_in = din("rcvb", [128, tt], bf16)
    na_in = din("na_nm", [NLOC, NE]); naT_in = din("naT", [NE, NLOC], bf16)
    posn_in = din("pos_nm", [NLOC, 3]); chg_in = din("chg_nm", [NLOC, 1])
    boh_in = din("boh", [NLOC, G])
    win = {n: din(n, WSHAPE[n]) for n in WSHAPE}
    wsk_in = din("Wsk", [NE, C, C])
    out_dip = nc.declare_dram_parameter("dip", [NLOC, 3], f32, isOutput=True)
    out_gs = nc.declare_dram_parameter("gsum", [16, 3], f32, isOutput=True)
    dbg = {}
    if debug:
        for nm, shp in [("dbg_a1", [128, NBLK, C4]), ("dbg_tab", [128, NBLK, C4]),
                        ("dbg_a2", [128, NBLK, C4]), ("dbg_hvT0", [C, NLOC]),
                        ("dbg_Bs", [C, NLOC]), ("dbg_h2T0", [C, NLOC]),
                        ("dbg_dipf0", [1, NLOC]), ("dbg_gat", [128, t_blk, C4]),
                        ("dbg_m4", [128, t_blk, 4, C]),
                        ("dbg_ft", [NB, epad]), ("dbg_uv", [128, t_blk, 2, C]),
                        ("dbg_S", [128, t_blk, 128]),
                        ("dbg_fts", [128, t_blk, NB]),
                        ("dbg_ln", [128, t_blk]), ("dbg_arg", [128, t_blk, NB]),
                        ("dbg_snb", [128, t_blk, NB]),
                        ("dbg_bes", [128, t_blk, NB]),
                        ("dbg_fc", [128, t_blk])]:
            dbg[nm] = nc.declare_dram_parameter(nm, shp, f32, isOutput=True)

    nchunks = []
    _o = 0
    while _o < NLOC:
        nchunks.append((_o, min(512, NLOC - _o)))
        _o += nchunks[-1][1]

    def echunks(ebs):
        half = ebs // 2
        if ebs <= 1024 and half % 128 == 0 and half > 0:
            return [(0, half), (half, ebs - half)]
        o, out = 0, []
        while o < ebs:
            w_ = min(512, ebs - o)
            out.append((o, w_))
            o += w_
        return out

    with tile.TileContext(nc) as tc:
        wp = tc.alloc_tile_pool(name="wp", bufs=1)
        dramp = tc.alloc_tile_pool(name="dram", bufs=1, space="DRAM")

        # ---- load + prep weights -------------------------------------
        wf = {}
        for n in WSHAPE:
            wf[n] = wp.tile(WSHAPE[n], f32, name="f" + n, tag="f" + n)
            nc.sync.dma_start(wf[n][:], win[n][:])
        wsk = wp.tile([C, NE, C], bf16, name="wsk", tag="wsk")
        nc.gpsimd.dma_start(wsk[:], wsk_in[:].transpose([1, 0, 2]))
        naTs = wp.tile([NE, NLOC], bf16, name="naTs", tag="naTs")
        nc.sync.dma_start(naTs[:], naT_in[:])
        rcvb = wp.tile([128, tt], bf16, name="rcvb", tag="rcvb")
        nc.sync.dma_start(rcvb[:], rcvb_in[:])
        sndi = wp.tile([128, epad // 16], i16, name="sndi", tag="sndi")
        nc.sync.dma_start(sndi[:], sndw_in[:])

        def tobf(name, src_ap, shape, scale=None):
            t = wp.tile(shape, bf16, name=name, tag=name)
            if scale is None:
                nc.vector.tensor_copy(t[:], src_ap)
            else:
                nc.vector.tensor_scalar(t[:], src_ap, float(scale), None,
                                        op0=OP.mult)
            return t

        wb = {}
        for n in ["Wr1_1", "Wr1_2", "Wr1_3", "Wr1_o", "Wr2_1", "Wr2_2",
                  "Wr2_3", "Lp1_v", "Lp2_v", "P1s1", "P1ss", "P1v1", "P1sv",
                  "P2v1", "P2sv", "R1c"]:
            wb[n] = tobf("b" + n, wf[n][:], WSHAPE[n])
        for n in ["Wlin1_s", "Wlin1_v", "Wlin2_s", "Wlin2_v"]:
            wb[n] = tobf("b" + n, wf[n][:], WSHAPE[n], scale=1.0 / AVG)
        wb["P1vv"] = tobf("bP1vv", wf["P1vv"][:], WSHAPE["P1vv"], scale=1.0 / SQ3)
        w2o = wp.tile([128, 4 * C], bf16, name="w2o", tag="w2o")
        for h in (0, 64):
            nc.vector.tensor_copy(w2o[h:h + 64, :], wf["Wr2_o"][:])
            nc.vector.tensor_scalar(w2o[h:h + 64, C:2 * C],
                                    wf["Wr2_o"][:, C:2 * C],
                                    1.0 / SQ3, None, op0=OP.mult)
        w1o = wp.tile([128, 2 * C], bf16, name="w1o", tag="w1o")
        for h in (0, 64):
            nc.vector.tensor_copy(w1o[h:h + 64, :], wf["Wr1_o"][:])
        # hidden-layer MLP weights duplicated into the upper PE quadrant
        wb2 = {}
        for n in ["Wr1_1", "Wr1_2", "Wr1_3", "Wr2_1", "Wr2_2", "Wr2_3"]:
            kk = WSHAPE[n][0]
            t = wp.tile([128, 64], bf16, name="q" + n, tag="q" + n)
            for h in (0, 64):
                nc.vector.tensor_copy(t[h:h + kk, :], wf[n][:])
            wb2[n] = t

        psw = tc.alloc_tile_pool(name="psw", bufs=2, space="PSUM")
        eup = psw.tile([NE, C], f32, tag="pw")
        nc.tensor.matmul(eup[:], wf["W_embT"][:], wf["W_up1"][:],
                         start=True, stop=True)
        embup = tobf("embup", eup[:], [NE, C])
        cs = psw.tile([C, C], f32, tag="pw")
        nc.tensor.matmul(cs[:], wf["Lp1_sT"][:], wf["Wup2_s"][:],
                         start=True, stop=True)
        combS = tobf("combS", cs[:], [C, C])
        cv = psw.tile([C, C], f32, tag="pw")
        nc.tensor.matmul(cv[:], wf["Lp1_vT"][:], wf["Wup2_v"][:],
                         start=True, stop=True)
        combV = tobf("combV", cv[:], [C, C])
        qp = psw.tile([C, 1], f32, tag="pw")
        nc.tensor.matmul(qp[:], wf["RmidT"][:], wf["Routc"][:],
                         start=True, stop=True)
        qcol = wp.tile([C, 1], bf16, name="qcol", tag="qcol")
        nc.vector.tensor_scalar(qcol[:], qp[:], 0.5, None, op0=OP.mult)

        ioi = wp.tile([128, t_blk, 128], i32, name="ioi", tag="ioi")
        nc.gpsimd.iota(ioi[:], pattern=[[0, t_blk], [1, 128]], base=0,
                       channel_multiplier=0)
        iob = wp.tile([128, t_blk, 128], bf16, name="iob", tag="iob")
        nc.vector.tensor_copy(iob[:], ioi[:])
        idi = wp.tile([128, 128], i32, name="idi", tag="idi")
        nc.gpsimd.iota(idi[:], pattern=[[1, 128]], base=0, channel_multiplier=-1)
        idf = wp.tile([128, 128], f32, name="idf", tag="idf")
        nc.vector.tensor_copy(idf[:], idi[:])
        ident = wp.tile([128, 128], bf16, name="ident", tag="ident")
        nc.vector.tensor_scalar(ident[:], idf[:], 0.0, None, op0=OP.is_equal)
        identf = wp.tile([128, 128], f32, name="identf", tag="identf")
        nc.vector.tensor_scalar(identf[:], idf[:], 0.0, None, op0=OP.is_equal)
        psw.release()

        eps12 = wp.tile([128, 1], f32, name="eps12", tag="eps12")
        nc.vector.memset(eps12[:], 1e-12)
        mpi = wp.tile([128, 1], f32, name="mpi", tag="mpi")
        nc.vector.memset(mpi[:], -np.pi)
        yv_all = wp.tile([128, tt, 3], bf16, name="yv_all", tag="yv_all")
        a1sb = wp.tile([128, NBLK, C4], bf16, name="a1sb", tag="a1sb")
        a2sb = wp.tile([128, NBLK, C4], bf16, name="a2sb", tag="a2sb")

        # ---------------- phase 1: edges, layer 1 ---------------------
        featsd = dramp.tile([NB, epad], bf16, tag="featsd")
        sdram = dramp.tile([128, tt, 128], bf16, tag="sdram")
        sydram = dramp.tile([128, tt, 3, 128], bf16, tag="sydram")
        p1 = tc.alloc_tile_pool(name="p1", bufs=2)
        p1c = tc.alloc_tile_pool(name="p1c", bufs=3)
        psA = tc.alloc_tile_pool(name="psA", bufs=2, space="PSUM")
        psM = tc.alloc_tile_pool(name="psM", bufs=2, space="PSUM")
        psL = tc.alloc_tile_pool(name="psL", bufs=2, space="PSUM")
        psE = tc.alloc_tile_pool(name="psE", bufs=2, space="PSUM")

        def run_mlp(ftile, names, psum_pool, act_pool, tagp, ebs):
            """3-layer silu MLP over ebs edges, partition-packed pairs.
            Returns per-512-chunk (a3_tile, half, chunk_off, width)."""
            offs = echunks(ebs)
            cinfo = []
            step = 2 if PACK_MLP else 1
            for pc in range(0, len(offs), step):
                pair = offs[pc:pc + step]
                cw = max(w_ for _, w_ in pair)
                npart = 64 * len(pair)
                a1 = act_pool.tile([128, cw], bf16, tag=tagp + "a1")
                a2 = act_pool.tile([128, cw], bf16, tag=tagp + "a2")
                a3 = act_pool.tile([128, cw], bf16, tag=tagp + "a3")
                prevs = [None, a1, a2]
                outs = [a1, a2, a3]
                for li in range(3):
                    hp = psum_pool.tile([128, cw], f32, tag=tagp + "h")
                    kk = NB if li == 0 else 64
                    for hi, (o_, w_) in enumerate(pair):
                        h = hi * 64
                        if li == 0:
                            rhs = ftile[h:h + NB, o_: o_ + w_]
                        else:
                            rhs = prevs[li][h:h + 64, :w_]
                        nc.tensor.matmul(hp[h:h + 64, :w_],
                                         wb2[names[li]][h:h + kk, :], rhs,
                                         start=True, stop=True)
                    full = len(pair) == 2 and pair[0][1] == pair[1][1]
                    regions = ([(0, 128, cw)] if full else
                               [(hi * 64, hi * 64 + 64, w_)
                                for hi, (o_, w_) in enumerate(pair)])
                    if USE_SILU:
                        for (h0, h1, ww) in regions:
                            nc.scalar.activation(outs[li][h0:h1, :ww],
                                                 hp[h0:h1, :ww], AF.Silu)
                    else:
                        sg = act_pool.tile([128, cw], bf16, tag=tagp + "sg")
                        for (h0, h1, ww) in regions:
                            nc.scalar.activation(sg[h0:h1, :ww],
                                                 hp[h0:h1, :ww], AF.Sigmoid)
                            nc.vector.tensor_tensor(
                                outs[li][h0:h1, :ww], sg[h0:h1, :ww],
                                hp[h0:h1, :ww], op=OP.mult)
                for hi, (o_, w_) in enumerate(pair):
                    cinfo.append((a3, hi, o_, w_))
            return cinfo

        for b in range(NBLK):
            ts0 = b * t_blk
            featsT = p1.tile([NB, eb], bf16, tag="featsT")
            ps_ = p1.tile([128, t_blk, 3], f32, tag="ps")
            pr_ = p1.tile([128, t_blk, 3], f32, tag="pr")
            sh_ = p1.tile([128, t_blk, 3], f32, tag="sh")
            nc.sync.dma_start(ps_[:], pos_s[:, ts0:ts0 + t_blk, :])
            nc.sync.dma_start(pr_[:], pos_r[:, ts0:ts0 + t_blk, :])
            nc.sync.dma_start(sh_[:], shfe[:, ts0:ts0 + t_blk, :])
            vec = p1.tile([128, t_blk, 3], f32, tag="vec")
            nc.vector.tensor_tensor(vec[:], pr_[:], ps_[:], op=OP.subtract)
            nc.vector.tensor_tensor(vec[:], vec[:], sh_[:], op=OP.add)
            sq = p1.tile([128, t_blk, 3], f32, tag="sq")
            nc.vector.tensor_tensor(sq[:], vec[:], vec[:], op=OP.mult)
            ln2 = p1.tile([128, t_blk], f32, tag="ln2")
            nc.vector.tensor_reduce(ln2[:], sq[:], axis=mybir.AxisListType.X,
                                    op=OP.add)
            ln = p1.tile([128, t_blk], f32, tag="ln")
            nc.scalar.activation(ln[:], ln2[:], AF.Sqrt, bias=eps12[:])
            rl = p1.tile([128, t_blk], f32, tag="rl")
            nc.vector.reciprocal(rl[:], ln[:])
            rl3 = rl[:].unsqueeze(-1).broadcast_to([128, t_blk, 3])
            nc.vector.scalar_tensor_tensor(
                yv_all[:, ts0:ts0 + t_blk, :], vec[:], SQ3, rl3,
                op0=OP.mult, op1=OP.mult)
            kb = wf["kpi5"][:].unsqueeze(1).broadcast_to([128, t_blk, NB])
            lnb = ln[:].unsqueeze(-1).broadcast_to([128, t_blk, NB])
            rlb = rl[:].unsqueeze(-1).broadcast_to([128, t_blk, NB])
            arg = p1.tile([128, t_blk, NB], f32, tag="arg")
            nc.vector.tensor_tensor(arg[:], kb, lnb, op=OP.mult)
            yq = p1.tile([128, t_blk, NB], f32, tag="yq")
            nc.vector.tensor_scalar(yq[:], arg[:], 1.0 / (2 * np.pi), None,
                                    op0=OP.mult)
            yqi = p1.tile([128, t_blk, NB], i32, tag="yqi")
            nc.vector.tensor_copy(yqi[:], yq[:])
            nc.vector.tensor_copy(yq[:], yqi[:])
            # r = arg - 2pi*k is in (-pi, 2pi) whether k was trunc or round;
            # fold the (pi, 2pi) tail back by another 2pi
            nc.vector.scalar_tensor_tensor(arg[:], yq[:], -2.0 * np.pi,
                                           arg[:], op0=OP.mult, op1=OP.add)
            nc.vector.tensor_scalar(yq[:], arg[:], np.pi, None, op0=OP.is_gt)
            nc.vector.scalar_tensor_tensor(arg[:], yq[:], -2.0 * np.pi,
                                           arg[:], op0=OP.mult, op1=OP.add)
            snb = p1.tile([128, t_blk, NB], f32, tag="snb")
            nc.scalar.activation(snb[:], arg[:], AF.Sin)
            bes = p1.tile([128, t_blk, NB], f32, tag="bes")
            nc.vector.scalar_tensor_tensor(bes[:], snb[:], (2.0 / RMAX) ** 0.5,
                                           rlb, op0=OP.mult, op1=OP.mult)
            u = p1.tile([128, t_blk], f32, tag="u")
            nc.vector.tensor_scalar(u[:], ln[:], 1.0 / RMAX, None, op0=OP.mult)
            u2 = p1.tile([128, t_blk], f32, tag="u2")
            nc.vector.tensor_tensor(u2[:], u[:], u[:], op=OP.mult)
            u4 = p1.tile([128, t_blk], f32, tag="u4")
            nc.vector.tensor_tensor(u4[:], u2[:], u2[:], op=OP.mult)
            u5 = p1.tile([128, t_blk], f32, tag="u5")
            nc.vector.tensor_tensor(u5[:], u4[:], u[:], op=OP.mult)
            w_ = p1.tile([128, t_blk], f32, tag="w_")
            nc.vector.tensor_scalar(w_[:], u[:], -15.0, 35.0, op0=OP.mult,
                                    op1=OP.add)
            nc.vector.tensor_tensor(w_[:], w_[:], u[:], op=OP.mult)
            nc.vector.tensor_scalar(w_[:], w_[:], -21.0, None, op0=OP.add)
            nc.vector.tensor_tensor(w_[:], w_[:], u5[:], op=OP.mult)
            nc.vector.tensor_scalar(w_[:], w_[:], 1.0, None, op0=OP.add)
            msk = p1.tile([128, t_blk], f32, tag="msk")
            nc.vector.tensor_scalar(msk[:], u[:], 1.0, None, op0=OP.is_lt)
            fc = p1.tile([128, t_blk], f32, tag="fc")
            nc.vector.tensor_tensor(fc[:], w_[:], msk[:], op=OP.mult)
            fcb = fc[:].unsqueeze(-1).broadcast_to([128, t_blk, NB])
            fts = p1.tile([128, t_blk, NB], bf16, tag="fts")
            nc.vector.tensor_tensor(fts[:], bes[:], fcb, op=OP.mult)
            for t in range(t_blk):
                fp = psL.tile([NB, 128], bf16, tag="w1p")
                nc.tensor.transpose(fp[:], fts[:, t, :], ident[:])
                nc.scalar.copy(featsT[:, t * 128:(t + 1) * 128], fp[:])
            nc.sync.dma_start(featsd[:, b * eb:(b + 1) * eb], featsT[:])
            if debug and b == 0:
                nc.gpsimd.dma_start(dbg["dbg_fts"][:], fts[:])
                nc.gpsimd.dma_start(dbg["dbg_ln"][:], ln[:])
                nc.gpsimd.dma_start(dbg["dbg_arg"][:], arg[:])
                nc.gpsimd.dma_start(dbg["dbg_snb"][:], snb[:])
                nc.gpsimd.dma_start(dbg["dbg_bes"][:], bes[:])
                nc.gpsimd.dma_start(dbg["dbg_fc"][:], fc[:])
        # ---------------- phase 1b: MLP1 + messages + scatter ---------

        def load_feats(pool, e0, ebs, tag):
            ft = pool.tile([128, TSUB * 128], bf16, tag=tag)
            nc.sync.dma_start(ft[0:NB, :ebs], featsd[:, e0:e0 + ebs])
            nc.sync.dma_start(ft[64:64 + NB, :ebs], ft[0:NB, :ebs])
            return ft

        def find_chunk(cinfo, o):
            for a3, hi, o_, w_ in cinfo:
                if o_ <= o < o_ + w_:
                    return a3, hi, o_, w_
            raise AssertionError(o)

        subs = [(s0, min(TSUB, t_blk - s0)) for s0 in range(0, t_blk, TSUB)]

        for b in range(NBLK):
            acc1 = p1.tile([128, C4], f32, tag="acc1", bufs=2)
            for si, (s0, sw) in enumerate(subs):
                ebs = sw * 128
                ts0 = b * t_blk + s0
                e0 = b * eb + s0 * 128
                attrs_sb = p1.tile([NE, TSUB * 128], bf16, tag="attrs_sb")
                nc.sync.dma_start(attrs_sb[:NE, :ebs], attrsT_in[:, e0:e0 + ebs])
                ft = load_feats(p1, e0, ebs, "featsT1")
                cinfo = run_mlp(ft, ["Wr1_1", "Wr1_2", "Wr1_3"], psM, p1c,
                                "m1", ebs)
                uv = p1.tile([128, TSUB, 2, C], bf16, tag="uv")
                for t in range(sw):
                    o = t * 128
                    a3, hi, o_, _ = find_chunk(cinfo, o)
                    lo = o - o_
                    w1p = psL.tile([128, 2 * C], f32, tag="w1p")
                    nc.tensor.matmul(w1p[:],
                                     a3[hi * 64:(hi + 1) * 64, lo:lo + 128],
                                     w1o[hi * 64:(hi + 1) * 64, :],
                                     start=True, stop=True)
                    ep = psE.tile([128, C], f32, tag="ep")
                    nc.tensor.matmul(ep[:], attrs_sb[:NE, o:o + 128],
                                     embup[:], start=True, stop=True)
                    heb = p1c.tile([128, C], bf16, tag="heb")
                    nc.scalar.copy(heb[:], ep[:])
                    epb = heb[:].unsqueeze(1).broadcast_to([128, 2, C])
                    w1v = w1p[:].rearrange("p (x c) -> p x c", c=C)
                    nc.vector.tensor_tensor(uv[:, t, :, :], w1v, epb, op=OP.mult)
                rb = rcvb[:, ts0:ts0 + sw].unsqueeze(-1).broadcast_to(
                    [128, sw, 128])
                S = p1.tile([128, TSUB, 128], bf16, tag="S")
                nc.vector.tensor_tensor(S[:, :sw, :], iob[:, :sw, :], rb,
                                        op=OP.is_equal)
                Sy = p1.tile([128, TSUB, 3, 128], bf16, tag="Sy")
                for d in range(3):
                    yb = yv_all[:, ts0:ts0 + sw, d].unsqueeze(-1).broadcast_to(
                        [128, sw, 128])
                    nc.vector.tensor_tensor(Sy[:, :sw, d, :], S[:, :sw, :], yb,
                                            op=OP.mult)
                A1 = psA.tile([128, C4], f32, tag="A1")
                for t in range(sw):
                    nc.tensor.matmul(A1[:, 0:C], S[:, t, :], uv[:, t, 0, :],
                                     start=(t == 0), stop=(t == sw - 1),
                                     skip_group_check=True)
                for d in range(3):
                    for t in range(sw):
                        nc.tensor.matmul(A1[:, C * (1 + d):C * (2 + d)],
                                         Sy[:, t, d, :], uv[:, t, 1, :],
                                         start=(t == 0), stop=(t == sw - 1),
                                         skip_group_check=True)
                nc.sync.dma_start(sdram[:, ts0:ts0 + sw, :], S[:, :sw, :])
                nc.sync.dma_start(sydram[:, ts0:ts0 + sw, :, :],
                                  Sy[:, :sw, :, :])
                if si == 0:
                    nc.scalar.copy(acc1[:], A1[:])
                else:
                    nc.vector.tensor_tensor(acc1[:], A1[:], acc1[:], op=OP.add)
            nc.vector.tensor_copy(a1sb[:, b, :], acc1[:])

        for p in (psE, psL, psM, psA, p1c, p1):
            p.release()
        if debug:
            nc.gpsimd.dma_start(dbg["dbg_a1"][:], a1sb[:])
            nc.gpsimd.dma_start(dbg["dbg_ft"][:], featsd[:])

        # ---------------- phase 2: node layer 1 -----------------------
        n1 = tc.alloc_tile_pool(name="n1", bufs=1)
        n1t = tc.alloc_tile_pool(name="n1t", bufs=2)
        n1m = tc.alloc_tile_pool(name="n1m", bufs=1)
        psN = tc.alloc_tile_pool(name="psN", bufs=2, space="PSUM")

        def transpose_blocks(src, c0, name, pool, pspool):
            dst = pool.tile([C, NLOC], bf16, name=name, tag=name)
            for b in range(NBLK):
                tp = pspool.tile([128, 128], bf16, tag="tp")
                nc.tensor.transpose(tp[:], src[:, b, c0:c0 + C], ident[:])
                nc.scalar.copy(dst[:, b * 128:(b + 1) * 128], tp[:])
            return dst

        def mm_wide(name, lhsT, rhs_tile, pool, pspool, dtype=bf16):
            out = pool.tile([C, NLOC], dtype, name=name, tag=name)
            for o, w_ in nchunks:
                pm = pspool.tile([C, 512], f32, tag="mmw")
                nc.tensor.matmul(pm[:, :w_], lhsT, rhs_tile[:, o:o + w_],
                                 start=True, stop=True)
                nc.scalar.copy(out[:, o:o + w_], pm[:, :w_])
            return out

        AsT = transpose_blocks(a1sb[:], 0, "AsT", n1m, psN)
        AvT = [transpose_blocks(a1sb[:], C * (1 + d), f"AvT{d}", n1m, psN)
               for d in range(3)]
        AsL = mm_wide("AsL", wb["Wlin1_s"][:], AsT[:], n1m, psN)
        AvL = [mm_wide(f"AvL{d}", wb["Wlin1_v"][:], AvT[d][:], n1m, psN)
               for d in range(3)]
        Pw = {n: mm_wide("w" + n, wb[n][:], naTs[:], n1m, psN)
              for n in ["P1s1", "P1ss", "P1vv", "P1v1", "P1sv"]}
        sqs = n1m.tile([C, NLOC], bf16, name="sqs", tag="sqs")
        nc.scalar.square(sqs[:], AsL[:])
        vv = n1m.tile([C, NLOC], f32, name="vv", tag="vv")
        sqv = n1m.tile([C, NLOC], f32, name="sqv", tag="sqv")
        nc.scalar.square(vv[:], AvL[0][:])
        for d in (1, 2):
            nc.scalar.square(sqv[:], AvL[d][:])
            nc.vector.tensor_tensor(vv[:], vv[:], sqv[:], op=OP.add)
        Bs = n1m.tile([C, NLOC], bf16, name="Bs", tag="Bs")
        t0 = n1t.tile([C, NLOC], bf16, tag="t0")
        nc.vector.tensor_tensor(Bs[:], Pw["P1s1"][:], AsL[:], op=OP.mult)
        nc.vector.tensor_tensor(t0[:], Pw["P1ss"][:], sqs[:], op=OP.mult)
        nc.vector.tensor_tensor(Bs[:], Bs[:], t0[:], op=OP.add)
        t1 = n1t.tile([C, NLOC], bf16, tag="t0")
        nc.vector.tensor_tensor(t1[:], Pw["P1vv"][:], vv[:], op=OP.mult)
        nc.vector.tensor_tensor(Bs[:], Bs[:], t1[:], op=OP.add)
        gsk = n1m.tile([C, NLOC], bf16, name="gsk", tag="gsk")
        nc.vector.tensor_tensor(gsk[:], Pw["P1sv"][:], AsL[:], op=OP.mult)
        nc.vector.tensor_tensor(gsk[:], gsk[:], Pw["P1v1"][:], op=OP.add)
        Bv = []
        for d in range(3):
            bvd = n1m.tile([C, NLOC], bf16, name=f"Bv{d}", tag=f"Bv{d}")
            nc.vector.tensor_tensor(bvd[:], gsk[:], AvL[d][:], op=OP.mult)
            Bv.append(bvd)
        hvT = [mm_wide(f"hvT{d}", wb["Lp1_v"][:], Bv[d][:], n1, psN)
               for d in range(3)]
        tabsb = n1m.tile([128, NBLK, C4], bf16, name="tabsb", tag="tabsb")
        for b in range(NBLK):
            pm = psN.tile([128, C], f32, tag="tab")
            nc.tensor.matmul(pm[:], Bs[:, b * 128:(b + 1) * 128], combS[:],
                             start=True, stop=True)
            nc.scalar.copy(tabsb[:, b, 0:C], pm[:])
            for d in range(3):
                pm2 = psN.tile([128, C], f32, tag="tab")
                nc.tensor.matmul(pm2[:], Bv[d][:, b * 128:(b + 1) * 128],
                                 combV[:], start=True, stop=True)
                nc.scalar.copy(tabsb[:, b, C * (1 + d):C * (2 + d)], pm2[:])
        if debug:
            nc.gpsimd.dma_start(dbg["dbg_tab"][:], tabsb[:])
        bounce = dramp.tile([NLOC, C4], bf16, tag="bounce")
        nc.sync.dma_start(bounce[:].rearrange("(b p) c -> p b c", p=128),
                          tabsb[:])
        tabdram = dramp.tile([NCORES * NLOC, C4], bf16, addr_space="Shared",
                             tag="tabdram")
        nc.gpsimd.collective_compute(
            "AllGather", OP.bypass, replica_groups=[list(range(NCORES))],
            ins=[bounce[:]], outs=[tabdram[:]])
        psN.release()
        n1m.release()

        # ---- skip-TP (depends only on hvT): overlaps with phase 4 ----
        sc_all = n1.tile([128, NBLK, 3, C], bf16, name="sc_all", tag="sc_all")
        na_sb = n1.tile([128, NBLK, NE], f32, name="na_sb", tag="na_sb")
        nc.sync.dma_start(na_sb[:], na_in[:].rearrange("(b p) e -> p b e", p=128))
        n5a = tc.alloc_tile_pool(name="n5a", bufs=2)
        ps5a = tc.alloc_tile_pool(name="ps5a", bufs=1, space="PSUM")
        for b in range(NBLK):
            sc = n5a.tile([128, 3, C], f32, tag="sc5")
            gp = ps5a.tile([128, 3, C], f32, tag="gp5")
            for s in range(NE):
                for d in range(3):
                    nc.tensor.matmul(gp[:, d, :],
                                     hvT[d][:, b * 128:(b + 1) * 128],
                                     wsk[:, s, :], start=True, stop=True)
                if s == 0:
                    nc.vector.tensor_scalar(sc[:], gp[:], na_sb[:, b, 0:1],
                                            None, op0=OP.mult)
                else:
                    nc.vector.scalar_tensor_tensor(
                        sc[:], gp[:], na_sb[:, b, s:s + 1], sc[:],
                        op0=OP.mult, op1=OP.add)
            nc.vector.tensor_copy(sc_all[:, b, :, :], sc[:])

        # ---------------- phase 4: edges, layer 2 ---------------------
        p4 = tc.alloc_tile_pool(name="p4", bufs=2)
        p4c = tc.alloc_tile_pool(name="p4c", bufs=4)
        psA2 = tc.alloc_tile_pool(name="psA2", bufs=2, space="PSUM")
        psM2 = tc.alloc_tile_pool(name="psM2", bufs=2, space="PSUM")
        psL2 = tc.alloc_tile_pool(name="psL2", bufs=3, space="PSUM")

        w2dram = dramp.tile([128, tt, 4 * C], bf16, tag="w2dram")
        # 4a: radial MLP2 for all subs (no dependency on the AllGather)
        for b in range(NBLK):
            for si, (s0, sw) in enumerate(subs):
                ebs = sw * 128
                ts0 = b * t_blk + s0
                e0 = b * eb + s0 * 128
                featsT2 = load_feats(p4, e0, ebs, "featsT2")
                cinfo = run_mlp(featsT2, ["Wr2_1", "Wr2_2", "Wr2_3"], psM2,
                                p4c, "m2", ebs)
                w2sb = p4.tile([128, TSUB, 4 * C], bf16, tag="w2sb", bufs=2)
                for t in range(sw):
                    o = t * 128
                    a3, hi, o_, _ = find_chunk(cinfo, o)
                    lo = o - o_
                    w2p = psL2.tile([128, 4 * C], f32, tag="w2p")
                    nc.tensor.matmul(w2p[:],
                                     a3[hi * 64:(hi + 1) * 64, lo:lo + 128],
                                     w2o[hi * 64:(hi + 1) * 64, :],
                                     start=True, stop=True)
                    if t % 4 == 3:
                        nc.vector.tensor_copy(w2sb[:, t, :], w2p[:])
                    else:
                        nc.scalar.copy(w2sb[:, t, :], w2p[:])
                nc.sync.dma_start(w2dram[:, ts0:ts0 + sw, :], w2sb[:, :sw, :])

        # 4b: gather + messages + scatter
        for b in range(NBLK):
            acc2 = p4.tile([128, C4], f32, tag="acc2", bufs=2)
            for si, (s0, sw) in enumerate(subs):
                ebs = sw * 128
                ts0 = b * t_blk + s0
                e0 = b * eb + s0 * 128
                gat = p4.tile([128, TSUB, C4], bf16, tag="gat")
                nc.gpsimd.dma_gather(
                    gat[:, :sw, :], tabdram[:],
                    sndi[:, e0 // 16:(e0 + ebs) // 16],
                    num_idxs=ebs, num_idxs_reg=ebs, elem_size=C4)
                gat4 = gat[:].rearrange("p t (x c) -> p t x c", c=C)
                w2sb = p4.tile([128, TSUB, 4 * C], bf16, tag="w2l", bufs=3)
                nc.sync.dma_start(w2sb[:, :sw, :], w2dram[:, ts0:ts0 + sw, :])
                S = p4.tile([128, TSUB, 128], bf16, tag="S4", bufs=3)
                nc.gpsimd.dma_start(S[:, :sw, :], sdram[:, ts0:ts0 + sw, :])
                Sy = p4.tile([128, TSUB, 3, 128], bf16, tag="Sy4", bufs=3)
                nc.gpsimd.dma_start(Sy[:, :sw, :, :],
                                    sydram[:, ts0:ts0 + sw, :, :])
                m4 = p4.tile([128, TSUB, 4, C], bf16, tag="m4")
                m5 = p4.tile([128, TSUB, 4, C], bf16, tag="m5")
                hsg = gat4[:, :sw, 0, :]
                nc.vector.tensor_tensor(m4[:, :sw, 0, :], w2sb[:, :sw, 0:C],
                                        hsg, op=OP.mult)
                nc.vector.tensor_tensor(m5[:, :sw, 3, :],
                                        w2sb[:, :sw, 2 * C:3 * C], hsg,
                                        op=OP.mult)
                for d in range(3):
                    hvg = gat4[:, :sw, 1 + d, :]
                    nc.vector.tensor_tensor(m4[:, :sw, 1 + d, :],
                                            w2sb[:, :sw, 3 * C:4 * C], hvg,
                                            op=OP.mult)
                    nc.vector.tensor_tensor(m5[:, :sw, d, :],
                                            w2sb[:, :sw, C:2 * C], hvg,
                                            op=OP.mult)
                A2 = psA2.tile([128, 4, C], f32, tag="A2")
                for t in range(sw):
                    nc.tensor.matmul(A2[:], S[:, t, :],
                                     m4[:, t, :, :], start=(t == 0), stop=False,
                                     skip_group_check=True)
                for d in range(3):
                    for t in range(sw):
                        nc.tensor.matmul(A2[:, 1 + d, :], Sy[:, t, d, :],
                                         m5[:, t, 3, :], start=False,
                                         stop=False, skip_group_check=True)
                for d in range(3):
                    for t in range(sw):
                        sp = (t == sw - 1) and (d == 2)
                        nc.tensor.matmul(A2[:, 0, :], Sy[:, t, d, :],
                                         m5[:, t, d, :], start=False, stop=sp,
                                         skip_group_check=True)
                a2f = A2[:].rearrange("p x c -> p (x c)")
                if si == 0:
                    nc.scalar.copy(acc2[:], a2f)
                else:
                    nc.vector.tensor_tensor(acc2[:], a2f, acc2[:], op=OP.add)
            nc.vector.tensor_copy(a2sb[:, b, :], acc2[:])
            if debug and b == 0:
                nc.gpsimd.dma_start(dbg["dbg_gat"][:, 0:min(TSUB, t_blk)],
                                    gat[:, 0:min(TSUB, t_blk)])
                nc.gpsimd.dma_start(dbg["dbg_m4"][:, 0:min(TSUB, t_blk)],
                                    m4[:, 0:min(TSUB, t_blk)])

        for p in (psL2, psM2, psA2, p4c, p4, ps5a, n5a):
            p.release()
        if debug:
            nc.gpsimd.dma_start(dbg["dbg_a2"][:], a2sb[:])

        # ---------------- phase 5: node layer 2 + outputs -------------
        n2 = tc.alloc_tile_pool(name="n2", bufs=1)
        psN2 = tc.alloc_tile_pool(name="psN2", bufs=2, space="PSUM")
        A2sT = transpose_blocks(a2sb[:], 0, "A2sT", n2, psN2)
        A2vT = [transpose_blocks(a2sb[:], C * (1 + d), f"A2vT{d}", n2, psN2)
                for d in range(3)]
        A2sL = mm_wide("A2sL", wb["Wlin2_s"][:], A2sT[:], n2, psN2)
        A2vL = [mm_wide(f"A2vL{d}", wb["Wlin2_v"][:], A2vT[d][:], n2, psN2)
                for d in range(3)]
        P2w = {n: mm_wide("w" + n, wb[n][:], naTs[:], n2, psN2)
               for n in ["P2v1", "P2sv"]}
        g2 = n2.tile([C, NLOC], bf16, name="g2", tag="g2")
        nc.vector.tensor_tensor(g2[:], P2w["P2sv"][:], A2sL[:], op=OP.mult)
        nc.vector.tensor_tensor(g2[:], g2[:], P2w["P2v1"][:], op=OP.add)
        B2v = []
        for d in range(3):
            b2d = n2.tile([C, NLOC], bf16, name=f"B2v{d}", tag=f"B2v{d}")
            nc.vector.tensor_tensor(b2d[:], g2[:], A2vL[d][:], op=OP.mult)
            B2v.append(b2d)
        psN2.release()

        # h2 (node-major) = B2v @ Lp2_v + skip-TP, then back to feat-major
        n2t = tc.alloc_tile_pool(name="n2t", bufs=2)
        psH = tc.alloc_tile_pool(name="psH", bufs=1, space="PSUM")
        psHt = tc.alloc_tile_pool(name="psHt", bufs=2, space="PSUM")
        h2T = [n2.tile([C, NLOC], bf16, name=f"h2T{d}", tag=f"h2T{d}")
               for d in range(3)]
        for b in range(NBLK):
            hp = psH.tile([128, 3, C], f32, tag="h2p")
            for d in range(3):
                nc.tensor.matmul(hp[:, d, :], B2v[d][:, b * 128:(b + 1) * 128],
                                 wb["Lp2_v"][:], start=True, stop=True)
            sc16 = n2t.tile([128, 3, C], bf16, tag="sc16")
            nc.vector.tensor_tensor(sc16[:], hp[:], sc_all[:, b, :, :],
                                    op=OP.add)
            for d in range(3):
                tp = psHt.tile([128, 128], bf16, tag="tph")
                nc.tensor.transpose(tp[:], sc16[:, d, :], ident[:])
                nc.scalar.copy(h2T[d][:, b * 128:(b + 1) * 128], tp[:])
        psHt.release()
        psH.release()

        psD = tc.alloc_tile_pool(name="psD", bufs=2, space="PSUM")
        dipf = [n2.tile([1, NLOC], f32, name=f"dipf{d}", tag=f"dipf{d}")
                for d in range(3)]
        for d in range(3):
            for o, w_ in nchunks:
                dp = psD.tile([1, 512], f32, tag="dp")
                nc.tensor.matmul(dp[:, :w_], wb["R1c"][:], hvT[d][:, o:o + w_],
                                 start=True, stop=False)
                nc.tensor.matmul(dp[:, :w_], qcol[:], h2T[d][:, o:o + w_],
                                 start=False, stop=True)
                nc.scalar.copy(dipf[d][0:1, o:o + w_], dp[:, :w_])

        posb = n2.tile([128, NBLK, 3], f32, name="posb", tag="posb")
        nc.sync.dma_start(posb[:], posn_in[:].rearrange("(b p) c -> p b c", p=128))
        chgb = n2.tile([128, NBLK], f32, name="chgb", tag="chgb")
        nc.sync.dma_start(chgb[:], chg_in[:].rearrange("(b p) c -> p (b c)", p=128))
        bohb = n2.tile([128, NBLK, G], f32, name="bohb", tag="bohb")
        nc.sync.dma_start(bohb[:], boh_in[:].rearrange("(b p) g -> p b g", p=128))
        dipo = n2.tile([128, NBLK, 3], f32, name="dipo", tag="dipo")
        gs = psD.tile([G, 3], f32, tag="gs", bufs=1)
        for b in range(NBLK):
            for d in range(3):
                dpp = psD.tile([128, 1], f32, tag="dpp")
                nc.tensor.transpose(dpp[:], dipf[d][0:1, b * 128:(b + 1) * 128],
                                    identf[0:1, 0:1])
                nc.scalar.copy(dipo[:, b, d:d + 1], dpp[:])
            cp = n2t.tile([128, 3], f32, tag="cp")
            nc.vector.tensor_scalar(cp[:], posb[:, b, :], chgb[:, b:b + 1],
                                    None, op0=OP.mult)
            nc.vector.tensor_tensor(cp[:], cp[:], dipo[:, b, :], op=OP.add)
            nc.tensor.matmul(gs[:], bohb[:, b, :], cp[:], start=(b == 0),
                             stop=(b == NBLK - 1))
        nc.sync.dma_start(out_dip[:].rearrange("(b p) c -> p b c", p=128),
                          dipo[:])
        if debug:
            nc.gpsimd.dma_start(dbg["dbg_hvT0"][:], hvT[0][:])
            nc.gpsimd.dma_start(dbg["dbg_Bs"][:], Bs[:])
            nc.gpsimd.dma_start(dbg["dbg_h2T0"][:], h2T[0][:])
            nc.gpsimd.dma_start(dbg["dbg_dipf0"][:], dipf[0][:])
        gso = n2.tile([G, 3], f32, name="gso", tag="gso")
        nc.scalar.copy(gso[:], gs[:])
        nc.sync.dma_start(out_gs[:], gso[:])

        for p in (psD, n2t, n2, n1t, n1):
            p.release()
        dramp.release()
        wp.release()

    nc.compile()
    return nc


_BUILD_CACHE = {}


def kernel(**inputs):
    in_maps, t_blk = prep_shards(inputs)
    nc = _BUILD_CACHE.get(t_blk)
    if nc is None:
        nc = build(t_blk)
        _BUILD_CACHE[t_blk] = nc
    res = run_bass_kernel_spmd(nc, in_maps, core_ids=list(range(NCORES)))
    dip = np.zeros((N, 3), np.float32)
    tot = np.zeros((G, 3), np.float32)
    for k in range(NCORES):
        dip[k * NPC:(k + 1) * NPC] = res.results[k]["dip"][:NPC]
        tot += res.results[k]["gsum"]
    return tot, dip


# revision 62
# speedup vs baseline: 1.2236x; 1.0048x over previous
"""AtomicDipolesMACE on 8 TRN2 NeuronCores.

Sharding: nodes are split into 8 contiguous ranges of 1250 (padded to 1280 =
10 blocks of 128). Each core owns the edges whose receiver falls in its range,
sorted and grouped by 128-node receiver block, each block segment padded to a
uniform T_BLK tiles of 128 edges. Scatters become per-block one-hot matmuls
(PSUM f32 accumulation, Yv folded into scaled one-hots); the layer-2 sender
gather is a dma_gather from an AllGathered bf16 node-feature table.
"""
import sys
sys.path.insert(0, "/opt/trn_rl_repo")
import numpy as np
import ml_dtypes

from concourse import bass, bacc, tile, mybir
from concourse.bass_utils import run_bass_kernel_spmd

f32 = mybir.dt.float32
bf16 = mybir.dt.bfloat16
i16 = mybir.dt.int16
i32 = mybir.dt.int32
AF = mybir.ActivationFunctionType
OP = mybir.AluOpType
nbf16 = ml_dtypes.bfloat16
USE_SILU = True
PACK_MLP = True
TSUB = 6

N, E, C, NE, G, NB = 10000, 160000, 128, 10, 16, 8
RMAX, AVG, SQ3 = 5.0, 16.0, 3.0 ** 0.5
NCORES = 8
NPC = N // NCORES          # 1250 real nodes per core
NBLK = 10                  # node blocks per core
NLOC = NBLK * 128          # 1280 padded local nodes
C4 = 4 * C                 # 512: table row / message width

WSHAPE = dict(W_embT=[C, NE], W_up1=[C, C], Wr1_1=[NB, 64], Wr1_2=[64, 64],
              Wr1_3=[64, 64], Wr1_o=[64, 2 * C], Wr2_1=[NB, 64],
              Wr2_2=[64, 64], Wr2_3=[64, 64], Wr2_o=[64, 4 * C],
              Wlin1_s=[C, C], Wlin1_v=[C, C], Wlin2_s=[C, C],
              Wlin2_v=[C, C], Lp1_sT=[C, C], Lp1_vT=[C, C], Lp1_v=[C, C],
              Lp2_v=[C, C], Wup2_s=[C, C], Wup2_v=[C, C],
              P1s1=[NE, C], P1ss=[NE, C], P1vv=[NE, C], P1v1=[NE, C],
              P1sv=[NE, C], P2v1=[NE, C], P2sv=[NE, C],
              R1c=[C, 1], RmidT=[16, C], Routc=[16, 1], kpi5=[128, NB])


# ---------------------------------------------------------------- host prep
def prep_shards(inp):
    snd = np.asarray(inp["edge_index"][0], dtype=np.int64)
    rcv = np.asarray(inp["edge_index"][1], dtype=np.int64)
    pos = np.asarray(inp["positions"], dtype=np.float32)
    na = np.asarray(inp["node_attrs"], dtype=np.float32)
    shf = np.asarray(inp["shifts"], dtype=np.float32)
    chg = np.asarray(inp["charges"], dtype=np.float32)
    bat = np.asarray(inp["batch"], dtype=np.int64)

    core = rcv // NPC
    loc = rcv - core * NPC
    blk = loc // 128

    order = np.lexsort((blk, core))
    snd_s, rcv_s, loc_s = snd[order], rcv[order], loc[order]
    core_s, blk_s = core[order], blk[order]
    shf_s = shf[order]

    counts = np.zeros((NCORES, NBLK), dtype=np.int64)
    np.add.at(counts, (core_s, blk_s), 1)
    t_blk = max(1, int(np.max((counts + 127) // 128)))
    eb = t_blk * 128
    epad = NBLK * eb
    tt = NBLK * t_blk

    snd_pad = (snd_s // NPC) * NLOC + (snd_s % NPC)  # padded-table row index

    starts = np.zeros(NCORES * NBLK, dtype=np.int64)
    starts[1:] = np.cumsum(counts.reshape(-1))[:-1]
    starts = starts.reshape(NCORES, NBLK)

    w = weights_prep(inp)
    in_maps = []
    for k in range(NCORES):
        pos_s = np.zeros((epad, 3), np.float32)
        pos_r = np.ones((epad, 3), np.float32)
        shfe = np.zeros((epad, 3), np.float32)
        attrsT = np.zeros((NE, epad), np.float32)
        sndp = np.zeros(epad, np.int64)
        rcvb = -np.ones(epad, np.float32)
        for b in range(NBLK):
            s0, n = starts[k, b], counts[k, b]
            sl = slice(s0, s0 + n)
            d0 = b * eb
            pos_s[d0:d0 + n] = pos[snd_s[sl]]
            pos_r[d0:d0 + n] = pos[rcv_s[sl]]
            shfe[d0:d0 + n] = shf_s[sl]
            attrsT[:, d0:d0 + n] = na[snd_s[sl]].T
            sndp[d0:d0 + n] = snd_pad[sl]
            rcvb[d0:d0 + n] = (loc_s[sl] - b * 128).astype(np.float32)

        def emaj(a):  # [epad, d] -> [128, tt, d]  (edge i -> (i%128, i//128))
            return np.ascontiguousarray(a.reshape(tt, 128, -1).transpose(1, 0, 2))

        sndw = np.zeros((128, epad // 16), np.int16)
        for b in range(NBLK):
            seg = sndp[b * eb:(b + 1) * eb].reshape(eb // 16, 16).T
            sndw[:, b * (eb // 16):(b + 1) * (eb // 16)] = np.tile(seg, (8, 1))

        nl0 = k * NPC
        na_nm = np.zeros((NLOC, NE), np.float32)
        na_nm[:NPC] = na[nl0:nl0 + NPC]
        pos_nm = np.zeros((NLOC, 3), np.float32)
        pos_nm[:NPC] = pos[nl0:nl0 + NPC]
        chg_nm = np.zeros((NLOC, 1), np.float32)
        chg_nm[:NPC, 0] = chg[nl0:nl0 + NPC]
        boh = np.zeros((NLOC, G), np.float32)
        boh[np.arange(NPC), bat[nl0:nl0 + NPC]] = 1.0

        m = dict(
            pos_s=emaj(pos_s), pos_r=emaj(pos_r), shfe=emaj(shfe),
            attrsT=attrsT.astype(nbf16), sndw=sndw,
            rcvb=emaj(rcvb)[:, :, 0].astype(nbf16),
            na_nm=na_nm, naT=np.ascontiguousarray(na_nm.T).astype(nbf16),
            pos_nm=pos_nm, chg_nm=chg_nm, boh=boh,
        )
        m.update(w)
        in_maps.append(m)
    return in_maps, t_blk


def weights_prep(inp):
    g = lambda k: np.ascontiguousarray(np.asarray(inp[k], dtype=np.float32))
    kvec = (np.pi / RMAX) * np.arange(1, NB + 1, dtype=np.float32)
    return dict(
        W_embT=g("W_emb").T.copy(), W_up1=g("W_up1"),
        Wr1_1=g("Wr1_1"), Wr1_2=g("Wr1_2"), Wr1_3=g("Wr1_3"), Wr1_o=g("Wr1_o"),
        Wr2_1=g("Wr2_1"), Wr2_2=g("Wr2_2"), Wr2_3=g("Wr2_3"), Wr2_o=g("Wr2_o"),
        Wlin1_s=g("Wlin1_s"), Wlin1_v=g("Wlin1_v"),
        Wlin2_s=g("Wlin2_s"), Wlin2_v=g("Wlin2_v"),
        Lp1_sT=g("Lp1_s").T.copy(), Lp1_vT=g("Lp1_v").T.copy(),
        Lp1_v=g("Lp1_v"), Lp2_v=g("Lp2_v"),
        Wup2_s=g("Wup2_s"), Wup2_v=g("Wup2_v"),
        P1s1=g("P1_s1"), P1ss=g("P1_ss"), P1vv=g("P1_vv"),
        P1v1=g("P1_v1"), P1sv=g("P1_sv"), P2v1=g("P2_v1"), P2sv=g("P2_sv"),
        Wsk=g("Wsk"),
        R1c=g("R1").reshape(C, 1), RmidT=g("Rmid").T.copy(),
        Routc=g("Rout").reshape(16, 1),
        kpi5=np.tile(kvec, (128, 1)),
    )


# ---------------------------------------------------------------- builder
def build(t_blk, debug=False):
    eb = t_blk * 128
    tt = NBLK * t_blk
    epad = NBLK * eb
    nc = bacc.Bacc(None, target_bir_lowering=False, num_devices=NCORES)

    def din(name, shape, dt=f32):
        return nc.declare_dram_parameter(name, shape, dt, isOutput=False)

    pos_s = din("pos_s", [128, tt, 3]); pos_r = din("pos_r", [128, tt, 3])
    shfe = din("shfe", [128, tt, 3])
    attrsT_in = din("attrsT", [NE, epad], bf16)
    sndw_in = din("sndw", [128, epad // 16], i16)
    rcvb_in = din("rcvb", [128, tt], bf16)
    na_in = din("na_nm", [NLOC, NE]); naT_in = din("naT", [NE, NLOC], bf16)
    posn_in = din("pos_nm", [NLOC, 3]); chg_in = din("chg_nm", [NLOC, 1])
    boh_in = din("boh", [NLOC, G])
    win = {n: din(n, WSHAPE[n]) for n in WSHAPE}
    wsk_in = din("Wsk", [NE, C, C])
    out_dip = nc.declare_dram_parameter("dip", [NLOC, 3], f32, isOutput=True)
    out_gs = nc.declare_dram_parameter("gsum", [16, 3], f32, isOutput=True)
    dbg = {}
    if debug:
        for nm, shp in [("dbg_a1", [128, NBLK, C4]), ("dbg_tab", [128, NBLK, C4]),
                        ("dbg_a2", [128, NBLK, C4]), ("dbg_hvT0", [C, NLOC]),
                        ("dbg_Bs", [C, NLOC]), ("dbg_h2T0", [C, NLOC]),
                        ("dbg_dipf0", [1, NLOC]), ("dbg_gat", [128, t_blk, C4]),
                        ("dbg_m4", [128, t_blk, 4, C]),
                        ("dbg_ft", [NB, epad]), ("dbg_uv", [128, t_blk, 2, C]),
                        ("dbg_S", [128, t_blk, 128]),
                        ("dbg_fts", [128, t_blk, NB]),
                        ("dbg_ln", [128, t_blk]), ("dbg_arg", [128, t_blk, NB]),
                        ("dbg_snb", [128, t_blk, NB]),
                        ("dbg_bes", [128, t_blk, NB]),
                        ("dbg_fc", [128, t_blk])]:
            dbg[nm] = nc.declare_dram_parameter(nm, shp, f32, isOutput=True)

    nchunks = []
    _o = 0
    while _o < NLOC:
        nchunks.append((_o, min(512, NLOC - _o)))
        _o += nchunks[-1][1]

    def echunks(ebs):
        half = ebs // 2
        if ebs <= 1024 and half % 128 == 0 and half > 0:
            return [(0, half), (half, ebs - half)]
        o, out = 0, []
        while o < ebs:
            w_ = min(512, ebs - o)
            out.append((o, w_))
            o += w_
        return out

    with tile.TileContext(nc) as tc:
        wp = tc.alloc_tile_pool(name="wp", bufs=1)
        dramp = tc.alloc_tile_pool(name="dram", bufs=1, space="DRAM")

        # ---- load + prep weights -------------------------------------
        wf = {}
        for n in WSHAPE:
            wf[n] = wp.tile(WSHAPE[n], f32, name="f" + n, tag="f" + n)
            nc.sync.dma_start(wf[n][:], win[n][:])
        wsk = wp.tile([C, NE, C], bf16, name="wsk", tag="wsk")
        nc.gpsimd.dma_start(wsk[:], wsk_in[:].transpose([1, 0, 2]))
        naTs = wp.tile([NE, NLOC], bf16, name="naTs", tag="naTs")
        nc.sync.dma_start(naTs[:], naT_in[:])
        rcvb = wp.tile([128, tt], bf16, name="rcvb", tag="rcvb")
        nc.sync.dma_start(rcvb[:], rcvb_in[:])
        sndi = wp.tile([128, epad // 16], i16, name="sndi", tag="sndi")
        nc.sync.dma_start(sndi[:], sndw_in[:])

        def tobf(name, src_ap, shape, scale=None):
            t = wp.tile(shape, bf16, name=name, tag=name)
            if scale is None:
                nc.vector.tensor_copy(t[:], src_ap)
            else:
                nc.vector.tensor_scalar(t[:], src_ap, float(scale), None,
                                        op0=OP.mult)
            return t

        wb = {}
        for n in ["Wr1_1", "Wr1_2", "Wr1_3", "Wr1_o", "Wr2_1", "Wr2_2",
                  "Wr2_3", "Lp1_v", "Lp2_v", "P1s1", "P1ss", "P1v1", "P1sv",
                  "P2v1", "P2sv", "R1c"]:
            wb[n] = tobf("b" + n, wf[n][:], WSHAPE[n])
        for n in ["Wlin1_s", "Wlin1_v", "Wlin2_s", "Wlin2_v"]:
            wb[n] = tobf("b" + n, wf[n][:], WSHAPE[n], scale=1.0 / AVG)
        wb["P1vv"] = tobf("bP1vv", wf["P1vv"][:], WSHAPE["P1vv"], scale=1.0 / SQ3)
        w2o = wp.tile([128, 4 * C], bf16, name="w2o", tag="w2o")
        for h in (0, 64):
            nc.vector.tensor_copy(w2o[h:h + 64, :], wf["Wr2_o"][:])
            nc.vector.tensor_scalar(w2o[h:h + 64, C:2 * C],
                                    wf["Wr2_o"][:, C:2 * C],
                                    1.0 / SQ3, None, op0=OP.mult)
        w1o = wp.tile([128, 2 * C], bf16, name="w1o", tag="w1o")
        for h in (0, 64):
            nc.vector.tensor_copy(w1o[h:h + 64, :], wf["Wr1_o"][:])
        # hidden-layer MLP weights duplicated into the upper PE quadrant
        wb2 = {}
        for n in ["Wr1_1", "Wr1_2", "Wr1_3", "Wr2_1", "Wr2_2", "Wr2_3"]:
            kk = WSHAPE[n][0]
            t = wp.tile([128, 64], bf16, name="q" + n, tag="q" + n)
            for h in (0, 64):
                nc.vector.tensor_copy(t[h:h + kk, :], wf[n][:])
            wb2[n] = t

        psw = tc.alloc_tile_pool(name="psw", bufs=2, space="PSUM")
        eup = psw.tile([NE, C], f32, tag="pw")
        nc.tensor.matmul(eup[:], wf["W_embT"][:], wf["W_up1"][:],
                         start=True, stop=True)
        embup = tobf("embup", eup[:], [NE, C])
        cs = psw.tile([C, C], f32, tag="pw")
        nc.tensor.matmul(cs[:], wf["Lp1_sT"][:], wf["Wup2_s"][:],
                         start=True, stop=True)
        combS = tobf("combS", cs[:], [C, C])
        cv = psw.tile([C, C], f32, tag="pw")
        nc.tensor.matmul(cv[:], wf["Lp1_vT"][:], wf["Wup2_v"][:],
                         start=True, stop=True)
        combV = tobf("combV", cv[:], [C, C])
        qp = psw.tile([C, 1], f32, tag="pw")
        nc.tensor.matmul(qp[:], wf["RmidT"][:], wf["Routc"][:],
                         start=True, stop=True)
        qcol = wp.tile([C, 1], bf16, name="qcol", tag="qcol")
        nc.vector.tensor_scalar(qcol[:], qp[:], 0.5, None, op0=OP.mult)

        ioi = wp.tile([128, t_blk, 128], i32, name="ioi", tag="ioi")
        nc.gpsimd.iota(ioi[:], pattern=[[0, t_blk], [1, 128]], base=0,
                       channel_multiplier=0)
        iob = wp.tile([128, t_blk, 128], bf16, name="iob", tag="iob")
        nc.vector.tensor_copy(iob[:], ioi[:])
        idi = wp.tile([128, 128], i32, name="idi", tag="idi")
        nc.gpsimd.iota(idi[:], pattern=[[1, 128]], base=0, channel_multiplier=-1)
        idf = wp.tile([128, 128], f32, name="idf", tag="idf")
        nc.vector.tensor_copy(idf[:], idi[:])
        ident = wp.tile([128, 128], bf16, name="ident", tag="ident")
        nc.vector.tensor_scalar(ident[:], idf[:], 0.0, None, op0=OP.is_equal)
        identf = wp.tile([128, 128], f32, name="identf", tag="identf")
        nc.vector.tensor_scalar(identf[:], idf[:], 0.0, None, op0=OP.is_equal)
        psw.release()

        eps12 = wp.tile([128, 1], f32, name="eps12", tag="eps12")
        nc.vector.memset(eps12[:], 1e-12)
        mpi = wp.tile([128, 1], f32, name="mpi", tag="mpi")
        nc.vector.memset(mpi[:], -np.pi)
        yv_all = wp.tile([128, tt, 3], bf16, name="yv_all", tag="yv_all")
        a1sb = wp.tile([128, NBLK, C4], bf16, name="a1sb", tag="a1sb")
        a2sb = wp.tile([128, NBLK, C4], bf16, name="a2sb", tag="a2sb")

        # ---------------- phase 1: edges, layer 1 ---------------------
        featsd = dramp.tile([NB, epad], bf16, tag="featsd")
        sdram = dramp.tile([128, tt, 128], bf16, tag="sdram")
        sydram = dramp.tile([128, tt, 3, 128], bf16, tag="sydram")
        p1 = tc.alloc_tile_pool(name="p1", bufs=2)
        p1c = tc.alloc_tile_pool(name="p1c", bufs=3)
        psA = tc.alloc_tile_pool(name="psA", bufs=2, space="PSUM")
        psM = tc.alloc_tile_pool(name="psM", bufs=2, space="PSUM")
        psL = tc.alloc_tile_pool(name="psL", bufs=2, space="PSUM")
        psE = tc.alloc_tile_pool(name="psE", bufs=2, space="PSUM")

        def run_mlp(ftile, names, psum_pool, act_pool, tagp, ebs):
            """3-layer silu MLP over ebs edges, partition-packed pairs.
            Returns per-512-chunk (a3_tile, half, chunk_off, width)."""
            offs = echunks(ebs)
            cinfo = []
            step = 2 if PACK_MLP else 1
            for pc in range(0, len(offs), step):
                pair = offs[pc:pc + step]
                cw = max(w_ for _, w_ in pair)
                npart = 64 * len(pair)
                a1 = act_pool.tile([128, cw], bf16, tag=tagp + "a1")
                a2 = act_pool.tile([128, cw], bf16, tag=tagp + "a2")
                a3 = act_pool.tile([128, cw], bf16, tag=tagp + "a3")
                prevs = [None, a1, a2]
                outs = [a1, a2, a3]
                for li in range(3):
                    hp = psum_pool.tile([128, cw], f32, tag=tagp + "h")
                    kk = NB if li == 0 else 64
                    for hi, (o_, w_) in enumerate(pair):
                        h = hi * 64
                        if li == 0:
                            rhs = ftile[h:h + NB, o_: o_ + w_]
                        else:
                            rhs = prevs[li][h:h + 64, :w_]
                        nc.tensor.matmul(hp[h:h + 64, :w_],
                                         wb2[names[li]][h:h + kk, :], rhs,
                                         start=True, stop=True)
                    full = len(pair) == 2 and pair[0][1] == pair[1][1]
                    regions = ([(0, 128, cw)] if full else
                               [(hi * 64, hi * 64 + 64, w_)
                                for hi, (o_, w_) in enumerate(pair)])
                    if USE_SILU:
                        for (h0, h1, ww) in regions:
                            nc.scalar.activation(outs[li][h0:h1, :ww],
                                                 hp[h0:h1, :ww], AF.Silu)
                    else:
                        sg = act_pool.tile([128, cw], bf16, tag=tagp + "sg")
                        for (h0, h1, ww) in regions:
                            nc.scalar.activation(sg[h0:h1, :ww],
                                                 hp[h0:h1, :ww], AF.Sigmoid)
                            nc.vector.tensor_tensor(
                                outs[li][h0:h1, :ww], sg[h0:h1, :ww],
                                hp[h0:h1, :ww], op=OP.mult)
                for hi, (o_, w_) in enumerate(pair):
                    cinfo.append((a3, hi, o_, w_))
            return cinfo

        for b in range(NBLK):
            ts0 = b * t_blk
            featsT = p1.tile([NB, eb], bf16, tag="featsT")
            ps_ = p1.tile([128, t_blk, 3], f32, tag="ps")
            pr_ = p1.tile([128, t_blk, 3], f32, tag="pr")
            sh_ = p1.tile([128, t_blk, 3], f32, tag="sh")
            nc.sync.dma_start(ps_[:], pos_s[:, ts0:ts0 + t_blk, :])
            nc.sync.dma_start(pr_[:], pos_r[:, ts0:ts0 + t_blk, :])
            nc.sync.dma_start(sh_[:], shfe[:, ts0:ts0 + t_blk, :])
            vec = p1.tile([128, t_blk, 3], f32, tag="vec")
            nc.vector.tensor_tensor(vec[:], pr_[:], ps_[:], op=OP.subtract)
            nc.vector.tensor_tensor(vec[:], vec[:], sh_[:], op=OP.add)
            sq = p1.tile([128, t_blk, 3], f32, tag="sq")
            nc.vector.tensor_tensor(sq[:], vec[:], vec[:], op=OP.mult)
            ln2 = p1.tile([128, t_blk], f32, tag="ln2")
            nc.vector.tensor_reduce(ln2[:], sq[:], axis=mybir.AxisListType.X,
                                    op=OP.add)
            ln = p1.tile([128, t_blk], f32, tag="ln")
            nc.scalar.activation(ln[:], ln2[:], AF.Sqrt, bias=eps12[:])
            rl = p1.tile([128, t_blk], f32, tag="rl")
            nc.vector.reciprocal(rl[:], ln[:])
            rl3 = rl[:].unsqueeze(-1).broadcast_to([128, t_blk, 3])
            nc.vector.scalar_tensor_tensor(
                yv_all[:, ts0:ts0 + t_blk, :], vec[:], SQ3, rl3,
                op0=OP.mult, op1=OP.mult)
            kb = wf["kpi5"][:].unsqueeze(1).broadcast_to([128, t_blk, NB])
            lnb = ln[:].unsqueeze(-1).broadcast_to([128, t_blk, NB])
            rlb = rl[:].unsqueeze(-1).broadcast_to([128, t_blk, NB])
            arg = p1.tile([128, t_blk, NB], f32, tag="arg")
            nc.vector.tensor_tensor(arg[:], kb, lnb, op=OP.mult)
            yq = p1.tile([128, t_blk, NB], f32, tag="yq")
            nc.vector.tensor_scalar(yq[:], arg[:], 1.0 / (2 * np.pi), None,
                                    op0=OP.mult)
            yqi = p1.tile([128, t_blk, NB], i32, tag="yqi")
            nc.vector.tensor_copy(yqi[:], yq[:])
            nc.vector.tensor_copy(yq[:], yqi[:])
            # r = arg - 2pi*k is in (-pi, 2pi) whether k was trunc or round;
            # fold the (pi, 2pi) tail back by another 2pi
            nc.vector.scalar_tensor_tensor(arg[:], yq[:], -2.0 * np.pi,
                                           arg[:], op0=OP.mult, op1=OP.add)
            nc.vector.tensor_scalar(yq[:], arg[:], np.pi, None, op0=OP.is_gt)
            nc.vector.scalar_tensor_tensor(arg[:], yq[:], -2.0 * np.pi,
                                           arg[:], op0=OP.mult, op1=OP.add)
            snb = p1.tile([128, t_blk, NB], f32, tag="snb")
            nc.scalar.activation(snb[:], arg[:], AF.Sin)
            bes = p1.tile([128, t_blk, NB], f32, tag="bes")
            nc.vector.scalar_tensor_tensor(bes[:], snb[:], (2.0 / RMAX) ** 0.5,
                                           rlb, op0=OP.mult, op1=OP.mult)
            u = p1.tile([128, t_blk], f32, tag="u")
            nc.vector.tensor_scalar(u[:], ln[:], 1.0 / RMAX, None, op0=OP.mult)
            u2 = p1.tile([128, t_blk], f32, tag="u2")
            nc.vector.tensor_tensor(u2[:], u[:], u[:], op=OP.mult)
            u4 = p1.tile([128, t_blk], f32, tag="u4")
            nc.vector.tensor_tensor(u4[:], u2[:], u2[:], op=OP.mult)
            u5 = p1.tile([128, t_blk], f32, tag="u5")
            nc.vector.tensor_tensor(u5[:], u4[:], u[:], op=OP.mult)
            w_ = p1.tile([128, t_blk], f32, tag="w_")
            nc.vector.tensor_scalar(w_[:], u[:], -15.0, 35.0, op0=OP.mult,
                                    op1=OP.add)
            nc.vector.tensor_tensor(w_[:], w_[:], u[:], op=OP.mult)
            nc.vector.tensor_scalar(w_[:], w_[:], -21.0, None, op0=OP.add)
            nc.vector.tensor_tensor(w_[:], w_[:], u5[:], op=OP.mult)
            nc.vector.tensor_scalar(w_[:], w_[:], 1.0, None, op0=OP.add)
            msk = p1.tile([128, t_blk], f32, tag="msk")
            nc.vector.tensor_scalar(msk[:], u[:], 1.0, None, op0=OP.is_lt)
            fc = p1.tile([128, t_blk], f32, tag="fc")
            nc.vector.tensor_tensor(fc[:], w_[:], msk[:], op=OP.mult)
            fcb = fc[:].unsqueeze(-1).broadcast_to([128, t_blk, NB])
            fts = p1.tile([128, t_blk, NB], bf16, tag="fts")
            nc.vector.tensor_tensor(fts[:], bes[:], fcb, op=OP.mult)
            for t in range(t_blk):
                fp = psL.tile([NB, 128], bf16, tag="w1p")
                nc.tensor.transpose(fp[:], fts[:, t, :], ident[:])
                nc.scalar.copy(featsT[:, t * 128:(t + 1) * 128], fp[:])
            nc.sync.dma_start(featsd[:, b * eb:(b + 1) * eb], featsT[:])
            if debug and b == 0:
                nc.gpsimd.dma_start(dbg["dbg_fts"][:], fts[:])
                nc.gpsimd.dma_start(dbg["dbg_ln"][:], ln[:])
                nc.gpsimd.dma_start(dbg["dbg_arg"][:], arg[:])
                nc.gpsimd.dma_start(dbg["dbg_snb"][:], snb[:])
                nc.gpsimd.dma_start(dbg["dbg_bes"][:], bes[:])
                nc.gpsimd.dma_start(dbg["dbg_fc"][:], fc[:])
        # ---------------- phase 1b: MLP1 + messages + scatter ---------

        def load_feats(pool, e0, ebs, tag):
            ft = pool.tile([128, TSUB * 128], bf16, tag=tag)
            nc.sync.dma_start(ft[0:NB, :ebs], featsd[:, e0:e0 + ebs])
            nc.sync.dma_start(ft[64:64 + NB, :ebs], ft[0:NB, :ebs])
            return ft

        def find_chunk(cinfo, o):
            for a3, hi, o_, w_ in cinfo:
                if o_ <= o < o_ + w_:
                    return a3, hi, o_, w_
            raise AssertionError(o)

        subs = [(s0, min(TSUB, t_blk - s0)) for s0 in range(0, t_blk, TSUB)]

        for b in range(NBLK):
            acc1 = p1.tile([128, C4], f32, tag="acc1", bufs=2)
            for si, (s0, sw) in enumerate(subs):
                ebs = sw * 128
                ts0 = b * t_blk + s0
                e0 = b * eb + s0 * 128
                attrs_sb = p1.tile([NE, TSUB * 128], bf16, tag="attrs_sb")
                nc.sync.dma_start(attrs_sb[:NE, :ebs], attrsT_in[:, e0:e0 + ebs])
                ft = load_feats(p1, e0, ebs, "featsT1")
                cinfo = run_mlp(ft, ["Wr1_1", "Wr1_2", "Wr1_3"], psM, p1c,
                                "m1", ebs)
                uv = p1.tile([128, TSUB, 2, C], bf16, tag="uv", bufs=3)
                for t in range(sw):
                    o = t * 128
                    a3, hi, o_, _ = find_chunk(cinfo, o)
                    lo = o - o_
                    w1p = psL.tile([128, 2 * C], f32, tag="w1p")
                    nc.tensor.matmul(w1p[:],
                                     a3[hi * 64:(hi + 1) * 64, lo:lo + 128],
                                     w1o[hi * 64:(hi + 1) * 64, :],
                                     start=True, stop=True)
                    ep = psE.tile([128, C], f32, tag="ep")
                    nc.tensor.matmul(ep[:], attrs_sb[:NE, o:o + 128],
                                     embup[:], start=True, stop=True)
                    heb = p1c.tile([128, C], bf16, tag="heb")
                    nc.scalar.copy(heb[:], ep[:])
                    epb = heb[:].unsqueeze(1).broadcast_to([128, 2, C])
                    w1v = w1p[:].rearrange("p (x c) -> p x c", c=C)
                    nc.vector.tensor_tensor(uv[:, t, :, :], w1v, epb, op=OP.mult)
                rb = rcvb[:, ts0:ts0 + sw].unsqueeze(-1).broadcast_to(
                    [128, sw, 128])
                S = p1.tile([128, TSUB, 128], bf16, tag="S", bufs=3)
                nc.vector.tensor_tensor(S[:, :sw, :], iob[:, :sw, :], rb,
                                        op=OP.is_equal)
                Sy = p1.tile([128, TSUB, 3, 128], bf16, tag="Sy", bufs=3)
                for d in range(3):
                    yb = yv_all[:, ts0:ts0 + sw, d].unsqueeze(-1).broadcast_to(
                        [128, sw, 128])
                    nc.vector.tensor_tensor(Sy[:, :sw, d, :], S[:, :sw, :], yb,
                                            op=OP.mult)
                A1 = psA.tile([128, C4], f32, tag="A1")
                for t in range(sw):
                    nc.tensor.matmul(A1[:, 0:C], S[:, t, :], uv[:, t, 0, :],
                                     start=(t == 0), stop=(t == sw - 1),
                                     skip_group_check=True)
                for d in range(3):
                    for t in range(sw):
                        nc.tensor.matmul(A1[:, C * (1 + d):C * (2 + d)],
                                         Sy[:, t, d, :], uv[:, t, 1, :],
                                         start=(t == 0), stop=(t == sw - 1),
                                         skip_group_check=True)
                nc.sync.dma_start(sdram[:, ts0:ts0 + sw, :], S[:, :sw, :])
                nc.sync.dma_start(sydram[:, ts0:ts0 + sw, :, :],
                                  Sy[:, :sw, :, :])
                if si == 0:
                    nc.scalar.copy(acc1[:], A1[:])
                else:
                    nc.vector.tensor_tensor(acc1[:], A1[:], acc1[:], op=OP.add)
            nc.vector.tensor_copy(a1sb[:, b, :], acc1[:])

        for p in (psE, psL, psM, psA, p1c, p1):
            p.release()
        if debug:
            nc.gpsimd.dma_start(dbg["dbg_a1"][:], a1sb[:])
            nc.gpsimd.dma_start(dbg["dbg_ft"][:], featsd[:])

        # ---------------- phase 2: node layer 1 -----------------------
        n1 = tc.alloc_tile_pool(name="n1", bufs=1)
        n1t = tc.alloc_tile_pool(name="n1t", bufs=2)
        n1m = tc.alloc_tile_pool(name="n1m", bufs=1)
        psN = tc.alloc_tile_pool(name="psN", bufs=2, space="PSUM")

        def transpose_blocks(src, c0, name, pool, pspool):
            dst = pool.tile([C, NLOC], bf16, name=name, tag=name)
            for b in range(NBLK):
                tp = pspool.tile([128, 128], bf16, tag="tp")
                nc.tensor.transpose(tp[:], src[:, b, c0:c0 + C], ident[:])
                nc.scalar.copy(dst[:, b * 128:(b + 1) * 128], tp[:])
            return dst

        def mm_wide(name, lhsT, rhs_tile, pool, pspool, dtype=bf16):
            out = pool.tile([C, NLOC], dtype, name=name, tag=name)
            for o, w_ in nchunks:
                pm = pspool.tile([C, 512], f32, tag="mmw")
                nc.tensor.matmul(pm[:, :w_], lhsT, rhs_tile[:, o:o + w_],
                                 start=True, stop=True)
                nc.scalar.copy(out[:, o:o + w_], pm[:, :w_])
            return out

        AsT = transpose_blocks(a1sb[:], 0, "AsT", n1m, psN)
        AvT = [transpose_blocks(a1sb[:], C * (1 + d), f"AvT{d}", n1m, psN)
               for d in range(3)]
        AsL = mm_wide("AsL", wb["Wlin1_s"][:], AsT[:], n1m, psN)
        AvL = [mm_wide(f"AvL{d}", wb["Wlin1_v"][:], AvT[d][:], n1m, psN)
               for d in range(3)]
        Pw = {n: mm_wide("w" + n, wb[n][:], naTs[:], n1m, psN)
              for n in ["P1s1", "P1ss", "P1vv", "P1v1", "P1sv"]}
        sqs = n1m.tile([C, NLOC], bf16, name="sqs", tag="sqs")
        nc.scalar.square(sqs[:], AsL[:])
        vv = n1m.tile([C, NLOC], f32, name="vv", tag="vv")
        sqv = n1m.tile([C, NLOC], f32, name="sqv", tag="sqv")
        nc.scalar.square(vv[:], AvL[0][:])
        for d in (1, 2):
            nc.scalar.square(sqv[:], AvL[d][:])
            nc.vector.tensor_tensor(vv[:], vv[:], sqv[:], op=OP.add)
        Bs = n1m.tile([C, NLOC], bf16, name="Bs", tag="Bs")
        t0 = n1t.tile([C, NLOC], bf16, tag="t0")
        nc.vector.tensor_tensor(Bs[:], Pw["P1s1"][:], AsL[:], op=OP.mult)
        nc.vector.tensor_tensor(t0[:], Pw["P1ss"][:], sqs[:], op=OP.mult)
        nc.vector.tensor_tensor(Bs[:], Bs[:], t0[:], op=OP.add)
        t1 = n1t.tile([C, NLOC], bf16, tag="t0")
        nc.vector.tensor_tensor(t1[:], Pw["P1vv"][:], vv[:], op=OP.mult)
        nc.vector.tensor_tensor(Bs[:], Bs[:], t1[:], op=OP.add)
        gsk = n1m.tile([C, NLOC], bf16, name="gsk", tag="gsk")
        nc.vector.tensor_tensor(gsk[:], Pw["P1sv"][:], AsL[:], op=OP.mult)
        nc.vector.tensor_tensor(gsk[:], gsk[:], Pw["P1v1"][:], op=OP.add)
        Bv = []
        for d in range(3):
            bvd = n1m.tile([C, NLOC], bf16, name=f"Bv{d}", tag=f"Bv{d}")
            nc.vector.tensor_tensor(bvd[:], gsk[:], AvL[d][:], op=OP.mult)
            Bv.append(bvd)
        hvT = [mm_wide(f"hvT{d}", wb["Lp1_v"][:], Bv[d][:], n1, psN)
               for d in range(3)]
        tabsb = n1m.tile([128, NBLK, C4], bf16, name="tabsb", tag="tabsb")
        for b in range(NBLK):
            pm = psN.tile([128, C], f32, tag="tab")
            nc.tensor.matmul(pm[:], Bs[:, b * 128:(b + 1) * 128], combS[:],
                             start=True, stop=True)
            nc.scalar.copy(tabsb[:, b, 0:C], pm[:])
            for d in range(3):
                pm2 = psN.tile([128, C], f32, tag="tab")
                nc.tensor.matmul(pm2[:], Bv[d][:, b * 128:(b + 1) * 128],
                                 combV[:], start=True, stop=True)
                nc.scalar.copy(tabsb[:, b, C * (1 + d):C * (2 + d)], pm2[:])
        if debug:
            nc.gpsimd.dma_start(dbg["dbg_tab"][:], tabsb[:])
        bounce = dramp.tile([NLOC, C4], bf16, tag="bounce")
        nc.sync.dma_start(bounce[:].rearrange("(b p) c -> p b c", p=128),
                          tabsb[:])
        tabdram = dramp.tile([NCORES * NLOC, C4], bf16, addr_space="Shared",
                             tag="tabdram")
        nc.gpsimd.collective_compute(
            "AllGather", OP.bypass, replica_groups=[list(range(NCORES))],
            ins=[bounce[:]], outs=[tabdram[:]])
        psN.release()
        n1m.release()

        # ---- skip-TP (depends only on hvT): overlaps with phase 4 ----
        sc_all = n1.tile([128, NBLK, 3, C], bf16, name="sc_all", tag="sc_all")
        na_sb = n1.tile([128, NBLK, NE], f32, name="na_sb", tag="na_sb")
        nc.sync.dma_start(na_sb[:], na_in[:].rearrange("(b p) e -> p b e", p=128))
        n5a = tc.alloc_tile_pool(name="n5a", bufs=2)
        ps5a = tc.alloc_tile_pool(name="ps5a", bufs=1, space="PSUM")
        for b in range(NBLK):
            sc = n5a.tile([128, 3, C], f32, tag="sc5")
            gp = ps5a.tile([128, 3, C], f32, tag="gp5")
            for s in range(NE):
                for d in range(3):
                    nc.tensor.matmul(gp[:, d, :],
                                     hvT[d][:, b * 128:(b + 1) * 128],
                                     wsk[:, s, :], start=True, stop=True)
                if s == 0:
                    nc.vector.tensor_scalar(sc[:], gp[:], na_sb[:, b, 0:1],
                                            None, op0=OP.mult)
                else:
                    nc.vector.scalar_tensor_tensor(
                        sc[:], gp[:], na_sb[:, b, s:s + 1], sc[:],
                        op0=OP.mult, op1=OP.add)
            nc.vector.tensor_copy(sc_all[:, b, :, :], sc[:])

        # ---------------- phase 4: edges, layer 2 ---------------------
        p4 = tc.alloc_tile_pool(name="p4", bufs=2)
        p4c = tc.alloc_tile_pool(name="p4c", bufs=4)
        psA2 = tc.alloc_tile_pool(name="psA2", bufs=2, space="PSUM")
        psM2 = tc.alloc_tile_pool(name="psM2", bufs=2, space="PSUM")
        psL2 = tc.alloc_tile_pool(name="psL2", bufs=3, space="PSUM")

        w2dram = dramp.tile([128, tt, 4 * C], bf16, tag="w2dram")
        # 4a: radial MLP2 for all subs (no dependency on the AllGather)
        for b in range(NBLK):
            for si, (s0, sw) in enumerate(subs):
                ebs = sw * 128
                ts0 = b * t_blk + s0
                e0 = b * eb + s0 * 128
                featsT2 = load_feats(p4, e0, ebs, "featsT2")
                cinfo = run_mlp(featsT2, ["Wr2_1", "Wr2_2", "Wr2_3"], psM2,
                                p4c, "m2", ebs)
                w2sb = p4.tile([128, TSUB, 4 * C], bf16, tag="w2sb", bufs=2)
                for t in range(sw):
                    o = t * 128
                    a3, hi, o_, _ = find_chunk(cinfo, o)
                    lo = o - o_
                    w2p = psL2.tile([128, 4 * C], f32, tag="w2p")
                    nc.tensor.matmul(w2p[:],
                                     a3[hi * 64:(hi + 1) * 64, lo:lo + 128],
                                     w2o[hi * 64:(hi + 1) * 64, :],
                                     start=True, stop=True)
                    if t % 4 == 3:
                        nc.vector.tensor_copy(w2sb[:, t, :], w2p[:])
                    else:
                        nc.scalar.copy(w2sb[:, t, :], w2p[:])
                nc.sync.dma_start(w2dram[:, ts0:ts0 + sw, :], w2sb[:, :sw, :])

        # 4b: gather + messages + scatter
        for b in range(NBLK):
            acc2 = p4.tile([128, C4], f32, tag="acc2", bufs=2)
            for si, (s0, sw) in enumerate(subs):
                ebs = sw * 128
                ts0 = b * t_blk + s0
                e0 = b * eb + s0 * 128
                gat = p4.tile([128, TSUB, C4], bf16, tag="gat")
                nc.gpsimd.dma_gather(
                    gat[:, :sw, :], tabdram[:],
                    sndi[:, e0 // 16:(e0 + ebs) // 16],
                    num_idxs=ebs, num_idxs_reg=ebs, elem_size=C4)
                gat4 = gat[:].rearrange("p t (x c) -> p t x c", c=C)
                w2sb = p4.tile([128, TSUB, 4 * C], bf16, tag="w2l", bufs=3)
                nc.sync.dma_start(w2sb[:, :sw, :], w2dram[:, ts0:ts0 + sw, :])
                S = p4.tile([128, TSUB, 128], bf16, tag="S4", bufs=3)
                nc.gpsimd.dma_start(S[:, :sw, :], sdram[:, ts0:ts0 + sw, :])
                Sy = p4.tile([128, TSUB, 3, 128], bf16, tag="Sy4", bufs=3)
                nc.gpsimd.dma_start(Sy[:, :sw, :, :],
                                    sydram[:, ts0:ts0 + sw, :, :])
                m4 = p4.tile([128, TSUB, 4, C], bf16, tag="m4")
                m5 = p4.tile([128, TSUB, 4, C], bf16, tag="m5")
                hsg = gat4[:, :sw, 0, :]
                nc.vector.tensor_tensor(m4[:, :sw, 0, :], w2sb[:, :sw, 0:C],
                                        hsg, op=OP.mult)
                nc.vector.tensor_tensor(m5[:, :sw, 3, :],
                                        w2sb[:, :sw, 2 * C:3 * C], hsg,
                                        op=OP.mult)
                for d in range(3):
                    hvg = gat4[:, :sw, 1 + d, :]
                    nc.vector.tensor_tensor(m4[:, :sw, 1 + d, :],
                                            w2sb[:, :sw, 3 * C:4 * C], hvg,
                                            op=OP.mult)
                    nc.vector.tensor_tensor(m5[:, :sw, d, :],
                                            w2sb[:, :sw, C:2 * C], hvg,
                                            op=OP.mult)
                A2 = psA2.tile([128, 4, C], f32, tag="A2")
                for t in range(sw):
                    nc.tensor.matmul(A2[:], S[:, t, :],
                                     m4[:, t, :, :], start=(t == 0), stop=False,
                                     skip_group_check=True)
                for d in range(3):
                    for t in range(sw):
                        nc.tensor.matmul(A2[:, 1 + d, :], Sy[:, t, d, :],
                                         m5[:, t, 3, :], start=False,
                                         stop=False, skip_group_check=True)
                for d in range(3):
                    for t in range(sw):
                        sp = (t == sw - 1) and (d == 2)
                        nc.tensor.matmul(A2[:, 0, :], Sy[:, t, d, :],
                                         m5[:, t, d, :], start=False, stop=sp,
                                         skip_group_check=True)
                a2f = A2[:].rearrange("p x c -> p (x c)")
                if si == 0:
                    nc.scalar.copy(acc2[:], a2f)
                else:
                    nc.vector.tensor_tensor(acc2[:], a2f, acc2[:], op=OP.add)
            nc.vector.tensor_copy(a2sb[:, b, :], acc2[:])
            if debug and b == 0:
                nc.gpsimd.dma_start(dbg["dbg_gat"][:, 0:min(TSUB, t_blk)],
                                    gat[:, 0:min(TSUB, t_blk)])
                nc.gpsimd.dma_start(dbg["dbg_m4"][:, 0:min(TSUB, t_blk)],
                                    m4[:, 0:min(TSUB, t_blk)])

        for p in (psL2, psM2, psA2, p4c, p4, ps5a, n5a):
            p.release()
        if debug:
            nc.gpsimd.dma_start(dbg["dbg_a2"][:], a2sb[:])

        # ---------------- phase 5: node layer 2 + outputs -------------
        n2 = tc.alloc_tile_pool(name="n2", bufs=1)
        psN2 = tc.alloc_tile_pool(name="psN2", bufs=2, space="PSUM")
        A2sT = transpose_blocks(a2sb[:], 0, "A2sT", n2, psN2)
        A2vT = [transpose_blocks(a2sb[:], C * (1 + d), f"A2vT{d}", n2, psN2)
                for d in range(3)]
        A2sL = mm_wide("A2sL", wb["Wlin2_s"][:], A2sT[:], n2, psN2)
        A2vL = [mm_wide(f"A2vL{d}", wb["Wlin2_v"][:], A2vT[d][:], n2, psN2)
                for d in range(3)]
        P2w = {n: mm_wide("w" + n, wb[n][:], naTs[:], n2, psN2)
               for n in ["P2v1", "P2sv"]}
        g2 = n2.tile([C, NLOC], bf16, name="g2", tag="g2")
        nc.vector.tensor_tensor(g2[:], P2w["P2sv"][:], A2sL[:], op=OP.mult)
        nc.vector.tensor_tensor(g2[:], g2[:], P2w["P2v1"][:], op=OP.add)
        B2v = []
        for d in range(3):
            b2d = n2.tile([C, NLOC], bf16, name=f"B2v{d}", tag=f"B2v{d}")
            nc.vector.tensor_tensor(b2d[:], g2[:], A2vL[d][:], op=OP.mult)
            B2v.append(b2d)
        psN2.release()

        # h2 (node-major) = B2v @ Lp2_v + skip-TP, then back to feat-major
        n2t = tc.alloc_tile_pool(name="n2t", bufs=2)
        psH = tc.alloc_tile_pool(name="psH", bufs=1, space="PSUM")
        psHt = tc.alloc_tile_pool(name="psHt", bufs=2, space="PSUM")
        h2T = [n2.tile([C, NLOC], bf16, name=f"h2T{d}", tag=f"h2T{d}")
               for d in range(3)]
        for b in range(NBLK):
            hp = psH.tile([128, 3, C], f32, tag="h2p")
            for d in range(3):
                nc.tensor.matmul(hp[:, d, :], B2v[d][:, b * 128:(b + 1) * 128],
                                 wb["Lp2_v"][:], start=True, stop=True)
            sc16 = n2t.tile([128, 3, C], bf16, tag="sc16")
            nc.vector.tensor_tensor(sc16[:], hp[:], sc_all[:, b, :, :],
                                    op=OP.add)
            for d in range(3):
                tp = psHt.tile([128, 128], bf16, tag="tph")
                nc.tensor.transpose(tp[:], sc16[:, d, :], ident[:])
                nc.scalar.copy(h2T[d][:, b * 128:(b + 1) * 128], tp[:])
        psHt.release()
        psH.release()

        psD = tc.alloc_tile_pool(name="psD", bufs=2, space="PSUM")
        dipf = [n2.tile([1, NLOC], f32, name=f"dipf{d}", tag=f"dipf{d}")
                for d in range(3)]
        for d in range(3):
            for o, w_ in nchunks:
                dp = psD.tile([1, 512], f32, tag="dp")
                nc.tensor.matmul(dp[:, :w_], wb["R1c"][:], hvT[d][:, o:o + w_],
                                 start=True, stop=False)
                nc.tensor.matmul(dp[:, :w_], qcol[:], h2T[d][:, o:o + w_],
                                 start=False, stop=True)
                nc.scalar.copy(dipf[d][0:1, o:o + w_], dp[:, :w_])

        posb = n2.tile([128, NBLK, 3], f32, name="posb", tag="posb")
        nc.sync.dma_start(posb[:], posn_in[:].rearrange("(b p) c -> p b c", p=128))
        chgb = n2.tile([128, NBLK], f32, name="chgb", tag="chgb")
        nc.sync.dma_start(chgb[:], chg_in[:].rearrange("(b p) c -> p (b c)", p=128))
        bohb = n2.tile([128, NBLK, G], f32, name="bohb", tag="bohb")
        nc.sync.dma_start(bohb[:], boh_in[:].rearrange("(b p) g -> p b g", p=128))
        dipo = n2.tile([128, NBLK, 3], f32, name="dipo", tag="dipo")
        gs = psD.tile([G, 3], f32, tag="gs", bufs=1)
        for b in range(NBLK):
            for d in range(3):
                dpp = psD.tile([128, 1], f32, tag="dpp")
                nc.tensor.transpose(dpp[:], dipf[d][0:1, b * 128:(b + 1) * 128],
                                    identf[0:1, 0:1])
                nc.scalar.copy(dipo[:, b, d:d + 1], dpp[:])
            cp = n2t.tile([128, 3], f32, tag="cp")
            nc.vector.tensor_scalar(cp[:], posb[:, b, :], chgb[:, b:b + 1],
                                    None, op0=OP.mult)
            nc.vector.tensor_tensor(cp[:], cp[:], dipo[:, b, :], op=OP.add)
            nc.tensor.matmul(gs[:], bohb[:, b, :], cp[:], start=(b == 0),
                             stop=(b == NBLK - 1))
        nc.sync.dma_start(out_dip[:].rearrange("(b p) c -> p b c", p=128),
                          dipo[:])
        if debug:
            nc.gpsimd.dma_start(dbg["dbg_hvT0"][:], hvT[0][:])
            nc.gpsimd.dma_start(dbg["dbg_Bs"][:], Bs[:])
            nc.gpsimd.dma_start(dbg["dbg_h2T0"][:], h2T[0][:])
            nc.gpsimd.dma_start(dbg["dbg_dipf0"][:], dipf[0][:])
        gso = n2.tile([G, 3], f32, name="gso", tag="gso")
        nc.scalar.copy(gso[:], gs[:])
        nc.sync.dma_start(out_gs[:], gso[:])

        for p in (psD, n2t, n2, n1t, n1):
            p.release()
        dramp.release()
        wp.release()

    nc.compile()
    return nc


_BUILD_CACHE = {}


def kernel(**inputs):
    in_maps, t_blk = prep_shards(inputs)
    nc = _BUILD_CACHE.get(t_blk)
    if nc is None:
        nc = build(t_blk)
        _BUILD_CACHE[t_blk] = nc
    res = run_bass_kernel_spmd(nc, in_maps, core_ids=list(range(NCORES)))
    dip = np.zeros((N, 3), np.float32)
    tot = np.zeros((G, 3), np.float32)
    for k in range(NCORES):
        dip[k * NPC:(k + 1) * NPC] = res.results[k]["dip"][:NPC]
        tot += res.results[k]["gsum"]
    return tot, dip
